# revision 1
# baseline (speedup 1.0000x reference)
"""Trainium2 Bass kernel for nn_Biholomorphic_k2.

Per row (N=1e6 rows, D=5):
  z = x_re + i*x_im                                  [5] complex
  zz = z[i5] * z[j5]          (triu pairs of 5)      [15] complex
  prod = zz[i15] * conj(zz[j15])  (triu pairs of 15) [120] complex
  out = [Re(prod) (120 cols), Im(prod offdiag) (105 cols)]  [225] f32

Sharding: pure data parallel over 8 cores; rows on SBUF partitions in
contiguous blocks (partition p of core c owns rows [c*NC + p*RT, ...)),
features along the free dim so every DMA is fully contiguous.

All compute is VectorE tensor_tensor ops with stride-0 (broadcast) access
patterns on the "a"-side operand; results are written directly into the
packed [128, R*225] output tile, so there is no separate gather step.
"""

import sys

import numpy as np

try:
    import concourse.bass as bass
except ImportError:
    for _p in ("/opt/trn_rl_repo", "/root/.axon_site/_ro/trn_rl_repo"):
        if _p not in sys.path:
            sys.path.insert(0, _p)
    import concourse.bass as bass
import concourse.mybir as mybir
from concourse.tile import TileContext
from concourse import bass_utils

P = 128          # SBUF partitions
D = 5
K = 15           # triu pairs of 5
NOUT = 225       # 120 re + 105 im
N_CORES = 8

# --- static index tables (row-major triu, matching np.triu_indices) ---
# step-1: for a in 0..4 produce zz[o1[a] : o1[a]+5-a] = z[a] * z[a:5]
O1 = [0, 5, 9, 12, 14]
# step-2 re: for a in 0..14, out[ro[a] : ro[a]+15-a] = Re(zz[a] * conj(zz[a:15]))
RO = np.concatenate([[0], np.cumsum([15 - a for a in range(15)])]).astype(int)
# step-2 im: for a in 0..13, out[120+io[a] : ...+14-a] = Im(zz[a] * conj(zz[a+1:15]))
IO = 120 + np.concatenate([[0], np.cumsum([14 - a for a in range(14)])]).astype(int)

F32 = mybir.dt.float32
F16 = mybir.dt.float16

# "fp16": k-major fp16 compute (2x DVE mode), ~7e-4 rel error.
# "fp32": row-major fp32 compute, ~6e-8 rel error, ~2x slower.
PRECISION = "fp16"
# route the step-2 im combines to GPSIMD (overlaps with DVE; shares one of
# DVE's two SBUF ports -- cost model says net win)
GPSIMD_IM = True
# additionally route re combines for these pair-start values to GPSIMD
GP_RE_PAIRS = ()

_MAX_CTRL_WAITS = 1


def _split_excess_waits(nc):
    """Workaround: this walrus build rejects Drain instructions carrying
    more than one sync wait ("Too many sync wait commands").  Move excess
    waits onto NOPs inserted immediately before, on the same engine."""
    engmap = {
        mybir.EngineType.SP: nc.sync,
        mybir.EngineType.DVE: nc.vector,
        mybir.EngineType.Activation: nc.scalar,
        mybir.EngineType.PE: nc.tensor,
        mybir.EngineType.Pool: nc.gpsimd,
    }
    for f in nc.m.functions:
        for blk in f.blocks:
            newlist = []
            for inst in blk.instructions:
                si = inst.sync_info
                if (
                    si is not None
                    and si.on_wait is not None
                    and len(si.on_wait) > _MAX_CTRL_WAITS
                ):
                    waits = list(si.on_wait)
                    head = waits[:-_MAX_CTRL_WAITS]
                    tail = waits[-_MAX_CTRL_WAITS:]
                    for s in range(0, len(head), _MAX_CTRL_WAITS):
                        chunk = head[s : s + _MAX_CTRL_WAITS]
                        bi = engmap[inst.engine].nop()
                        nop_inst = bi.ins if hasattr(bi, "ins") else bi
                        for b2 in f.blocks:
                            if nop_inst in b2.instructions:
                                b2.instructions.remove(nop_inst)
                        nop_inst.sync_info = mybir.SyncInfo(on_wait=chunk, on_update=[])
                        nop_inst.engine = inst.engine
                        newlist.append(nop_inst)
                    inst.sync_info = mybir.SyncInfo(
                        on_wait=tail, on_update=list(si.on_update or [])
                    )
                newlist.append(inst)
            blk.instructions[:] = newlist


def _build(n_c, rt, tile_rs):
    """Build the Bass program for one core's shard: [n_c, 5] x2 -> [n_c, 225].
    n_c = P * rt rows; processed in free-dim chunks of R rows/partition."""
    nc = bass.Bass()
    xr_d = nc.dram_tensor("x_re", [n_c, D], F32, kind="ExternalInput")
    xi_d = nc.dram_tensor("x_im", [n_c, D], F32, kind="ExternalInput")
    out_d = nc.dram_tensor("out", [n_c, NOUT], F32, kind="ExternalOutput")

    xr_v = xr_d[:, :].rearrange("(p r) d -> p r d", p=P)     # [128, rt, 5]
    xi_v = xi_d[:, :].rearrange("(p r) d -> p r d", p=P)
    out_v = out_d[:, :].rearrange("(p r) c -> p r c", p=P)   # [128, rt, 225]

    with TileContext(nc) as tc:
        with (
            tc.tile_pool(name="io", bufs=3) as iop,
            tc.tile_pool(name="zz", bufs=2) as zzp,
            tc.tile_pool(name="scr", bufs=2) as scp,
            tc.tile_pool(name="outp", bufs=2) as outp,
        ):
            r0 = 0
            for R in tile_rs:
                xr_t = iop.tile([P, R * D], F32, tag="xr")
                xi_t = iop.tile([P, R * D], F32, tag="xi")
                nc.sync.dma_start(
                    out=xr_t[:].rearrange("p (r d) -> p r d", d=D),
                    in_=xr_v[:, r0 : r0 + R, :],
                )
                nc.sync.dma_start(
                    out=xi_t[:].rearrange("p (r d) -> p r d", d=D),
                    in_=xi_v[:, r0 : r0 + R, :],
                )
                xr3 = xr_t[:].rearrange("p (r d) -> p r d", d=D)   # [128,R,5]
                xi3 = xi_t[:].rearrange("p (r d) -> p r d", d=D)

                ar_t = zzp.tile([P, R * K], F32, tag="ar")
                ai_t = zzp.tile([P, R * K], F32, tag="ai")
                ar3 = ar_t[:].rearrange("p (r k) -> p r k", k=K)   # [128,R,15]
                ai3 = ai_t[:].rearrange("p (r k) -> p r k", k=K)

                # ---- step 1: zz = z[a] * z[a:5] for a in 0..4 ----
                for a in range(D):
                    L = D - a
                    o = O1[a]
                    s1 = scp.tile([P, R * D], F32, tag="s1")
                    s2 = scp.tile([P, R * D], F32, tag="s2")
                    s1v = s1[:].rearrange("p (r d) -> p r d", d=D)[:, :, :L]
                    s2v = s2[:].rearrange("p (r d) -> p r d", d=D)[:, :, :L]
                    bra = xr3[:, :, a : a + 1].broadcast_to([P, R, L])
                    bia = xi3[:, :, a : a + 1].broadcast_to([P, R, L])
                    # re: xr_a*xr_b - xi_a*xi_b
                    nc.vector.tensor_mul(out=s1v, in0=bra, in1=xr3[:, :, a:D])
                    nc.vector.tensor_mul(out=s2v, in0=bia, in1=xi3[:, :, a:D])
                    nc.vector.tensor_sub(
                        out=ar3[:, :, o : o + L], in0=s1v, in1=s2v
                    )
                    # im: xr_a*xi_b + xi_a*xr_b
                    s3 = scp.tile([P, R * D], F32, tag="s3")
                    s4 = scp.tile([P, R * D], F32, tag="s4")
                    s3v = s3[:].rearrange("p (r d) -> p r d", d=D)[:, :, :L]
                    s4v = s4[:].rearrange("p (r d) -> p r d", d=D)[:, :, :L]
                    nc.vector.tensor_mul(out=s3v, in0=bra, in1=xi3[:, :, a:D])
                    nc.vector.tensor_mul(out=s4v, in0=bia, in1=xr3[:, :, a:D])
                    nc.vector.tensor_add(
                        out=ai3[:, :, o : o + L], in0=s3v, in1=s4v
                    )

                out_t = outp.tile([P, R * NOUT], F32, tag="out")
                out3 = out_t[:].rearrange("p (r c) -> p r c", c=NOUT)

                # ---- step 2: prod = zz[a] * conj(zz[b]), b >= a ----
                for a in range(K):
                    L = K - a
                    bar = ar3[:, :, a : a + 1].broadcast_to([P, R, L])
                    bai = ai3[:, :, a : a + 1].broadcast_to([P, R, L])
                    m1 = scp.tile([P, R * K], F32, tag="m1")
                    m2 = scp.tile([P, R * K], F32, tag="m2")
                    m1v = m1[:].rearrange("p (r k) -> p r k", k=K)[:, :, :L]
                    m2v = m2[:].rearrange("p (r k) -> p r k", k=K)[:, :, :L]
                    # re: ar_a*ar_b + ai_a*ai_b
                    nc.vector.tensor_mul(out=m1v, in0=bar, in1=ar3[:, :, a:K])
                    nc.vector.tensor_mul(out=m2v, in0=bai, in1=ai3[:, :, a:K])
                    ro = int(RO[a])
                    nc.vector.tensor_add(
                        out=out3[:, :, ro : ro + L], in0=m1v, in1=m2v
                    )
                    # im (offdiag only): ai_a*ar_b - ar_a*ai_b
                    if a < K - 1:
                        L2 = L - 1
                        m3 = scp.tile([P, R * K], F32, tag="m3")
                        m4 = scp.tile([P, R * K], F32, tag="m4")
                        m3v = m3[:].rearrange("p (r k) -> p r k", k=K)[:, :, :L2]
                        m4v = m4[:].rearrange("p (r k) -> p r k", k=K)[:, :, :L2]
                        bar2 = ar3[:, :, a : a + 1].broadcast_to([P, R, L2])
                        bai2 = ai3[:, :, a : a + 1].broadcast_to([P, R, L2])
                        nc.vector.tensor_mul(
                            out=m3v, in0=bai2, in1=ar3[:, :, a + 1 : K]
                        )
                        nc.vector.tensor_mul(
                            out=m4v, in0=bar2, in1=ai3[:, :, a + 1 : K]
                        )
                        io = int(IO[a])
                        nc.vector.tensor_sub(
                            out=out3[:, :, io : io + L2], in0=m3v, in1=m4v
                        )

                nc.sync.dma_start(out=out_v[:, r0 : r0 + R, :], in_=out3)
                r0 += R

    _split_excess_waits(nc)
    return nc


def _build_fp16(n_c, rt, tile_rs):
    """k-major fp16 build: within each partition, every tensor is stored
    feature-major ([k, r] with r innermost, step 1) so all DVE tensor_tensor
    operands have a 2-byte dtype, innermost step 1, and 4B-aligned run starts
    (R even) -> 2x_1p DVE mode throughout.  ScalarE does the fp32->fp16
    transpose-in and the fp16->fp32 transpose-out (its own SBUF ports, 1x).
    """
    KP = 16  # zz column padding (alignment headroom)
    nc = bass.Bass()
    xr_d = nc.dram_tensor("x_re", [n_c, D], F32, kind="ExternalInput")
    xi_d = nc.dram_tensor("x_im", [n_c, D], F32, kind="ExternalInput")
    out_d = nc.dram_tensor("out", [n_c, NOUT], F32, kind="ExternalOutput")

    xr_v = xr_d[:, :].rearrange("(p r) d -> p r d", p=P)     # [128, rt, 5]
    xi_v = xi_d[:, :].rearrange("(p r) d -> p r d", p=P)
    out_v = out_d[:, :].rearrange("(p r) c -> p r c", p=P)   # [128, rt, 225]

    with TileContext(nc) as tc:
        with (
            tc.tile_pool(name="io", bufs=3) as iop,
            tc.tile_pool(name="km", bufs=2) as kmp,
            tc.tile_pool(name="zz", bufs=2) as zzp,
            tc.tile_pool(name="scr", bufs=2) as scp,
            tc.tile_pool(name="outk", bufs=2) as okp,
            tc.tile_pool(name="outf", bufs=2) as ofp,
        ):
            r0 = 0
            for R in tile_rs:
                assert R % 2 == 0
                R2 = R // 2
                xr_s = iop.tile([P, R * D], F32, tag="xr")
                xi_s = iop.tile([P, R * D], F32, tag="xi")
                nc.sync.dma_start(
                    out=xr_s[:].rearrange("p (r d) -> p r d", d=D),
                    in_=xr_v[:, r0 : r0 + R, :],
                )
                nc.sync.dma_start(
                    out=xi_s[:].rearrange("p (r d) -> p r d", d=D),
                    in_=xi_v[:, r0 : r0 + R, :],
                )
                # ScalarE: cast fp32->fp16 + transpose row-major -> k-major
                xr_k = kmp.tile([P, D * R], F16, tag="xrk")
                xi_k = kmp.tile([P, D * R], F16, tag="xik")
                # src [r, d] -> view [d, r]
                nc.scalar.copy(
                    out=xr_k[:].rearrange("p (d r) -> p d r", d=D),
                    in_=xr_s[:].rearrange("p (r d) -> p r d", d=D).transpose([0, 2, 1]),
                )
                nc.scalar.copy(
                    out=xi_k[:].rearrange("p (d r) -> p d r", d=D),
                    in_=xi_s[:].rearrange("p (r d) -> p r d", d=D).transpose([0, 2, 1]),
                )
                xr3 = xr_k[:].rearrange("p (d r) -> p d r", d=D)   # [128,5,R]
                xi3 = xi_k[:].rearrange("p (d r) -> p d r", d=D)

                ar_t = zzp.tile([P, KP * R], F16, tag="ar")
                ai_t = zzp.tile([P, KP * R], F16, tag="ai")
                ar3 = ar_t[:].rearrange("p (k r) -> p k r", k=KP)  # [128,16,R]
                ai3 = ai_t[:].rearrange("p (k r) -> p k r", k=KP)

                # ---- step 1: zz[o1[a]:o1[a]+L] = z[a] * z[a:5] ----
                for a in range(D):
                    L = D - a
                    o = O1[a]
                    bra = xr3[:, a : a + 1, :].broadcast_to([P, L, R])
                    bia = xi3[:, a : a + 1, :].broadcast_to([P, L, R])
                    s1 = scp.tile([P, D * R], F16, tag="s1")
                    s2 = scp.tile([P, D * R], F16, tag="s2")
                    s1v = s1[:].rearrange("p (k r) -> p k r", k=D)[:, :L, :]
                    s2v = s2[:].rearrange("p (k r) -> p k r", k=D)[:, :L, :]
                    nc.vector.tensor_mul(out=s1v, in0=bra, in1=xr3[:, a:D, :])
                    nc.vector.tensor_mul(out=s2v, in0=bia, in1=xi3[:, a:D, :])
                    nc.vector.tensor_sub(out=ar3[:, o : o + L, :], in0=s1v, in1=s2v)
                    s3 = scp.tile([P, D * R], F16, tag="s3")
                    s4 = scp.tile([P, D * R], F16, tag="s4")
                    s3v = s3[:].rearrange("p (k r) -> p k r", k=D)[:, :L, :]
                    s4v = s4[:].rearrange("p (k r) -> p k r", k=D)[:, :L, :]
                    nc.vector.tensor_mul(out=s3v, in0=bra, in1=xi3[:, a:D, :])
                    nc.vector.tensor_mul(out=s4v, in0=bia, in1=xr3[:, a:D, :])
                    nc.vector.tensor_add(out=ai3[:, o : o + L, :], in0=s3v, in1=s4v)

                out_k = okp.tile([P, NOUT * R], F16, tag="outk")
                ok3 = out_k[:].rearrange("p (c r) -> p c r", c=NOUT)  # [128,225,R]

                # ---- step 2 ----
                for a in range(K):
                    L = K - a
                    bar = ar3[:, a : a + 1, :].broadcast_to([P, L, R])
                    bai = ai3[:, a : a + 1, :].broadcast_to([P, L, R])
                    m1 = scp.tile([P, KP * R], F16, tag="m1")
                    m2 = scp.tile([P, KP * R], F16, tag="m2")
                    m1v = m1[:].rearrange("p (k r) -> p k r", k=KP)[:, :L, :]
                    m2v = m2[:].rearrange("p (k r) -> p k r", k=KP)[:, :L, :]
                    nc.vector.tensor_mul(out=m1v, in0=bar, in1=ar3[:, a:K, :])
                    nc.vector.tensor_mul(out=m2v, in0=bai, in1=ai3[:, a:K, :])
                    ro = int(RO[a])
                    nc.vector.tensor_add(out=ok3[:, ro : ro + L, :], in0=m1v, in1=m2v)
                    if a < K - 1:
                        L2 = L - 1
                        bar2 = ar3[:, a : a + 1, :].broadcast_to([P, L2, R])
                        bai2 = ai3[:, a : a + 1, :].broadcast_to([P, L2, R])
                        m3 = scp.tile([P, KP * R], F16, tag="m3")
                        m4 = scp.tile([P, KP * R], F16, tag="m4")
                        m3v = m3[:].rearrange("p (k r) -> p k r", k=KP)[:, :L2, :]
                        m4v = m4[:].rearrange("p (k r) -> p k r", k=KP)[:, :L2, :]
                        nc.vector.tensor_mul(out=m3v, in0=bai2, in1=ar3[:, a + 1 : K, :])
                        nc.vector.tensor_mul(out=m4v, in0=bar2, in1=ai3[:, a + 1 : K, :])
                        io = int(IO[a])
                        nc.vector.tensor_sub(
                            out=ok3[:, io : io + L2, :], in0=m3v, in1=m4v
                        )

                # ---- ScalarE: fp16 k-major -> fp32 row-major, in row-chunks ----
                n_chunks = 4 if R % 4 == 0 else 2
                Rc = R // n_chunks
                for h in range(n_chunks):
                    of = ofp.tile([P, Rc * NOUT], F32, tag="outf")
                    of3 = of[:].rearrange("p (r c) -> p r c", c=NOUT)  # [128,Rc,225]
                    src = ok3[:, :, h * Rc : (h + 1) * Rc].transpose([0, 2, 1])
                    nc.scalar.copy(out=of3, in_=src)
                    nc.sync.dma_start(
                        out=out_v[:, r0 + h * Rc : r0 + (h + 1) * Rc, :], in_=of3
                    )
                r0 += R

    _split_excess_waits(nc)
    return nc


def _ap4(t2d, col_off, jstep_cols, L, R):
    """4-dim AP over a k-major [cols x R] SBUF tile view: two runs (j=0,1)
    of L columns x R rows, run j starting at column col_off + j*jstep_cols."""
    pdim = list(t2d.ap[0])
    return bass.AP(
        t2d.tensor,
        t2d.offset + col_off * R,
        [pdim, [jstep_cols * R, 2], [R, L], [1, R]],
    )


def _build_fp16_paired(n_c, rt, tile_rs):
    """Like _build_fp16 but batches consecutive-a groups in pairs via 4-dim
    APs, halving DVE instruction count.  The second run of each pair reads one
    padded junk column and writes one column past its end; emission order
    guarantees a later group rewrites the overshoot column with real data."""
    KP = 16       # zz padded to 16 cols (col 15 = junk read by pair overshoot)
    D2 = 6        # input padded to 6 cols (col 5 = junk)
    NP = NOUT + 1 # out_k padded by 1 col for im-pair overshoot
    nc = bass.Bass()
    xr_d = nc.dram_tensor("x_re", [n_c, D], F32, kind="ExternalInput")
    xi_d = nc.dram_tensor("x_im", [n_c, D], F32, kind="ExternalInput")
    out_d = nc.dram_tensor("out", [n_c, NOUT], F32, kind="ExternalOutput")

    xr_v = xr_d[:, :].rearrange("(p r) d -> p r d", p=P)
    xi_v = xi_d[:, :].rearrange("(p r) d -> p r d", p=P)
    out_v = out_d[:, :].rearrange("(p r) c -> p r c", p=P)

    with TileContext(nc) as tc:
        with (
            tc.tile_pool(name="io", bufs=2) as iop,
            tc.tile_pool(name="km", bufs=2) as kmp,
            tc.tile_pool(name="zz", bufs=2) as zzp,
            tc.tile_pool(name="scr", bufs=2) as scp,
            tc.tile_pool(name="outk", bufs=2) as okp,
            tc.tile_pool(name="outf", bufs=4) as ofp,
        ):
            r0 = 0
            for R in tile_rs:
                assert R % 2 == 0
                xr_s = iop.tile([P, R * D], F32, tag="xr")
                xi_s = iop.tile([P, R * D], F32, tag="xi")
                nc.sync.dma_start(
                    out=xr_s[:].rearrange("p (r d) -> p r d", d=D),
                    in_=xr_v[:, r0 : r0 + R, :],
                )
                nc.sync.dma_start(
                    out=xi_s[:].rearrange("p (r d) -> p r d", d=D),
                    in_=xi_v[:, r0 : r0 + R, :],
                )
                xr_k = kmp.tile([P, D2 * R], F16, tag="xrk")
                xi_k = kmp.tile([P, D2 * R], F16, tag="xik")
                nc.scalar.copy(
                    out=xr_k[:].rearrange("p (d r) -> p d r", d=D2)[:, :D, :],
                    in_=xr_s[:].rearrange("p (r d) -> p r d", d=D).transpose([0, 2, 1]),
                )
                nc.scalar.copy(
                    out=xi_k[:].rearrange("p (d r) -> p d r", d=D2)[:, :D, :],
                    in_=xi_s[:].rearrange("p (r d) -> p r d", d=D).transpose([0, 2, 1]),
                )
                xr3 = xr_k[:].rearrange("p (d r) -> p d r", d=D2)   # [128,6,R]
                xi3 = xi_k[:].rearrange("p (d r) -> p d r", d=D2)

                ar_t = zzp.tile([P, KP * R], F16, tag="ar")
                ai_t = zzp.tile([P, KP * R], F16, tag="ai")
                ar3 = ar_t[:].rearrange("p (k r) -> p k r", k=KP)
                ai3 = ai_t[:].rearrange("p (k r) -> p k r", k=KP)

                # ---- step 1 (pairs (0,1),(2,3) then single a=4) ----
                for a in (0, 2):
                    L = D - a
                    b_r = xr3[:, a : a + 2, :].unsqueeze(2).broadcast_to([P, 2, L, R])
                    b_i = xi3[:, a : a + 2, :].unsqueeze(2).broadcast_to([P, 2, L, R])
                    w_r = _ap4(xr_k[:], a, 1, L, R)
                    w_i = _ap4(xi_k[:], a, 1, L, R)
                    s1 = scp.tile([P, 2 * D * R], F16, tag="s1")
                    s2 = scp.tile([P, 2 * D * R], F16, tag="s2")
                    s3 = scp.tile([P, 2 * D * R], F16, tag="s3")
                    s4 = scp.tile([P, 2 * D * R], F16, tag="s4")
                    s1v = s1[:, : 2 * L * R].rearrange("p (j k r) -> p j k r", j=2, k=L)
                    s2v = s2[:, : 2 * L * R].rearrange("p (j k r) -> p j k r", j=2, k=L)
                    s3v = s3[:, : 2 * L * R].rearrange("p (j k r) -> p j k r", j=2, k=L)
                    s4v = s4[:, : 2 * L * R].rearrange("p (j k r) -> p j k r", j=2, k=L)
                    nc.vector.tensor_mul(out=s1v, in0=b_r, in1=w_r)
                    nc.vector.tensor_mul(out=s2v, in0=b_i, in1=w_i)
                    nc.vector.tensor_sub(out=_ap4(ar_t[:], O1[a], L, L, R), in0=s1v, in1=s2v)
                    nc.vector.tensor_mul(out=s3v, in0=b_r, in1=w_i)
                    nc.vector.tensor_mul(out=s4v, in0=b_i, in1=w_r)
                    nc.vector.tensor_add(out=_ap4(ai_t[:], O1[a], L, L, R), in0=s3v, in1=s4v)
                # single a=4 (L=1)
                a = 4
                bra = xr3[:, a : a + 1, :]
                bia = xi3[:, a : a + 1, :]
                s1 = scp.tile([P, 2 * D * R], F16, tag="s1")
                s2 = scp.tile([P, 2 * D * R], F16, tag="s2")
                s1v = s1[:, :R].unsqueeze(1)
                s2v = s2[:, :R].unsqueeze(1)
                nc.vector.tensor_mul(out=s1v, in0=bra, in1=xr3[:, a : a + 1, :])
                nc.vector.tensor_mul(out=s2v, in0=bia, in1=xi3[:, a : a + 1, :])
                nc.vector.tensor_sub(out=ar3[:, 14:15, :], in0=s1v, in1=s2v)
                s3 = scp.tile([P, 2 * D * R], F16, tag="s3")
                s4 = scp.tile([P, 2 * D * R], F16, tag="s4")
                s3v = s3[:, :R].unsqueeze(1)
                s4v = s4[:, :R].unsqueeze(1)
                nc.vector.tensor_mul(out=s3v, in0=bra, in1=xi3[:, a : a + 1, :])
                nc.vector.tensor_mul(out=s4v, in0=bia, in1=xr3[:, a : a + 1, :])
                nc.vector.tensor_add(out=ai3[:, 14:15, :], in0=s3v, in1=s4v)

                out_k = okp.tile([P, NP * R], F16, tag="outk")
                ok3 = out_k[:].rearrange("p (c r) -> p c r", c=NP)

                # Gauss 3-mult: with v = ar+ai, u = ar-ai:
                #   k1 = v_a * ar_b;  k3 = ai_a * u_b;  k2 = ar_a * v_b
                #   re(a,b) = k1 - k3;  im(a,b) = k1 - k2   (b >= a+1 for im)
                v_t = zzp.tile([P, KP * R], F16, tag="vt")
                u_t = zzp.tile([P, KP * R], F16, tag="ut")
                nc.vector.tensor_add(out=v_t[:], in0=ar_t[:], in1=ai_t[:])
                nc.vector.tensor_sub(out=u_t[:], in0=ar_t[:], in1=ai_t[:])
                v3 = v_t[:].rearrange("p (k r) -> p k r", k=KP)

                # ---- step 2: re pairs a=0,2,..,12 + single a=14 ----
                im_eng = nc.gpsimd if GPSIMD_IM else nc.vector
                for a in range(0, K - 1, 2):
                    L = K - a
                    L2 = L - 1
                    b_v = v3[:, a : a + 2, :].unsqueeze(2).broadcast_to([P, 2, L, R])
                    b_ai = ai3[:, a : a + 2, :].unsqueeze(2).broadcast_to([P, 2, L, R])
                    b_ar = ar3[:, a : a + 2, :].unsqueeze(2).broadcast_to([P, 2, L2, R])
                    w_ar = _ap4(ar_t[:], a, 1, L, R)
                    w_u = _ap4(u_t[:], a, 1, L, R)
                    w_v = _ap4(v_t[:], a + 1, 1, L2, R)
                    k1 = scp.tile([P, 2 * K * R], F16, tag="m1")
                    k2 = scp.tile([P, 2 * K * R], F16, tag="m2")
                    k3 = scp.tile([P, 2 * K * R], F16, tag="m3")
                    k1v = k1[:, : 2 * L * R].rearrange("p (j k r) -> p j k r", j=2, k=L)
                    k3v = k3[:, : 2 * L * R].rearrange("p (j k r) -> p j k r", j=2, k=L)
                    k2v = k2[:, : 2 * L2 * R].rearrange("p (j k r) -> p j k r", j=2, k=L2)
                    nc.vector.tensor_mul(out=k1v, in0=b_v, in1=w_ar)
                    nc.vector.tensor_mul(out=k3v, in0=b_ai, in1=w_u)
                    nc.vector.tensor_mul(out=k2v, in0=b_ar, in1=w_v)
                    re_eng = nc.gpsimd if a in GP_RE_PAIRS else nc.vector
                    re_eng.tensor_sub(
                        out=_ap4(out_k[:], int(RO[a]), L, L, R), in0=k1v, in1=k3v
                    )
                    im_eng.tensor_sub(
                        out=_ap4(out_k[:], int(IO[a]), L2, L2, R),
                        in0=k1v[:, :, 1:, :],
                        in1=k2v,
                    )
                # single a=14 re (L=1): pr = ar^2 + ai^2
                m1 = scp.tile([P, 2 * K * R], F16, tag="m1")
                m2 = scp.tile([P, 2 * K * R], F16, tag="m2")
                m1v = m1[:, :R].unsqueeze(1)
                m2v = m2[:, :R].unsqueeze(1)
                nc.vector.tensor_mul(out=m1v, in0=ar3[:, 14:15, :], in1=ar3[:, 14:15, :])
                nc.vector.tensor_mul(out=m2v, in0=ai3[:, 14:15, :], in1=ai3[:, 14:15, :])
                nc.vector.tensor_add(out=ok3[:, 119:120, :], in0=m1v, in1=m2v)

                # ---- ScalarE: fp16 k-major -> fp32 row-major, ~12-row chunks ----
                RC = 8
                h0 = 0
                while h0 < R:
                    rc = min(RC, R - h0)
                    of = ofp.tile([P, RC * NOUT], F32, tag="outf")
                    of3 = of[:, : rc * NOUT].rearrange("p (r c) -> p r c", c=NOUT)
                    src = ok3[:, :NOUT, h0 : h0 + rc].transpose([0, 2, 1])
                    nc.scalar.copy(out=of3, in_=src)
                    nc.sync.dma_start(
                        out=out_v[:, r0 + h0 : r0 + h0 + rc, :], in_=of3
                    )
                    h0 += rc
                r0 += R

    _split_excess_waits(nc)
    return nc


_CACHE = {}


def _get_program(n):
    """Geometry + compiled program for total row count n."""
    key = (n, PRECISION)
    if key in _CACHE:
        return _CACHE[key]
    per_core = -(-n // N_CORES)              # ceil
    rt = -(-per_core // P)                   # rows per partition
    if PRECISION == "fp16":
        rt += rt % 2                         # even rt (fp16 4B alignment needs even R only)
        n_c = P * rt
        r_max = 100                          # divisible by 4; best per cost-model sweep
        tile_rs = [r_max] * (rt // r_max)
        if rt % r_max:
            tile_rs.append(rt % r_max)
        nc = _build_fp16_paired(n_c, rt, tile_rs)
    else:
        n_c = P * rt
        r_max = 64
        tile_rs = [r_max] * (rt // r_max)
        if rt % r_max:
            tile_rs.append(rt % r_max)
        nc = _build(n_c, rt, tile_rs)
    _CACHE[key] = (nc, n_c)
    return _CACHE[key]


def kernel(x_re, x_im, _trace=False):
    x_re = np.ascontiguousarray(np.asarray(x_re), dtype=np.float32)
    x_im = np.ascontiguousarray(np.asarray(x_im), dtype=np.float32)
    n = x_re.shape[0]
    nc, n_c = _get_program(n)
    n_pad = n_c * N_CORES
    if n_pad != n:
        pad = np.zeros((n_pad - n, D), dtype=np.float32)
        xr = np.concatenate([x_re, pad], axis=0)
        xi = np.concatenate([x_im, pad], axis=0)
    else:
        xr, xi = x_re, x_im
    xr_sh = xr.reshape(N_CORES, n_c, D)
    xi_sh = xi.reshape(N_CORES, n_c, D)
    in_maps = [
        {"x_re": np.ascontiguousarray(xr_sh[i]), "x_im": np.ascontiguousarray(xi_sh[i])}
        for i in range(N_CORES)
    ]
    res = bass_utils.run_bass_kernel_spmd(
        nc, in_maps, core_ids=list(range(N_CORES)), trace=_trace
    )
    out = np.concatenate([r["out"] for r in res.results], axis=0)[:n]
    if _trace:
        return out, res
    return out



# revision 36
# speedup vs baseline: 1.4428x; 1.4428x over previous
"""Trainium2 Bass kernel for nn_Biholomorphic_k2.

Per row (N=1e6 rows, D=5):
  z = x_re + i*x_im                                  [5] complex
  zz = z[i5] * z[j5]          (triu pairs of 5)      [15] complex
  prod = zz[i15] * conj(zz[j15])  (triu pairs of 15) [120] complex
  out = [Re(prod) (120 cols), Im(prod offdiag) (105 cols)]  [225] f32

Sharding: pure data parallel over 8 cores; rows on SBUF partitions in
contiguous blocks (partition p of core c owns rows [c*NC + p*RT, ...)),
features along the free dim so every DMA is fully contiguous.

All compute is VectorE tensor_tensor ops with stride-0 (broadcast) access
patterns on the "a"-side operand; results are written directly into the
packed [128, R*225] output tile, so there is no separate gather step.
"""

import sys

import numpy as np

try:
    import concourse.bass as bass
except ImportError:
    for _p in ("/opt/trn_rl_repo", "/root/.axon_site/_ro/trn_rl_repo"):
        if _p not in sys.path:
            sys.path.insert(0, _p)
    import concourse.bass as bass
import concourse.mybir as mybir
from concourse.tile import TileContext
from concourse import bass_utils

P = 128          # SBUF partitions
D = 5
K = 15           # triu pairs of 5
NOUT = 225       # 120 re + 105 im
N_CORES = 8

# --- static index tables (row-major triu, matching np.triu_indices) ---
# step-1: for a in 0..4 produce zz[o1[a] : o1[a]+5-a] = z[a] * z[a:5]
O1 = [0, 5, 9, 12, 14]
# step-2 re: for a in 0..14, out[ro[a] : ro[a]+15-a] = Re(zz[a] * conj(zz[a:15]))
RO = np.concatenate([[0], np.cumsum([15 - a for a in range(15)])]).astype(int)
# step-2 im: for a in 0..13, out[120+io[a] : ...+14-a] = Im(zz[a] * conj(zz[a+1:15]))
IO = 120 + np.concatenate([[0], np.cumsum([14 - a for a in range(14)])]).astype(int)

F32 = mybir.dt.float32
F16 = mybir.dt.float16

# "fp16": k-major fp16 compute (2x DVE mode), ~7e-4 rel error.
# "fp32": row-major fp32 compute, ~6e-8 rel error, ~2x slower.
PRECISION = "fp16_pe"
# route the step-2 im combines to GPSIMD (overlaps with DVE; shares one of
# DVE's two SBUF ports -- cost model says net win)
GPSIMD_IM = True
# additionally route re combines for these pair-start values to GPSIMD
GP_RE_PAIRS = ()
# pair-start values whose k2 mult runs on GPSIMD (fp16-kmout build)
GP_K2_PAIRS = (0, 2, 4, 6)
# pair-start values whose im combine runs on GPSIMD (fp16-kmout build)
GP_IM_PAIRS = (0, 2, 4, 6, 8, 10, 12)

_MAX_CTRL_WAITS = 1


def _split_excess_waits(nc):
    """Workaround: this walrus build rejects Drain instructions carrying
    more than one sync wait ("Too many sync wait commands").  Move excess
    waits onto NOPs inserted immediately before, on the same engine."""
    engmap = {
        mybir.EngineType.SP: nc.sync,
        mybir.EngineType.DVE: nc.vector,
        mybir.EngineType.Activation: nc.scalar,
        mybir.EngineType.PE: nc.tensor,
        mybir.EngineType.Pool: nc.gpsimd,
    }
    for f in nc.m.functions:
        for blk in f.blocks:
            newlist = []
            for inst in blk.instructions:
                si = inst.sync_info
                if (
                    si is not None
                    and si.on_wait is not None
                    and len(si.on_wait) > _MAX_CTRL_WAITS
                ):
                    waits = list(si.on_wait)
                    head = waits[:-_MAX_CTRL_WAITS]
                    tail = waits[-_MAX_CTRL_WAITS:]
                    for s in range(0, len(head), _MAX_CTRL_WAITS):
                        chunk = head[s : s + _MAX_CTRL_WAITS]
                        bi = engmap[inst.engine].nop()
                        nop_inst = bi.ins if hasattr(bi, "ins") else bi
                        for b2 in f.blocks:
                            if nop_inst in b2.instructions:
                                b2.instructions.remove(nop_inst)
                        nop_inst.sync_info = mybir.SyncInfo(on_wait=chunk, on_update=[])
                        nop_inst.engine = inst.engine
                        newlist.append(nop_inst)
                    inst.sync_info = mybir.SyncInfo(
                        on_wait=tail, on_update=list(si.on_update or [])
                    )
                newlist.append(inst)
            blk.instructions[:] = newlist


def _build(n_c, rt, tile_rs):
    """Build the Bass program for one core's shard: [n_c, 5] x2 -> [n_c, 225].
    n_c = P * rt rows; processed in free-dim chunks of R rows/partition."""
    nc = bass.Bass()
    xr_d = nc.dram_tensor("x_re", [n_c, D], F32, kind="ExternalInput")
    xi_d = nc.dram_tensor("x_im", [n_c, D], F32, kind="ExternalInput")
    out_d = nc.dram_tensor("out", [n_c, NOUT], F32, kind="ExternalOutput")

    xr_v = xr_d[:, :].rearrange("(p r) d -> p r d", p=P)     # [128, rt, 5]
    xi_v = xi_d[:, :].rearrange("(p r) d -> p r d", p=P)
    out_v = out_d[:, :].rearrange("(p r) c -> p r c", p=P)   # [128, rt, 225]

    with TileContext(nc) as tc:
        with (
            tc.tile_pool(name="io", bufs=3) as iop,
            tc.tile_pool(name="zz", bufs=2) as zzp,
            tc.tile_pool(name="scr", bufs=2) as scp,
            tc.tile_pool(name="outp", bufs=2) as outp,
        ):
            r0 = 0
            for R in tile_rs:
                xr_t = iop.tile([P, R * D], F32, tag="xr")
                xi_t = iop.tile([P, R * D], F32, tag="xi")
                nc.sync.dma_start(
                    out=xr_t[:].rearrange("p (r d) -> p r d", d=D),
                    in_=xr_v[:, r0 : r0 + R, :],
                )
                nc.sync.dma_start(
                    out=xi_t[:].rearrange("p (r d) -> p r d", d=D),
                    in_=xi_v[:, r0 : r0 + R, :],
                )
                xr3 = xr_t[:].rearrange("p (r d) -> p r d", d=D)   # [128,R,5]
                xi3 = xi_t[:].rearrange("p (r d) -> p r d", d=D)

                ar_t = zzp.tile([P, R * K], F32, tag="ar")
                ai_t = zzp.tile([P, R * K], F32, tag="ai")
                ar3 = ar_t[:].rearrange("p (r k) -> p r k", k=K)   # [128,R,15]
                ai3 = ai_t[:].rearrange("p (r k) -> p r k", k=K)

                # ---- step 1: zz = z[a] * z[a:5] for a in 0..4 ----
                for a in range(D):
                    L = D - a
                    o = O1[a]
                    s1 = scp.tile([P, R * D], F32, tag="s1")
                    s2 = scp.tile([P, R * D], F32, tag="s2")
                    s1v = s1[:].rearrange("p (r d) -> p r d", d=D)[:, :, :L]
                    s2v = s2[:].rearrange("p (r d) -> p r d", d=D)[:, :, :L]
                    bra = xr3[:, :, a : a + 1].broadcast_to([P, R, L])
                    bia = xi3[:, :, a : a + 1].broadcast_to([P, R, L])
                    # re: xr_a*xr_b - xi_a*xi_b
                    nc.vector.tensor_mul(out=s1v, in0=bra, in1=xr3[:, :, a:D])
                    nc.vector.tensor_mul(out=s2v, in0=bia, in1=xi3[:, :, a:D])
                    nc.vector.tensor_sub(
                        out=ar3[:, :, o : o + L], in0=s1v, in1=s2v
                    )
                    # im: xr_a*xi_b + xi_a*xr_b
                    s3 = scp.tile([P, R * D], F32, tag="s3")
                    s4 = scp.tile([P, R * D], F32, tag="s4")
                    s3v = s3[:].rearrange("p (r d) -> p r d", d=D)[:, :, :L]
                    s4v = s4[:].rearrange("p (r d) -> p r d", d=D)[:, :, :L]
                    nc.vector.tensor_mul(out=s3v, in0=bra, in1=xi3[:, :, a:D])
                    nc.vector.tensor_mul(out=s4v, in0=bia, in1=xr3[:, :, a:D])
                    nc.vector.tensor_add(
                        out=ai3[:, :, o : o + L], in0=s3v, in1=s4v
                    )

                out_t = outp.tile([P, R * NOUT], F32, tag="out")
                out3 = out_t[:].rearrange("p (r c) -> p r c", c=NOUT)

                # ---- step 2: prod = zz[a] * conj(zz[b]), b >= a ----
                for a in range(K):
                    L = K - a
                    bar = ar3[:, :, a : a + 1].broadcast_to([P, R, L])
                    bai = ai3[:, :, a : a + 1].broadcast_to([P, R, L])
                    m1 = scp.tile([P, R * K], F32, tag="m1")
                    m2 = scp.tile([P, R * K], F32, tag="m2")
                    m1v = m1[:].rearrange("p (r k) -> p r k", k=K)[:, :, :L]
                    m2v = m2[:].rearrange("p (r k) -> p r k", k=K)[:, :, :L]
                    # re: ar_a*ar_b + ai_a*ai_b
                    nc.vector.tensor_mul(out=m1v, in0=bar, in1=ar3[:, :, a:K])
                    nc.vector.tensor_mul(out=m2v, in0=bai, in1=ai3[:, :, a:K])
                    ro = int(RO[a])
                    nc.vector.tensor_add(
                        out=out3[:, :, ro : ro + L], in0=m1v, in1=m2v
                    )
                    # im (offdiag only): ai_a*ar_b - ar_a*ai_b
                    if a < K - 1:
                        L2 = L - 1
                        m3 = scp.tile([P, R * K], F32, tag="m3")
                        m4 = scp.tile([P, R * K], F32, tag="m4")
                        m3v = m3[:].rearrange("p (r k) -> p r k", k=K)[:, :, :L2]
                        m4v = m4[:].rearrange("p (r k) -> p r k", k=K)[:, :, :L2]
                        bar2 = ar3[:, :, a : a + 1].broadcast_to([P, R, L2])
                        bai2 = ai3[:, :, a : a + 1].broadcast_to([P, R, L2])
                        nc.vector.tensor_mul(
                            out=m3v, in0=bai2, in1=ar3[:, :, a + 1 : K]
                        )
                        nc.vector.tensor_mul(
                            out=m4v, in0=bar2, in1=ai3[:, :, a + 1 : K]
                        )
                        io = int(IO[a])
                        nc.vector.tensor_sub(
                            out=out3[:, :, io : io + L2], in0=m3v, in1=m4v
                        )

                nc.sync.dma_start(out=out_v[:, r0 : r0 + R, :], in_=out3)
                r0 += R

    _split_excess_waits(nc)
    return nc


def _build_fp16(n_c, rt, tile_rs):
    """k-major fp16 build: within each partition, every tensor is stored
    feature-major ([k, r] with r innermost, step 1) so all DVE tensor_tensor
    operands have a 2-byte dtype, innermost step 1, and 4B-aligned run starts
    (R even) -> 2x_1p DVE mode throughout.  ScalarE does the fp32->fp16
    transpose-in and the fp16->fp32 transpose-out (its own SBUF ports, 1x).
    """
    KP = 16  # zz column padding (alignment headroom)
    nc = bass.Bass()
    xr_d = nc.dram_tensor("x_re", [n_c, D], F32, kind="ExternalInput")
    xi_d = nc.dram_tensor("x_im", [n_c, D], F32, kind="ExternalInput")
    out_d = nc.dram_tensor("out", [n_c, NOUT], F32, kind="ExternalOutput")

    xr_v = xr_d[:, :].rearrange("(p r) d -> p r d", p=P)     # [128, rt, 5]
    xi_v = xi_d[:, :].rearrange("(p r) d -> p r d", p=P)
    out_v = out_d[:, :].rearrange("(p r) c -> p r c", p=P)   # [128, rt, 225]

    with TileContext(nc) as tc:
        with (
            tc.tile_pool(name="io", bufs=3) as iop,
            tc.tile_pool(name="km", bufs=2) as kmp,
            tc.tile_pool(name="zz", bufs=2) as zzp,
            tc.tile_pool(name="scr", bufs=2) as scp,
            tc.tile_pool(name="outk", bufs=2) as okp,
            tc.tile_pool(name="outf", bufs=2) as ofp,
        ):
            r0 = 0
            for R in tile_rs:
                assert R % 2 == 0
                R2 = R // 2
                xr_s = iop.tile([P, R * D], F32, tag="xr")
                xi_s = iop.tile([P, R * D], F32, tag="xi")
                nc.sync.dma_start(
                    out=xr_s[:].rearrange("p (r d) -> p r d", d=D),
                    in_=xr_v[:, r0 : r0 + R, :],
                )
                nc.sync.dma_start(
                    out=xi_s[:].rearrange("p (r d) -> p r d", d=D),
                    in_=xi_v[:, r0 : r0 + R, :],
                )
                # ScalarE: cast fp32->fp16 + transpose row-major -> k-major
                xr_k = kmp.tile([P, D * R], F16, tag="xrk")
                xi_k = kmp.tile([P, D * R], F16, tag="xik")
                # src [r, d] -> view [d, r]
                nc.scalar.copy(
                    out=xr_k[:].rearrange("p (d r) -> p d r", d=D),
                    in_=xr_s[:].rearrange("p (r d) -> p r d", d=D).transpose([0, 2, 1]),
                )
                nc.scalar.copy(
                    out=xi_k[:].rearrange("p (d r) -> p d r", d=D),
                    in_=xi_s[:].rearrange("p (r d) -> p r d", d=D).transpose([0, 2, 1]),
                )
                xr3 = xr_k[:].rearrange("p (d r) -> p d r", d=D)   # [128,5,R]
                xi3 = xi_k[:].rearrange("p (d r) -> p d r", d=D)

                ar_t = zzp.tile([P, KP * R], F16, tag="ar")
                ai_t = zzp.tile([P, KP * R], F16, tag="ai")
                ar3 = ar_t[:].rearrange("p (k r) -> p k r", k=KP)  # [128,16,R]
                ai3 = ai_t[:].rearrange("p (k r) -> p k r", k=KP)

                # ---- step 1: zz[o1[a]:o1[a]+L] = z[a] * z[a:5] ----
                for a in range(D):
                    L = D - a
                    o = O1[a]
                    bra = xr3[:, a : a + 1, :].broadcast_to([P, L, R])
                    bia = xi3[:, a : a + 1, :].broadcast_to([P, L, R])
                    s1 = scp.tile([P, D * R], F16, tag="s1")
                    s2 = scp.tile([P, D * R], F16, tag="s2")
                    s1v = s1[:].rearrange("p (k r) -> p k r", k=D)[:, :L, :]
                    s2v = s2[:].rearrange("p (k r) -> p k r", k=D)[:, :L, :]
                    nc.vector.tensor_mul(out=s1v, in0=bra, in1=xr3[:, a:D, :])
                    nc.vector.tensor_mul(out=s2v, in0=bia, in1=xi3[:, a:D, :])
                    nc.vector.tensor_sub(out=ar3[:, o : o + L, :], in0=s1v, in1=s2v)
                    s3 = scp.tile([P, D * R], F16, tag="s3")
                    s4 = scp.tile([P, D * R], F16, tag="s4")
                    s3v = s3[:].rearrange("p (k r) -> p k r", k=D)[:, :L, :]
                    s4v = s4[:].rearrange("p (k r) -> p k r", k=D)[:, :L, :]
                    nc.vector.tensor_mul(out=s3v, in0=bra, in1=xi3[:, a:D, :])
                    nc.vector.tensor_mul(out=s4v, in0=bia, in1=xr3[:, a:D, :])
                    nc.vector.tensor_add(out=ai3[:, o : o + L, :], in0=s3v, in1=s4v)

                out_k = okp.tile([P, NOUT * R], F16, tag="outk")
                ok3 = out_k[:].rearrange("p (c r) -> p c r", c=NOUT)  # [128,225,R]

                # ---- step 2 ----
                for a in range(K):
                    L = K - a
                    bar = ar3[:, a : a + 1, :].broadcast_to([P, L, R])
                    bai = ai3[:, a : a + 1, :].broadcast_to([P, L, R])
                    m1 = scp.tile([P, KP * R], F16, tag="m1")
                    m2 = scp.tile([P, KP * R], F16, tag="m2")
                    m1v = m1[:].rearrange("p (k r) -> p k r", k=KP)[:, :L, :]
                    m2v = m2[:].rearrange("p (k r) -> p k r", k=KP)[:, :L, :]
                    nc.vector.tensor_mul(out=m1v, in0=bar, in1=ar3[:, a:K, :])
                    nc.vector.tensor_mul(out=m2v, in0=bai, in1=ai3[:, a:K, :])
                    ro = int(RO[a])
                    nc.vector.tensor_add(out=ok3[:, ro : ro + L, :], in0=m1v, in1=m2v)
                    if a < K - 1:
                        L2 = L - 1
                        bar2 = ar3[:, a : a + 1, :].broadcast_to([P, L2, R])
                        bai2 = ai3[:, a : a + 1, :].broadcast_to([P, L2, R])
                        m3 = scp.tile([P, KP * R], F16, tag="m3")
                        m4 = scp.tile([P, KP * R], F16, tag="m4")
                        m3v = m3[:].rearrange("p (k r) -> p k r", k=KP)[:, :L2, :]
                        m4v = m4[:].rearrange("p (k r) -> p k r", k=KP)[:, :L2, :]
                        nc.vector.tensor_mul(out=m3v, in0=bai2, in1=ar3[:, a + 1 : K, :])
                        nc.vector.tensor_mul(out=m4v, in0=bar2, in1=ai3[:, a + 1 : K, :])
                        io = int(IO[a])
                        nc.vector.tensor_sub(
                            out=ok3[:, io : io + L2, :], in0=m3v, in1=m4v
                        )

                # ---- ScalarE: fp16 k-major -> fp32 row-major, in row-chunks ----
                n_chunks = 4 if R % 4 == 0 else 2
                Rc = R // n_chunks
                for h in range(n_chunks):
                    of = ofp.tile([P, Rc * NOUT], F32, tag="outf")
                    of3 = of[:].rearrange("p (r c) -> p r c", c=NOUT)  # [128,Rc,225]
                    src = ok3[:, :, h * Rc : (h + 1) * Rc].transpose([0, 2, 1])
                    nc.scalar.copy(out=of3, in_=src)
                    nc.sync.dma_start(
                        out=out_v[:, r0 + h * Rc : r0 + (h + 1) * Rc, :], in_=of3
                    )
                r0 += R

    _split_excess_waits(nc)
    return nc


def _ap4(t2d, col_off, jstep_cols, L, R):
    """4-dim AP over a k-major [cols x R] SBUF tile view: two runs (j=0,1)
    of L columns x R rows, run j starting at column col_off + j*jstep_cols."""
    pdim = list(t2d.ap[0])
    return bass.AP(
        t2d.tensor,
        t2d.offset + col_off * R,
        [pdim, [jstep_cols * R, 2], [R, L], [1, R]],
    )


def _build_fp16_paired(n_c, rt, tile_rs):
    """Like _build_fp16 but batches consecutive-a groups in pairs via 4-dim
    APs, halving DVE instruction count.  The second run of each pair reads one
    padded junk column and writes one column past its end; emission order
    guarantees a later group rewrites the overshoot column with real data."""
    KP = 16       # zz padded to 16 cols (col 15 = junk read by pair overshoot)
    D2 = 6        # input padded to 6 cols (col 5 = junk)
    NP = NOUT + 1 # out_k padded by 1 col for im-pair overshoot
    nc = bass.Bass()
    xr_d = nc.dram_tensor("x_re", [n_c, D], F32, kind="ExternalInput")
    xi_d = nc.dram_tensor("x_im", [n_c, D], F32, kind="ExternalInput")
    out_d = nc.dram_tensor("out", [n_c, NOUT], F32, kind="ExternalOutput")

    xr_v = xr_d[:, :].rearrange("(p r) d -> p r d", p=P)
    xi_v = xi_d[:, :].rearrange("(p r) d -> p r d", p=P)
    out_v = out_d[:, :].rearrange("(p r) c -> p r c", p=P)

    with TileContext(nc) as tc:
        with (
            tc.tile_pool(name="io", bufs=2) as iop,
            tc.tile_pool(name="km", bufs=2) as kmp,
            tc.tile_pool(name="zz", bufs=2) as zzp,
            tc.tile_pool(name="scr", bufs=2) as scp,
            tc.tile_pool(name="outk", bufs=2) as okp,
            tc.tile_pool(name="outf", bufs=4) as ofp,
        ):
            r0 = 0
            for R in tile_rs:
                assert R % 2 == 0
                xr_s = iop.tile([P, R * D], F32, tag="xr")
                xi_s = iop.tile([P, R * D], F32, tag="xi")
                nc.sync.dma_start(
                    out=xr_s[:].rearrange("p (r d) -> p r d", d=D),
                    in_=xr_v[:, r0 : r0 + R, :],
                )
                nc.sync.dma_start(
                    out=xi_s[:].rearrange("p (r d) -> p r d", d=D),
                    in_=xi_v[:, r0 : r0 + R, :],
                )
                xr_k = kmp.tile([P, D2 * R], F16, tag="xrk")
                xi_k = kmp.tile([P, D2 * R], F16, tag="xik")
                nc.scalar.copy(
                    out=xr_k[:].rearrange("p (d r) -> p d r", d=D2)[:, :D, :],
                    in_=xr_s[:].rearrange("p (r d) -> p r d", d=D).transpose([0, 2, 1]),
                )
                nc.scalar.copy(
                    out=xi_k[:].rearrange("p (d r) -> p d r", d=D2)[:, :D, :],
                    in_=xi_s[:].rearrange("p (r d) -> p r d", d=D).transpose([0, 2, 1]),
                )
                xr3 = xr_k[:].rearrange("p (d r) -> p d r", d=D2)   # [128,6,R]
                xi3 = xi_k[:].rearrange("p (d r) -> p d r", d=D2)

                ar_t = zzp.tile([P, KP * R], F16, tag="ar")
                ai_t = zzp.tile([P, KP * R], F16, tag="ai")
                ar3 = ar_t[:].rearrange("p (k r) -> p k r", k=KP)
                ai3 = ai_t[:].rearrange("p (k r) -> p k r", k=KP)

                # ---- step 1 (pairs (0,1),(2,3) then single a=4) ----
                for a in (0, 2):
                    L = D - a
                    b_r = xr3[:, a : a + 2, :].unsqueeze(2).broadcast_to([P, 2, L, R])
                    b_i = xi3[:, a : a + 2, :].unsqueeze(2).broadcast_to([P, 2, L, R])
                    w_r = _ap4(xr_k[:], a, 1, L, R)
                    w_i = _ap4(xi_k[:], a, 1, L, R)
                    s1 = scp.tile([P, 2 * D * R], F16, tag="s1")
                    s2 = scp.tile([P, 2 * D * R], F16, tag="s2")
                    s3 = scp.tile([P, 2 * D * R], F16, tag="s3")
                    s4 = scp.tile([P, 2 * D * R], F16, tag="s4")
                    s1v = s1[:, : 2 * L * R].rearrange("p (j k r) -> p j k r", j=2, k=L)
                    s2v = s2[:, : 2 * L * R].rearrange("p (j k r) -> p j k r", j=2, k=L)
                    s3v = s3[:, : 2 * L * R].rearrange("p (j k r) -> p j k r", j=2, k=L)
                    s4v = s4[:, : 2 * L * R].rearrange("p (j k r) -> p j k r", j=2, k=L)
                    nc.vector.tensor_mul(out=s1v, in0=b_r, in1=w_r)
                    nc.vector.tensor_mul(out=s2v, in0=b_i, in1=w_i)
                    nc.vector.tensor_sub(out=_ap4(ar_t[:], O1[a], L, L, R), in0=s1v, in1=s2v)
                    nc.vector.tensor_mul(out=s3v, in0=b_r, in1=w_i)
                    nc.vector.tensor_mul(out=s4v, in0=b_i, in1=w_r)
                    nc.vector.tensor_add(out=_ap4(ai_t[:], O1[a], L, L, R), in0=s3v, in1=s4v)
                # single a=4 (L=1)
                a = 4
                bra = xr3[:, a : a + 1, :]
                bia = xi3[:, a : a + 1, :]
                s1 = scp.tile([P, 2 * D * R], F16, tag="s1")
                s2 = scp.tile([P, 2 * D * R], F16, tag="s2")
                s1v = s1[:, :R].unsqueeze(1)
                s2v = s2[:, :R].unsqueeze(1)
                nc.vector.tensor_mul(out=s1v, in0=bra, in1=xr3[:, a : a + 1, :])
                nc.vector.tensor_mul(out=s2v, in0=bia, in1=xi3[:, a : a + 1, :])
                nc.vector.tensor_sub(out=ar3[:, 14:15, :], in0=s1v, in1=s2v)
                s3 = scp.tile([P, 2 * D * R], F16, tag="s3")
                s4 = scp.tile([P, 2 * D * R], F16, tag="s4")
                s3v = s3[:, :R].unsqueeze(1)
                s4v = s4[:, :R].unsqueeze(1)
                nc.vector.tensor_mul(out=s3v, in0=bra, in1=xi3[:, a : a + 1, :])
                nc.vector.tensor_mul(out=s4v, in0=bia, in1=xr3[:, a : a + 1, :])
                nc.vector.tensor_add(out=ai3[:, 14:15, :], in0=s3v, in1=s4v)

                out_k = okp.tile([P, NP * R], F16, tag="outk")
                ok3 = out_k[:].rearrange("p (c r) -> p c r", c=NP)

                # Gauss 3-mult: with v = ar+ai, u = ar-ai:
                #   k1 = v_a * ar_b;  k3 = ai_a * u_b;  k2 = ar_a * v_b
                #   re(a,b) = k1 - k3;  im(a,b) = k1 - k2   (b >= a+1 for im)
                v_t = zzp.tile([P, KP * R], F16, tag="vt")
                u_t = zzp.tile([P, KP * R], F16, tag="ut")
                nc.vector.tensor_add(out=v_t[:], in0=ar_t[:], in1=ai_t[:])
                nc.vector.tensor_sub(out=u_t[:], in0=ar_t[:], in1=ai_t[:])
                v3 = v_t[:].rearrange("p (k r) -> p k r", k=KP)

                # ---- step 2: re pairs a=0,2,..,12 + single a=14 ----
                im_eng = nc.gpsimd if GPSIMD_IM else nc.vector
                for a in range(0, K - 1, 2):
                    L = K - a
                    L2 = L - 1
                    b_v = v3[:, a : a + 2, :].unsqueeze(2).broadcast_to([P, 2, L, R])
                    b_ai = ai3[:, a : a + 2, :].unsqueeze(2).broadcast_to([P, 2, L, R])
                    b_ar = ar3[:, a : a + 2, :].unsqueeze(2).broadcast_to([P, 2, L2, R])
                    w_ar = _ap4(ar_t[:], a, 1, L, R)
                    w_u = _ap4(u_t[:], a, 1, L, R)
                    w_v = _ap4(v_t[:], a + 1, 1, L2, R)
                    k1 = scp.tile([P, 2 * K * R], F16, tag="m1")
                    k2 = scp.tile([P, 2 * K * R], F16, tag="m2")
                    k3 = scp.tile([P, 2 * K * R], F16, tag="m3")
                    k1v = k1[:, : 2 * L * R].rearrange("p (j k r) -> p j k r", j=2, k=L)
                    k3v = k3[:, : 2 * L * R].rearrange("p (j k r) -> p j k r", j=2, k=L)
                    k2v = k2[:, : 2 * L2 * R].rearrange("p (j k r) -> p j k r", j=2, k=L2)
                    nc.vector.tensor_mul(out=k1v, in0=b_v, in1=w_ar)
                    nc.vector.tensor_mul(out=k3v, in0=b_ai, in1=w_u)
                    nc.vector.tensor_mul(out=k2v, in0=b_ar, in1=w_v)
                    re_eng = nc.gpsimd if a in GP_RE_PAIRS else nc.vector
                    re_eng.tensor_sub(
                        out=_ap4(out_k[:], int(RO[a]), L, L, R), in0=k1v, in1=k3v
                    )
                    im_eng.tensor_sub(
                        out=_ap4(out_k[:], int(IO[a]), L2, L2, R),
                        in0=k1v[:, :, 1:, :],
                        in1=k2v,
                    )
                # single a=14 re (L=1): pr = ar^2 + ai^2
                m1 = scp.tile([P, 2 * K * R], F16, tag="m1")
                m2 = scp.tile([P, 2 * K * R], F16, tag="m2")
                m1v = m1[:, :R].unsqueeze(1)
                m2v = m2[:, :R].unsqueeze(1)
                nc.vector.tensor_mul(out=m1v, in0=ar3[:, 14:15, :], in1=ar3[:, 14:15, :])
                nc.vector.tensor_mul(out=m2v, in0=ai3[:, 14:15, :], in1=ai3[:, 14:15, :])
                nc.vector.tensor_add(out=ok3[:, 119:120, :], in0=m1v, in1=m2v)

                # ---- ScalarE: fp16 k-major -> fp32 row-major, ~12-row chunks ----
                RC = 8
                h0 = 0
                while h0 < R:
                    rc = min(RC, R - h0)
                    of = ofp.tile([P, RC * NOUT], F32, tag="outf")
                    of3 = of[:, : rc * NOUT].rearrange("p (r c) -> p r c", c=NOUT)
                    src = ok3[:, :NOUT, h0 : h0 + rc].transpose([0, 2, 1])
                    nc.scalar.copy(out=of3, in_=src)
                    nc.sync.dma_start(
                        out=out_v[:, r0 + h0 : r0 + h0 + rc, :], in_=of3
                    )
                    h0 += rc
                r0 += R

    _split_excess_waits(nc)
    return nc


def _build_fp16_kmout(
    n_c,
    rt,
    tile_rs,
    gp_k2=None,
    gp_im=None,
    gp_re=None,
    k1_bufs=4,
    k2_bufs=4,
    k2_first=True,
):
    """fp16 k-major compute (paired 4-dim AP groups) with the output DMA'd
    straight from the k-major fp16 tile: no on-chip transpose-out and no
    fp32 upconvert.  The DRAM output is a per-partition slab of
    rt*NOUT fp16 values laid out [tile][col][row-in-tile]; the host
    de-interleaves and upcasts during unshard.  Mult ops that run on
    GPSIMD go through scalar_tensor_tensor (out=(in0*1)op in1).
    k1 scratch gets its own deeper ring so DVE isn't back-pressured by the
    GPSIMD im consumer."""
    if gp_k2 is None:
        gp_k2 = GP_K2_PAIRS
    if gp_im is None:
        gp_im = GP_IM_PAIRS
    if gp_re is None:
        gp_re = GP_RE_PAIRS
    KP = 16       # zz padded to 16 cols (col 15 = junk read by pair overshoot)
    D2 = 6        # input padded to 6 cols (col 5 = junk)
    NP = NOUT + 1 # out_k padded by 1 col for im-pair overshoot
    nc = bass.Bass()
    xr_d = nc.dram_tensor("x_re", [n_c, D], F32, kind="ExternalInput")
    xi_d = nc.dram_tensor("x_im", [n_c, D], F32, kind="ExternalInput")
    out_d = nc.dram_tensor("out", [P, rt * NOUT], F16, kind="ExternalOutput")

    xr_v = xr_d[:, :].rearrange("(p r) d -> p r d", p=P)
    xi_v = xi_d[:, :].rearrange("(p r) d -> p r d", p=P)

    mul_op = mybir.AluOpType.mult
    sub_op = mybir.AluOpType.subtract

    def gp_mul(out, in0, in1):
        nc.gpsimd.scalar_tensor_tensor(
            out=out, in0=in0, scalar=1.0, in1=in1, op0=mul_op, op1=mul_op
        )

    def gp_sub(out, in0, in1):
        nc.gpsimd.scalar_tensor_tensor(
            out=out, in0=in0, scalar=1.0, in1=in1, op0=mul_op, op1=sub_op
        )

    with TileContext(nc) as tc:
        with (
            tc.tile_pool(name="io", bufs=2) as iop,
            tc.tile_pool(name="km", bufs=2) as kmp,
            tc.tile_pool(name="zz", bufs=2) as zzp,
            tc.tile_pool(name="scr", bufs=2) as scp,
            tc.tile_pool(name="k1p", bufs=k1_bufs) as k1p,
            tc.tile_pool(name="k2p", bufs=k2_bufs) as k2p,
            tc.tile_pool(name="outk", bufs=2) as okp,
        ):
            r0 = 0
            for R in tile_rs:
                assert R % 2 == 0
                xr_s = iop.tile([P, R * D], F32, tag="xr")
                xi_s = iop.tile([P, R * D], F32, tag="xi")
                nc.sync.dma_start(
                    out=xr_s[:].rearrange("p (r d) -> p r d", d=D),
                    in_=xr_v[:, r0 : r0 + R, :],
                )
                nc.sync.dma_start(
                    out=xi_s[:].rearrange("p (r d) -> p r d", d=D),
                    in_=xi_v[:, r0 : r0 + R, :],
                )
                xr_k = kmp.tile([P, D2 * R], F16, tag="xrk")
                xi_k = kmp.tile([P, D2 * R], F16, tag="xik")
                nc.scalar.copy(
                    out=xr_k[:].rearrange("p (d r) -> p d r", d=D2)[:, :D, :],
                    in_=xr_s[:].rearrange("p (r d) -> p r d", d=D).transpose([0, 2, 1]),
                )
                nc.scalar.copy(
                    out=xi_k[:].rearrange("p (d r) -> p d r", d=D2)[:, :D, :],
                    in_=xi_s[:].rearrange("p (r d) -> p r d", d=D).transpose([0, 2, 1]),
                )
                xr3 = xr_k[:].rearrange("p (d r) -> p d r", d=D2)   # [128,6,R]
                xi3 = xi_k[:].rearrange("p (d r) -> p d r", d=D2)

                ar_t = zzp.tile([P, KP * R], F16, tag="ar")
                ai_t = zzp.tile([P, KP * R], F16, tag="ai")
                ar3 = ar_t[:].rearrange("p (k r) -> p k r", k=KP)
                ai3 = ai_t[:].rearrange("p (k r) -> p k r", k=KP)

                # ---- step 1 (pairs (0,1),(2,3) then single a=4) ----
                for a in (0, 2):
                    L = D - a
                    b_r = xr3[:, a : a + 2, :].unsqueeze(2).broadcast_to([P, 2, L, R])
                    b_i = xi3[:, a : a + 2, :].unsqueeze(2).broadcast_to([P, 2, L, R])
                    w_r = _ap4(xr_k[:], a, 1, L, R)
                    w_i = _ap4(xi_k[:], a, 1, L, R)
                    s1 = scp.tile([P, 2 * D * R], F16, tag="s1")
                    s2 = scp.tile([P, 2 * D * R], F16, tag="s2")
                    s3 = scp.tile([P, 2 * D * R], F16, tag="s3")
                    s4 = scp.tile([P, 2 * D * R], F16, tag="s4")
                    s1v = s1[:, : 2 * L * R].rearrange("p (j k r) -> p j k r", j=2, k=L)
                    s2v = s2[:, : 2 * L * R].rearrange("p (j k r) -> p j k r", j=2, k=L)
                    s3v = s3[:, : 2 * L * R].rearrange("p (j k r) -> p j k r", j=2, k=L)
                    s4v = s4[:, : 2 * L * R].rearrange("p (j k r) -> p j k r", j=2, k=L)
                    nc.vector.tensor_mul(out=s1v, in0=b_r, in1=w_r)
                    nc.vector.tensor_mul(out=s2v, in0=b_i, in1=w_i)
                    nc.vector.tensor_sub(out=_ap4(ar_t[:], O1[a], L, L, R), in0=s1v, in1=s2v)
                    nc.vector.tensor_mul(out=s3v, in0=b_r, in1=w_i)
                    nc.vector.tensor_mul(out=s4v, in0=b_i, in1=w_r)
                    nc.vector.tensor_add(out=_ap4(ai_t[:], O1[a], L, L, R), in0=s3v, in1=s4v)
                # single a=4 (L=1)
                a = 4
                bra = xr3[:, a : a + 1, :]
                bia = xi3[:, a : a + 1, :]
                s1 = scp.tile([P, 2 * D * R], F16, tag="s1")
                s2 = scp.tile([P, 2 * D * R], F16, tag="s2")
                s1v = s1[:, :R].unsqueeze(1)
                s2v = s2[:, :R].unsqueeze(1)
                nc.vector.tensor_mul(out=s1v, in0=bra, in1=xr3[:, a : a + 1, :])
                nc.vector.tensor_mul(out=s2v, in0=bia, in1=xi3[:, a : a + 1, :])
                nc.vector.tensor_sub(out=ar3[:, 14:15, :], in0=s1v, in1=s2v)
                s3 = scp.tile([P, 2 * D * R], F16, tag="s3")
                s4 = scp.tile([P, 2 * D * R], F16, tag="s4")
                s3v = s3[:, :R].unsqueeze(1)
                s4v = s4[:, :R].unsqueeze(1)
                nc.vector.tensor_mul(out=s3v, in0=bra, in1=xi3[:, a : a + 1, :])
                nc.vector.tensor_mul(out=s4v, in0=bia, in1=xr3[:, a : a + 1, :])
                nc.vector.tensor_add(out=ai3[:, 14:15, :], in0=s3v, in1=s4v)

                out_k = okp.tile([P, NP * R], F16, tag="outk")
                ok3 = out_k[:].rearrange("p (c r) -> p c r", c=NP)

                # Gauss 3-mult: with v = ar+ai, u = ar-ai:
                #   k1 = v_a * ar_b;  k3 = ai_a * u_b;  k2 = ar_a * v_b
                #   re(a,b) = k1 - k3;  im(a,b) = k1 - k2   (b >= a+1 for im)
                v_t = zzp.tile([P, KP * R], F16, tag="vt")
                u_t = zzp.tile([P, KP * R], F16, tag="ut")
                nc.vector.tensor_add(out=v_t[:], in0=ar_t[:], in1=ai_t[:])
                nc.vector.tensor_sub(out=u_t[:], in0=ar_t[:], in1=ai_t[:])
                v3 = v_t[:].rearrange("p (k r) -> p k r", k=KP)

                # ---- step 2: re pairs a=0,2,..,12 + single a=14 ----
                for a in range(0, K - 1, 2):
                    L = K - a
                    L2 = L - 1
                    b_v = v3[:, a : a + 2, :].unsqueeze(2).broadcast_to([P, 2, L, R])
                    b_ai = ai3[:, a : a + 2, :].unsqueeze(2).broadcast_to([P, 2, L, R])
                    b_ar = ar3[:, a : a + 2, :].unsqueeze(2).broadcast_to([P, 2, L2, R])
                    w_ar = _ap4(ar_t[:], a, 1, L, R)
                    w_u = _ap4(u_t[:], a, 1, L, R)
                    w_v = _ap4(v_t[:], a + 1, 1, L2, R)
                    k1 = k1p.tile([P, 2 * K * R], F16, tag="m1")
                    k2 = k2p.tile([P, 2 * K * R], F16, tag="m2")
                    k3 = scp.tile([P, 2 * K * R], F16, tag="m3")
                    k1v = k1[:, : 2 * L * R].rearrange("p (j k r) -> p j k r", j=2, k=L)
                    k3v = k3[:, : 2 * L * R].rearrange("p (j k r) -> p j k r", j=2, k=L)
                    k2v = k2[:, : 2 * L2 * R].rearrange("p (j k r) -> p j k r", j=2, k=L2)

                    def emit_k2():
                        if a in gp_k2:
                            gp_mul(k2v, b_ar, w_v)
                        else:
                            nc.vector.tensor_mul(out=k2v, in0=b_ar, in1=w_v)

                    if k2_first:
                        emit_k2()
                    nc.vector.tensor_mul(out=k1v, in0=b_v, in1=w_ar)
                    nc.vector.tensor_mul(out=k3v, in0=b_ai, in1=w_u)
                    if not k2_first:
                        emit_k2()
                    if a in gp_re:
                        gp_sub(_ap4(out_k[:], int(RO[a]), L, L, R), k1v, k3v)
                    else:
                        nc.vector.tensor_sub(
                            out=_ap4(out_k[:], int(RO[a]), L, L, R), in0=k1v, in1=k3v
                        )
                    if a in gp_im:
                        gp_sub(_ap4(out_k[:], int(IO[a]), L2, L2, R), k1v[:, :, 1:, :], k2v)
                    else:
                        nc.vector.tensor_sub(
                            out=_ap4(out_k[:], int(IO[a]), L2, L2, R),
                            in0=k1v[:, :, 1:, :],
                            in1=k2v,
                        )
                # single a=14 re (L=1): pr = ar^2 + ai^2
                m1 = scp.tile([P, 2 * D * R], F16, tag="s1")
                m2 = scp.tile([P, 2 * D * R], F16, tag="s2")
                m1v = m1[:, :R].unsqueeze(1)
                m2v = m2[:, :R].unsqueeze(1)
                nc.vector.tensor_mul(out=m1v, in0=ar3[:, 14:15, :], in1=ar3[:, 14:15, :])
                nc.vector.tensor_mul(out=m2v, in0=ai3[:, 14:15, :], in1=ai3[:, 14:15, :])
                nc.vector.tensor_add(out=ok3[:, 119:120, :], in0=m1v, in1=m2v)

                # ---- direct k-major fp16 DMA out (contiguous slab) ----
                nc.sync.dma_start(
                    out=out_d[:, r0 * NOUT : (r0 + R) * NOUT],
                    in_=out_k[:, : NOUT * R],
                )
                r0 += R

    _split_excess_waits(nc)
    return nc


def _build_fp16_pe(
    n_c,
    rt,
    tile_rs,
    im_asn=None,     # pair-start a -> 'pe' | 'dve' | 'pool'
    ring_bufs=6,
    psum_bufs=4,
    chunk=512,
    psb=1024,        # PSUM super-chunk (elements; 2 banks) per Act copy
):
    """fp16 k-major compute with the PE (tensor) engine doing the re (and
    selected im) combines as identity-weight matmul accumulations in PSUM,
    Act converting PSUM fp32 -> fp16 out_k, and the output DMA'd k-major.

    Per pair-group (a, a+1), both runs length L = 15-a:
      k1 = v_a * ar[b]      (DVE)   v = ar+ai
      k3'= ai_a * u2[b]     (DVE)   u2 = ai-ar   (negated Gauss operand)
      k2'= -(ar_a * v[b+1]) (Pool STT, scalar=-1)
      re(a,b)  = k1 + k3'   (PE accumulate, Act copy)
      im(a,b)  = k1[1:] + k2'  (PE / DVE / Pool per im_asn)
    The identity stationary matrix ships as an extra ExternalInput."""
    if im_asn is None:
        im_asn = {a: ("dve" if a >= 10 else "pe") for a in range(0, K - 1, 2)}
    KP = 16
    D2 = 6
    NP = NOUT + 1
    nc = bass.Bass()
    xr_d = nc.dram_tensor("x_re", [n_c, D], F32, kind="ExternalInput")
    xi_d = nc.dram_tensor("x_im", [n_c, D], F32, kind="ExternalInput")
    id_d = nc.dram_tensor("ident", [P, P], F16, kind="ExternalInput")
    out_d = nc.dram_tensor("out", [P, rt * NOUT], F16, kind="ExternalOutput")

    xr_v = xr_d[:, :].rearrange("(p r) d -> p r d", p=P)
    xi_v = xi_d[:, :].rearrange("(p r) d -> p r d", p=P)

    mul_op = mybir.AluOpType.mult
    add_op = mybir.AluOpType.add
    IOr = IO - 120  # im col offsets within the im triangle

    with TileContext(nc) as tc:
        with (
            tc.tile_pool(name="io", bufs=2) as iop,
            tc.tile_pool(name="km", bufs=2) as kmp,
            tc.tile_pool(name="ident", bufs=1) as idp,
            tc.tile_pool(name="zz", bufs=2) as zzp,
            tc.tile_pool(name="scr", bufs=2) as scp,
            tc.tile_pool(name="k1p", bufs=ring_bufs) as k1p,
            tc.tile_pool(name="k2p", bufs=ring_bufs) as k2p,
            tc.tile_pool(name="k3p", bufs=ring_bufs) as k3p,
            tc.psum_pool(name="ps", bufs=psum_bufs) as psp,
            tc.tile_pool(name="outk", bufs=2) as okp,
        ):
            ident = idp.tile([P, P], F16, tag="I")
            nc.sync.dma_start(out=ident[:], in_=id_d[:, :])

            def load_tile(R, r0):
                """Input DMA + Act transpose-cast for one tile; emitted one
                tile ahead so Act's in-order queue never parks the next
                tile's transposes behind this tile's PSUM copies."""
                xr_s = iop.tile([P, R * D], F32, tag="xr")
                xi_s = iop.tile([P, R * D], F32, tag="xi")
                nc.sync.dma_start(
                    out=xr_s[:].rearrange("p (r d) -> p r d", d=D),
                    in_=xr_v[:, r0 : r0 + R, :],
                )
                nc.sync.dma_start(
                    out=xi_s[:].rearrange("p (r d) -> p r d", d=D),
                    in_=xi_v[:, r0 : r0 + R, :],
                )
                xr_k = kmp.tile([P, D2 * R], F16, tag="xrk")
                xi_k = kmp.tile([P, D2 * R], F16, tag="xik")
                nc.scalar.copy(
                    out=xr_k[:].rearrange("p (d r) -> p d r", d=D2)[:, :D, :],
                    in_=xr_s[:].rearrange("p (r d) -> p r d", d=D).transpose([0, 2, 1]),
                )
                nc.scalar.copy(
                    out=xi_k[:].rearrange("p (d r) -> p d r", d=D2)[:, :D, :],
                    in_=xi_s[:].rearrange("p (r d) -> p r d", d=D).transpose([0, 2, 1]),
                )
                return xr_k, xi_k

            offs = []
            _o = 0
            for _R in tile_rs:
                offs.append(_o)
                _o += _R
            loaded = load_tile(tile_rs[0], offs[0])
            for ti, R in enumerate(tile_rs):
                assert R % 2 == 0
                r0 = offs[ti]
                xr_k, xi_k = loaded
                if ti + 1 < len(tile_rs):
                    loaded = load_tile(tile_rs[ti + 1], offs[ti + 1])
                xr3 = xr_k[:].rearrange("p (d r) -> p d r", d=D2)
                xi3 = xi_k[:].rearrange("p (d r) -> p d r", d=D2)

                ar_t = zzp.tile([P, KP * R], F16, tag="ar")
                ai_t = zzp.tile([P, KP * R], F16, tag="ai")
                ar3 = ar_t[:].rearrange("p (k r) -> p k r", k=KP)
                ai3 = ai_t[:].rearrange("p (k r) -> p k r", k=KP)

                out_k = okp.tile([P, NP * R], F16, tag="outk")
                ok3 = out_k[:].rearrange("p (c r) -> p c r", c=NP)

                v_t = zzp.tile([P, KP * R], F16, tag="vt")
                u2_t = zzp.tile([P, KP * R], F16, tag="ut")
                an_t = zzp.tile([P, KP * R], F16, tag="an")
                v3 = v_t[:].rearrange("p (k r) -> p k r", k=KP)
                an3 = an_t[:].rearrange("p (k r) -> p k r", k=KP)

                def vua(c0, c1):
                    """v = ar+ai, u2 = ai-ar, an = -ar for zz cols [c0, c1) —
                    emitted right after the step-1 block producing them so
                    step-2 work on high columns can start early."""
                    sl = slice(c0 * R, c1 * R)
                    nc.vector.tensor_add(out=v_t[:, sl], in0=ar_t[:, sl], in1=ai_t[:, sl])
                    nc.vector.tensor_sub(out=u2_t[:, sl], in0=ai_t[:, sl], in1=ar_t[:, sl])
                    nc.vector.tensor_scalar_mul(an_t[:, sl], ar_t[:, sl], -1.0)

                # ---- step 1, high zz columns first (single a=4, then pair
                # (2,3), then pair (0,1)), v/u/an produced incrementally ----
                a = 4
                bra = xr3[:, a : a + 1, :]
                bia = xi3[:, a : a + 1, :]
                s1 = scp.tile([P, 2 * D * R], F16, tag="s1")
                s2 = scp.tile([P, 2 * D * R], F16, tag="s2")
                s1v = s1[:, :R].unsqueeze(1)
                s2v = s2[:, :R].unsqueeze(1)
                nc.vector.tensor_mul(out=s1v, in0=bra, in1=xr3[:, a : a + 1, :])
                nc.vector.tensor_mul(out=s2v, in0=bia, in1=xi3[:, a : a + 1, :])
                nc.vector.tensor_sub(out=ar3[:, 14:15, :], in0=s1v, in1=s2v)
                s3 = scp.tile([P, 2 * D * R], F16, tag="s3")
                s4 = scp.tile([P, 2 * D * R], F16, tag="s4")
                s3v = s3[:, :R].unsqueeze(1)
                s4v = s4[:, :R].unsqueeze(1)
                nc.vector.tensor_mul(out=s3v, in0=bra, in1=xi3[:, a : a + 1, :])
                nc.vector.tensor_mul(out=s4v, in0=bia, in1=xr3[:, a : a + 1, :])
                nc.vector.tensor_add(out=ai3[:, 14:15, :], in0=s3v, in1=s4v)
                vua(14, KP)  # col 15 pad reads junk ar/ai: never consumed

                # single a=14 mults right away (re col 119 inputs)
                k1s = k1p.tile([P, 2 * K * R], F16, tag="m1")
                k3s = k3p.tile([P, 2 * K * R], F16, tag="m3")
                nc.vector.tensor_mul(
                    out=k1s[:, :R].unsqueeze(1), in0=v3[:, 14:15, :], in1=ar3[:, 14:15, :]
                )
                nc.vector.tensor_mul(
                    out=k3s[:, :R].unsqueeze(1),
                    in0=ai3[:, 14:15, :],
                    in1=u2_t[:].rearrange("p (k r) -> p k r", k=KP)[:, 14:15, :],
                )

                for a in (2, 0):
                    L = D - a
                    b_r = xr3[:, a : a + 2, :].unsqueeze(2).broadcast_to([P, 2, L, R])
                    b_i = xi3[:, a : a + 2, :].unsqueeze(2).broadcast_to([P, 2, L, R])
                    w_r = _ap4(xr_k[:], a, 1, L, R)
                    w_i = _ap4(xi_k[:], a, 1, L, R)
                    s1 = scp.tile([P, 2 * D * R], F16, tag="s1")
                    s2 = scp.tile([P, 2 * D * R], F16, tag="s2")
                    s3 = scp.tile([P, 2 * D * R], F16, tag="s3")
                    s4 = scp.tile([P, 2 * D * R], F16, tag="s4")
                    s1v = s1[:, : 2 * L * R].rearrange("p (j k r) -> p j k r", j=2, k=L)
                    s2v = s2[:, : 2 * L * R].rearrange("p (j k r) -> p j k r", j=2, k=L)
                    s3v = s3[:, : 2 * L * R].rearrange("p (j k r) -> p j k r", j=2, k=L)
                    s4v = s4[:, : 2 * L * R].rearrange("p (j k r) -> p j k r", j=2, k=L)
                    nc.vector.tensor_mul(out=s1v, in0=b_r, in1=w_r)
                    nc.vector.tensor_mul(out=s2v, in0=b_i, in1=w_i)
                    nc.vector.tensor_mul(out=s3v, in0=b_r, in1=w_i)
                    nc.vector.tensor_mul(out=s4v, in0=b_i, in1=w_r)
                    # exact-length combines per group: the paired scratch's
                    # j=1 run has a junk tail column that must not reach zz
                    # (later groups are already written in descending order)
                    def _j0(t):
                        return t[:, : L * R].rearrange("p (k r) -> p k r", k=L)

                    def _j1(t):
                        return t[:, L * R : (2 * L - 1) * R].rearrange(
                            "p (k r) -> p k r", k=L - 1
                        )

                    nc.vector.tensor_sub(
                        out=ar3[:, O1[a] : O1[a] + L, :], in0=_j0(s1), in1=_j0(s2)
                    )
                    nc.vector.tensor_sub(
                        out=ar3[:, O1[a + 1] : O1[a + 1] + L - 1, :],
                        in0=_j1(s1), in1=_j1(s2),
                    )
                    nc.vector.tensor_add(
                        out=ai3[:, O1[a] : O1[a] + L, :], in0=_j0(s3), in1=_j0(s4)
                    )
                    nc.vector.tensor_add(
                        out=ai3[:, O1[a + 1] : O1[a + 1] + L - 1, :],
                        in0=_j1(s3), in1=_j1(s4),
                    )
                    vua(O1[a], O1[a + 2] if a + 2 < D else 14)

                # ---- step 2 mults, pair-grouped, descending (high pairs
                # depend only on high zz cols -> unlock earliest) ----
                pair_tiles = {}
                for a in range(K - 3, -1, -2):
                    L = K - a
                    L2 = L - 1
                    b_v = v3[:, a : a + 2, :].unsqueeze(2).broadcast_to([P, 2, L, R])
                    b_ai = ai3[:, a : a + 2, :].unsqueeze(2).broadcast_to([P, 2, L, R])
                    w_ar = _ap4(ar_t[:], a, 1, L, R)
                    w_u = _ap4(u2_t[:], a, 1, L, R)
                    w_v = _ap4(v_t[:], a + 1, 1, L2, R)
                    k1 = k1p.tile([P, 2 * K * R], F16, tag="m1")
                    k2 = k2p.tile([P, 2 * K * R], F16, tag="m2")
                    k3 = k3p.tile([P, 2 * K * R], F16, tag="m3")
                    k1v = k1[:, : 2 * L * R].rearrange("p (j k r) -> p j k r", j=2, k=L)
                    k3v = k3[:, : 2 * L * R].rearrange("p (j k r) -> p j k r", j=2, k=L)
                    # k2' = (-ar)_a * v[b] on Pool (plain TT mult; the only
                    # tensor op the Pool engine supports on hw)
                    b_an = an3[:, a : a + 2, :].unsqueeze(2).broadcast_to([P, 2, L2, R])
                    k2v = k2[:, : 2 * L2 * R].rearrange("p (j k r) -> p j k r", j=2, k=L2)
                    nc.gpsimd.tensor_mul(out=k2v, in0=b_an, in1=w_v)
                    nc.vector.tensor_mul(out=k1v, in0=b_v, in1=w_ar)
                    nc.vector.tensor_mul(out=k3v, in0=b_ai, in1=w_u)
                    pair_tiles[a] = (k1, k2, k3, L, L2)

                # ---- combines ----
                def pe_accum(dst_off, n_el, rhs1, rhs2):
                    """PSUM-accumulate rhs1+rhs2 (each [P, n_el] fp16 slices)
                    into out_k[:, dst_off : dst_off + n_el], in psb-sized
                    super-chunks each finished by one Act convert-copy."""
                    s0 = 0
                    while s0 < n_el:
                        se = min(s0 + psb, n_el)
                        ps = psp.tile([P, psb], F32, tag="ps")
                        c0 = s0
                        while c0 < se:
                            ce = min(c0 + chunk, se)
                            nc.tensor.matmul(
                                out=ps[:, c0 - s0 : ce - s0], lhsT=ident[:],
                                rhs=rhs1[:, c0:ce], start=True, stop=False,
                            )
                            nc.tensor.matmul(
                                out=ps[:, c0 - s0 : ce - s0], lhsT=ident[:],
                                rhs=rhs2[:, c0:ce], start=False, stop=True,
                            )
                            c0 = ce
                        nc.scalar.copy(
                            out=out_k[:, dst_off + s0 : dst_off + se],
                            in_=ps[:, : se - s0],
                        )
                        s0 = se

                # combines, descending, re/im interleaved per pair: aligned
                # with the descending mult production so every consumer's
                # inputs are the producers' most recent outputs
                pe_accum(119 * R, R, k1s[:, :R], k3s[:, :R])
                for a in range(K - 3, -1, -2):
                    k1, k2, k3, L, L2 = pair_tiles[a]
                    # re pair: k1 + k3' over the contiguous (2L-1) col region
                    # (excludes the pair tile's final junk column)
                    n_re = (2 * L - 1) * R
                    pe_accum(RO[a] * R, n_re, k1[:, :n_re], k3[:, :n_re])
                    # im pair: k1[1:] + k2' ; groups a (L2 cols) and a+1 (L2-1)
                    asn = im_asn.get(a, "dve")
                    if asn == "pe":
                        pe_accum(IO[a] * R, L2 * R, k1[:, R : L * R], k2[:, : L2 * R])
                        if L2 - 1 > 0:
                            pe_accum(
                                IO[a + 1] * R, (L2 - 1) * R,
                                k1[:, L * R + R : L * R + (L - 1) * R],
                                k2[:, L2 * R : L2 * R + (L2 - 1) * R],
                            )
                    elif asn == "pool":
                        nc.gpsimd.tensor_add(
                            out=ok3[:, IO[a] : IO[a] + L2, :],
                            in0=k1[:, R : L * R].rearrange("p (k r) -> p k r", k=L2),
                            in1=k2[:, : L2 * R].rearrange("p (k r) -> p k r", k=L2),
                        )
                        if L2 - 1 > 0:
                            nc.gpsimd.tensor_add(
                                out=ok3[:, IO[a + 1] : IO[a + 1] + L2 - 1, :],
                                in0=k1[:, L * R + R : L * R + (L - 1) * R].rearrange(
                                    "p (k r) -> p k r", k=L2 - 1
                                ),
                                in1=k2[:, L2 * R : L2 * R + (L2 - 1) * R].rearrange(
                                    "p (k r) -> p k r", k=L2 - 1
                                ),
                            )
                    else:
                        nc.vector.tensor_add(
                            out=ok3[:, IO[a] : IO[a] + L2, :],
                            in0=k1[:, R : L * R].rearrange("p (k r) -> p k r", k=L2),
                            in1=k2[:, : L2 * R].rearrange("p (k r) -> p k r", k=L2),
                        )
                        if L2 - 1 > 0:
                            nc.vector.tensor_add(
                                out=ok3[:, IO[a + 1] : IO[a + 1] + L2 - 1, :],
                                in0=k1[:, L * R + R : L * R + (L - 1) * R].rearrange(
                                    "p (k r) -> p k r", k=L2 - 1
                                ),
                                in1=k2[:, L2 * R : L2 * R + (L2 - 1) * R].rearrange(
                                    "p (k r) -> p k r", k=L2 - 1
                                ),
                            )

                # ---- direct k-major fp16 DMA out ----
                nc.sync.dma_start(
                    out=out_d[:, r0 * NOUT : (r0 + R) * NOUT],
                    in_=out_k[:, : NOUT * R],
                )

    _split_excess_waits(nc)
    return nc


_CACHE = {}


def _make_tiles(rt, r_max, first=0, last=0):
    """Split rt rows into tiles of r_max with optional small first/last
    tiles (fast pipeline fill/drain)."""
    rem = rt - first - last
    tiles = ([first] if first else []) + [r_max] * (rem // r_max)
    r = rem % r_max
    if r:
        tiles.append(r)
    if last:
        tiles.append(last)
    assert sum(tiles) == rt and all(t % 2 == 0 for t in tiles)
    return tiles


def _get_program(n):
    """Geometry + compiled program for total row count n.
    Returns (nc, n_c, rt, tile_rs)."""
    key = (n, PRECISION)
    if key in _CACHE:
        return _CACHE[key]
    per_core = -(-n // N_CORES)              # ceil
    rt = -(-per_core // P)                   # rows per partition
    if PRECISION == "fp16_pe":
        rt += rt % 2
        n_c = P * rt
        tile_rs = _make_tiles(rt, 72, first=32, last=32)
        nc = _build_fp16_pe(n_c, rt, tile_rs)
    elif PRECISION == "fp16":
        rt += rt % 2                         # even rt (fp16 4B alignment needs even R only)
        n_c = P * rt
        r_max = 100                          # divisible by 4; best per cost-model sweep
        tile_rs = [r_max] * (rt // r_max)
        if rt % r_max:
            tile_rs.append(rt % r_max)
        nc = _build_fp16_kmout(n_c, rt, tile_rs)
    else:
        n_c = P * rt
        r_max = 64
        tile_rs = [r_max] * (rt // r_max)
        if rt % r_max:
            tile_rs.append(rt % r_max)
        nc = _build(n_c, rt, tile_rs)
    _CACHE[key] = (nc, n_c, rt, tile_rs)
    return _CACHE[key]


def kernel(x_re, x_im, _trace=False):
    x_re = np.ascontiguousarray(np.asarray(x_re), dtype=np.float32)
    x_im = np.ascontiguousarray(np.asarray(x_im), dtype=np.float32)
    n = x_re.shape[0]
    nc, n_c, rt, tile_rs = _get_program(n)
    n_pad = n_c * N_CORES
    if n_pad != n:
        pad = np.zeros((n_pad - n, D), dtype=np.float32)
        xr = np.concatenate([x_re, pad], axis=0)
        xi = np.concatenate([x_im, pad], axis=0)
    else:
        xr, xi = x_re, x_im
    xr_sh = xr.reshape(N_CORES, n_c, D)
    xi_sh = xi.reshape(N_CORES, n_c, D)
    in_maps = [
        {"x_re": np.ascontiguousarray(xr_sh[i]), "x_im": np.ascontiguousarray(xi_sh[i])}
        for i in range(N_CORES)
    ]
    if PRECISION == "fp16_pe":
        ident = np.eye(P, dtype=np.float16)
        for m in in_maps:
            m["ident"] = ident
    res = bass_utils.run_bass_kernel_spmd(
        nc, in_maps, core_ids=list(range(N_CORES)), trace=_trace
    )
    if PRECISION in ("fp16", "fp16_pe"):
        # device output is [P, rt*NOUT] fp16, per tile [col][row-in-tile]
        # (k-major); de-interleave to [n_c, NOUT] and upcast per core.
        out = np.empty((n_pad, NOUT), dtype=np.float32)
        for i, r in enumerate(res.results):
            raw = r["out"]  # [P, rt*NOUT] fp16
            core_out = out[i * n_c : (i + 1) * n_c].reshape(P, rt, NOUT)
            r0 = 0
            for R in tile_rs:
                blk = raw[:, r0 * NOUT : (r0 + R) * NOUT].reshape(P, NOUT, R)
                core_out[:, r0 : r0 + R, :] = blk.transpose(0, 2, 1)
                r0 += R
        out = out[:n]
    else:
        out = np.concatenate([r["out"] for r in res.results], axis=0)[:n]
    if _trace:
        return out, res
    return out



# revision 37
# speedup vs baseline: 1.4475x; 1.0032x over previous
"""Trainium2 Bass kernel for nn_Biholomorphic_k2.

Per row (N=1e6 rows, D=5):
  z = x_re + i*x_im                                  [5] complex
  zz = z[i5] * z[j5]          (triu pairs of 5)      [15] complex
  prod = zz[i15] * conj(zz[j15])  (triu pairs of 15) [120] complex
  out = [Re(prod) (120 cols), Im(prod offdiag) (105 cols)]  [225] f32

Sharding: pure data parallel over 8 cores; rows on SBUF partitions in
contiguous blocks (partition p of core c owns rows [c*NC + p*RT, ...)),
features along the free dim so every DMA is fully contiguous.

All compute is VectorE tensor_tensor ops with stride-0 (broadcast) access
patterns on the "a"-side operand; results are written directly into the
packed [128, R*225] output tile, so there is no separate gather step.
"""

import sys

import numpy as np

try:
    import concourse.bass as bass
except ImportError:
    for _p in ("/opt/trn_rl_repo", "/root/.axon_site/_ro/trn_rl_repo"):
        if _p not in sys.path:
            sys.path.insert(0, _p)
    import concourse.bass as bass
import concourse.mybir as mybir
from concourse.tile import TileContext
from concourse import bass_utils

P = 128          # SBUF partitions
D = 5
K = 15           # triu pairs of 5
NOUT = 225       # 120 re + 105 im
N_CORES = 8

# --- static index tables (row-major triu, matching np.triu_indices) ---
# step-1: for a in 0..4 produce zz[o1[a] : o1[a]+5-a] = z[a] * z[a:5]
O1 = [0, 5, 9, 12, 14]
# step-2 re: for a in 0..14, out[ro[a] : ro[a]+15-a] = Re(zz[a] * conj(zz[a:15]))
RO = np.concatenate([[0], np.cumsum([15 - a for a in range(15)])]).astype(int)
# step-2 im: for a in 0..13, out[120+io[a] : ...+14-a] = Im(zz[a] * conj(zz[a+1:15]))
IO = 120 + np.concatenate([[0], np.cumsum([14 - a for a in range(14)])]).astype(int)

F32 = mybir.dt.float32
F16 = mybir.dt.float16

# "fp16": k-major fp16 compute (2x DVE mode), ~7e-4 rel error.
# "fp32": row-major fp32 compute, ~6e-8 rel error, ~2x slower.
PRECISION = "fp16_pe"
# route the step-2 im combines to GPSIMD (overlaps with DVE; shares one of
# DVE's two SBUF ports -- cost model says net win)
GPSIMD_IM = True
# additionally route re combines for these pair-start values to GPSIMD
GP_RE_PAIRS = ()
# pair-start values whose k2 mult runs on GPSIMD (fp16-kmout build)
GP_K2_PAIRS = (0, 2, 4, 6)
# pair-start values whose im combine runs on GPSIMD (fp16-kmout build)
GP_IM_PAIRS = (0, 2, 4, 6, 8, 10, 12)

_MAX_CTRL_WAITS = 1


def _split_excess_waits(nc):
    """Workaround: this walrus build rejects Drain instructions carrying
    more than one sync wait ("Too many sync wait commands").  Move excess
    waits onto NOPs inserted immediately before, on the same engine."""
    engmap = {
        mybir.EngineType.SP: nc.sync,
        mybir.EngineType.DVE: nc.vector,
        mybir.EngineType.Activation: nc.scalar,
        mybir.EngineType.PE: nc.tensor,
        mybir.EngineType.Pool: nc.gpsimd,
    }
    for f in nc.m.functions:
        for blk in f.blocks:
            newlist = []
            for inst in blk.instructions:
                si = inst.sync_info
                if (
                    si is not None
                    and si.on_wait is not None
                    and len(si.on_wait) > _MAX_CTRL_WAITS
                ):
                    waits = list(si.on_wait)
                    head = waits[:-_MAX_CTRL_WAITS]
                    tail = waits[-_MAX_CTRL_WAITS:]
                    for s in range(0, len(head), _MAX_CTRL_WAITS):
                        chunk = head[s : s + _MAX_CTRL_WAITS]
                        bi = engmap[inst.engine].nop()
                        nop_inst = bi.ins if hasattr(bi, "ins") else bi
                        for b2 in f.blocks:
                            if nop_inst in b2.instructions:
                                b2.instructions.remove(nop_inst)
                        nop_inst.sync_info = mybir.SyncInfo(on_wait=chunk, on_update=[])
                        nop_inst.engine = inst.engine
                        newlist.append(nop_inst)
                    inst.sync_info = mybir.SyncInfo(
                        on_wait=tail, on_update=list(si.on_update or [])
                    )
                newlist.append(inst)
            blk.instructions[:] = newlist


def _build(n_c, rt, tile_rs):
    """Build the Bass program for one core's shard: [n_c, 5] x2 -> [n_c, 225].
    n_c = P * rt rows; processed in free-dim chunks of R rows/partition."""
    nc = bass.Bass()
    xr_d = nc.dram_tensor("x_re", [n_c, D], F32, kind="ExternalInput")
    xi_d = nc.dram_tensor("x_im", [n_c, D], F32, kind="ExternalInput")
    out_d = nc.dram_tensor("out", [n_c, NOUT], F32, kind="ExternalOutput")

    xr_v = xr_d[:, :].rearrange("(p r) d -> p r d", p=P)     # [128, rt, 5]
    xi_v = xi_d[:, :].rearrange("(p r) d -> p r d", p=P)
    out_v = out_d[:, :].rearrange("(p r) c -> p r c", p=P)   # [128, rt, 225]

    with TileContext(nc) as tc:
        with (
            tc.tile_pool(name="io", bufs=3) as iop,
            tc.tile_pool(name="zz", bufs=2) as zzp,
            tc.tile_pool(name="scr", bufs=2) as scp,
            tc.tile_pool(name="outp", bufs=2) as outp,
        ):
            r0 = 0
            for R in tile_rs:
                xr_t = iop.tile([P, R * D], F32, tag="xr")
                xi_t = iop.tile([P, R * D], F32, tag="xi")
                nc.sync.dma_start(
                    out=xr_t[:].rearrange("p (r d) -> p r d", d=D),
                    in_=xr_v[:, r0 : r0 + R, :],
                )
                nc.sync.dma_start(
                    out=xi_t[:].rearrange("p (r d) -> p r d", d=D),
                    in_=xi_v[:, r0 : r0 + R, :],
                )
                xr3 = xr_t[:].rearrange("p (r d) -> p r d", d=D)   # [128,R,5]
                xi3 = xi_t[:].rearrange("p (r d) -> p r d", d=D)

                ar_t = zzp.tile([P, R * K], F32, tag="ar")
                ai_t = zzp.tile([P, R * K], F32, tag="ai")
                ar3 = ar_t[:].rearrange("p (r k) -> p r k", k=K)   # [128,R,15]
                ai3 = ai_t[:].rearrange("p (r k) -> p r k", k=K)

                # ---- step 1: zz = z[a] * z[a:5] for a in 0..4 ----
                for a in range(D):
                    L = D - a
                    o = O1[a]
                    s1 = scp.tile([P, R * D], F32, tag="s1")
                    s2 = scp.tile([P, R * D], F32, tag="s2")
                    s1v = s1[:].rearrange("p (r d) -> p r d", d=D)[:, :, :L]
                    s2v = s2[:].rearrange("p (r d) -> p r d", d=D)[:, :, :L]
                    bra = xr3[:, :, a : a + 1].broadcast_to([P, R, L])
                    bia = xi3[:, :, a : a + 1].broadcast_to([P, R, L])
                    # re: xr_a*xr_b - xi_a*xi_b
                    nc.vector.tensor_mul(out=s1v, in0=bra, in1=xr3[:, :, a:D])
                    nc.vector.tensor_mul(out=s2v, in0=bia, in1=xi3[:, :, a:D])
                    nc.vector.tensor_sub(
                        out=ar3[:, :, o : o + L], in0=s1v, in1=s2v
                    )
                    # im: xr_a*xi_b + xi_a*xr_b
                    s3 = scp.tile([P, R * D], F32, tag="s3")
                    s4 = scp.tile([P, R * D], F32, tag="s4")
                    s3v = s3[:].rearrange("p (r d) -> p r d", d=D)[:, :, :L]
                    s4v = s4[:].rearrange("p (r d) -> p r d", d=D)[:, :, :L]
                    nc.vector.tensor_mul(out=s3v, in0=bra, in1=xi3[:, :, a:D])
                    nc.vector.tensor_mul(out=s4v, in0=bia, in1=xr3[:, :, a:D])
                    nc.vector.tensor_add(
                        out=ai3[:, :, o : o + L], in0=s3v, in1=s4v
                    )

                out_t = outp.tile([P, R * NOUT], F32, tag="out")
                out3 = out_t[:].rearrange("p (r c) -> p r c", c=NOUT)

                # ---- step 2: prod = zz[a] * conj(zz[b]), b >= a ----
                for a in range(K):
                    L = K - a
                    bar = ar3[:, :, a : a + 1].broadcast_to([P, R, L])
                    bai = ai3[:, :, a : a + 1].broadcast_to([P, R, L])
                    m1 = scp.tile([P, R * K], F32, tag="m1")
                    m2 = scp.tile([P, R * K], F32, tag="m2")
                    m1v = m1[:].rearrange("p (r k) -> p r k", k=K)[:, :, :L]
                    m2v = m2[:].rearrange("p (r k) -> p r k", k=K)[:, :, :L]
                    # re: ar_a*ar_b + ai_a*ai_b
                    nc.vector.tensor_mul(out=m1v, in0=bar, in1=ar3[:, :, a:K])
                    nc.vector.tensor_mul(out=m2v, in0=bai, in1=ai3[:, :, a:K])
                    ro = int(RO[a])
                    nc.vector.tensor_add(
                        out=out3[:, :, ro : ro + L], in0=m1v, in1=m2v
                    )
                    # im (offdiag only): ai_a*ar_b - ar_a*ai_b
                    if a < K - 1:
                        L2 = L - 1
                        m3 = scp.tile([P, R * K], F32, tag="m3")
                        m4 = scp.tile([P, R * K], F32, tag="m4")
                        m3v = m3[:].rearrange("p (r k) -> p r k", k=K)[:, :, :L2]
                        m4v = m4[:].rearrange("p (r k) -> p r k", k=K)[:, :, :L2]
                        bar2 = ar3[:, :, a : a + 1].broadcast_to([P, R, L2])
                        bai2 = ai3[:, :, a : a + 1].broadcast_to([P, R, L2])
                        nc.vector.tensor_mul(
                            out=m3v, in0=bai2, in1=ar3[:, :, a + 1 : K]
                        )
                        nc.vector.tensor_mul(
                            out=m4v, in0=bar2, in1=ai3[:, :, a + 1 : K]
                        )
                        io = int(IO[a])
                        nc.vector.tensor_sub(
                            out=out3[:, :, io : io + L2], in0=m3v, in1=m4v
                        )

                nc.sync.dma_start(out=out_v[:, r0 : r0 + R, :], in_=out3)
                r0 += R

    _split_excess_waits(nc)
    return nc


def _build_fp16(n_c, rt, tile_rs):
    """k-major fp16 build: within each partition, every tensor is stored
    feature-major ([k, r] with r innermost, step 1) so all DVE tensor_tensor
    operands have a 2-byte dtype, innermost step 1, and 4B-aligned run starts
    (R even) -> 2x_1p DVE mode throughout.  ScalarE does the fp32->fp16
    transpose-in and the fp16->fp32 transpose-out (its own SBUF ports, 1x).
    """
    KP = 16  # zz column padding (alignment headroom)
    nc = bass.Bass()
    xr_d = nc.dram_tensor("x_re", [n_c, D], F32, kind="ExternalInput")
    xi_d = nc.dram_tensor("x_im", [n_c, D], F32, kind="ExternalInput")
    out_d = nc.dram_tensor("out", [n_c, NOUT], F32, kind="ExternalOutput")

    xr_v = xr_d[:, :].rearrange("(p r) d -> p r d", p=P)     # [128, rt, 5]
    xi_v = xi_d[:, :].rearrange("(p r) d -> p r d", p=P)
    out_v = out_d[:, :].rearrange("(p r) c -> p r c", p=P)   # [128, rt, 225]

    with TileContext(nc) as tc:
        with (
            tc.tile_pool(name="io", bufs=3) as iop,
            tc.tile_pool(name="km", bufs=2) as kmp,
            tc.tile_pool(name="zz", bufs=2) as zzp,
            tc.tile_pool(name="scr", bufs=2) as scp,
            tc.tile_pool(name="outk", bufs=2) as okp,
            tc.tile_pool(name="outf", bufs=2) as ofp,
        ):
            r0 = 0
            for R in tile_rs:
                assert R % 2 == 0
                R2 = R // 2
                xr_s = iop.tile([P, R * D], F32, tag="xr")
                xi_s = iop.tile([P, R * D], F32, tag="xi")
                nc.sync.dma_start(
                    out=xr_s[:].rearrange("p (r d) -> p r d", d=D),
                    in_=xr_v[:, r0 : r0 + R, :],
                )
                nc.sync.dma_start(
                    out=xi_s[:].rearrange("p (r d) -> p r d", d=D),
                    in_=xi_v[:, r0 : r0 + R, :],
                )
                # ScalarE: cast fp32->fp16 + transpose row-major -> k-major
                xr_k = kmp.tile([P, D * R], F16, tag="xrk")
                xi_k = kmp.tile([P, D * R], F16, tag="xik")
                # src [r, d] -> view [d, r]
                nc.scalar.copy(
                    out=xr_k[:].rearrange("p (d r) -> p d r", d=D),
                    in_=xr_s[:].rearrange("p (r d) -> p r d", d=D).transpose([0, 2, 1]),
                )
                nc.scalar.copy(
                    out=xi_k[:].rearrange("p (d r) -> p d r", d=D),
                    in_=xi_s[:].rearrange("p (r d) -> p r d", d=D).transpose([0, 2, 1]),
                )
                xr3 = xr_k[:].rearrange("p (d r) -> p d r", d=D)   # [128,5,R]
                xi3 = xi_k[:].rearrange("p (d r) -> p d r", d=D)

                ar_t = zzp.tile([P, KP * R], F16, tag="ar")
                ai_t = zzp.tile([P, KP * R], F16, tag="ai")
                ar3 = ar_t[:].rearrange("p (k r) -> p k r", k=KP)  # [128,16,R]
                ai3 = ai_t[:].rearrange("p (k r) -> p k r", k=KP)

                # ---- step 1: zz[o1[a]:o1[a]+L] = z[a] * z[a:5] ----
                for a in range(D):
                    L = D - a
                    o = O1[a]
                    bra = xr3[:, a : a + 1, :].broadcast_to([P, L, R])
                    bia = xi3[:, a : a + 1, :].broadcast_to([P, L, R])
                    s1 = scp.tile([P, D * R], F16, tag="s1")
                    s2 = scp.tile([P, D * R], F16, tag="s2")
                    s1v = s1[:].rearrange("p (k r) -> p k r", k=D)[:, :L, :]
                    s2v = s2[:].rearrange("p (k r) -> p k r", k=D)[:, :L, :]
                    nc.vector.tensor_mul(out=s1v, in0=bra, in1=xr3[:, a:D, :])
                    nc.vector.tensor_mul(out=s2v, in0=bia, in1=xi3[:, a:D, :])
                    nc.vector.tensor_sub(out=ar3[:, o : o + L, :], in0=s1v, in1=s2v)
                    s3 = scp.tile([P, D * R], F16, tag="s3")
                    s4 = scp.tile([P, D * R], F16, tag="s4")
                    s3v = s3[:].rearrange("p (k r) -> p k r", k=D)[:, :L, :]
                    s4v = s4[:].rearrange("p (k r) -> p k r", k=D)[:, :L, :]
                    nc.vector.tensor_mul(out=s3v, in0=bra, in1=xi3[:, a:D, :])
                    nc.vector.tensor_mul(out=s4v, in0=bia, in1=xr3[:, a:D, :])
                    nc.vector.tensor_add(out=ai3[:, o : o + L, :], in0=s3v, in1=s4v)

                out_k = okp.tile([P, NOUT * R], F16, tag="outk")
                ok3 = out_k[:].rearrange("p (c r) -> p c r", c=NOUT)  # [128,225,R]

                # ---- step 2 ----
                for a in range(K):
                    L = K - a
                    bar = ar3[:, a : a + 1, :].broadcast_to([P, L, R])
                    bai = ai3[:, a : a + 1, :].broadcast_to([P, L, R])
                    m1 = scp.tile([P, KP * R], F16, tag="m1")
                    m2 = scp.tile([P, KP * R], F16, tag="m2")
                    m1v = m1[:].rearrange("p (k r) -> p k r", k=KP)[:, :L, :]
                    m2v = m2[:].rearrange("p (k r) -> p k r", k=KP)[:, :L, :]
                    nc.vector.tensor_mul(out=m1v, in0=bar, in1=ar3[:, a:K, :])
                    nc.vector.tensor_mul(out=m2v, in0=bai, in1=ai3[:, a:K, :])
                    ro = int(RO[a])
                    nc.vector.tensor_add(out=ok3[:, ro : ro + L, :], in0=m1v, in1=m2v)
                    if a < K - 1:
                        L2 = L - 1
                        bar2 = ar3[:, a : a + 1, :].broadcast_to([P, L2, R])
                        bai2 = ai3[:, a : a + 1, :].broadcast_to([P, L2, R])
                        m3 = scp.tile([P, KP * R], F16, tag="m3")
                        m4 = scp.tile([P, KP * R], F16, tag="m4")
                        m3v = m3[:].rearrange("p (k r) -> p k r", k=KP)[:, :L2, :]
                        m4v = m4[:].rearrange("p (k r) -> p k r", k=KP)[:, :L2, :]
                        nc.vector.tensor_mul(out=m3v, in0=bai2, in1=ar3[:, a + 1 : K, :])
                        nc.vector.tensor_mul(out=m4v, in0=bar2, in1=ai3[:, a + 1 : K, :])
                        io = int(IO[a])
                        nc.vector.tensor_sub(
                            out=ok3[:, io : io + L2, :], in0=m3v, in1=m4v
                        )

                # ---- ScalarE: fp16 k-major -> fp32 row-major, in row-chunks ----
                n_chunks = 4 if R % 4 == 0 else 2
                Rc = R // n_chunks
                for h in range(n_chunks):
                    of = ofp.tile([P, Rc * NOUT], F32, tag="outf")
                    of3 = of[:].rearrange("p (r c) -> p r c", c=NOUT)  # [128,Rc,225]
                    src = ok3[:, :, h * Rc : (h + 1) * Rc].transpose([0, 2, 1])
                    nc.scalar.copy(out=of3, in_=src)
                    nc.sync.dma_start(
                        out=out_v[:, r0 + h * Rc : r0 + (h + 1) * Rc, :], in_=of3
                    )
                r0 += R

    _split_excess_waits(nc)
    return nc


def _ap4(t2d, col_off, jstep_cols, L, R):
    """4-dim AP over a k-major [cols x R] SBUF tile view: two runs (j=0,1)
    of L columns x R rows, run j starting at column col_off + j*jstep_cols."""
    pdim = list(t2d.ap[0])
    return bass.AP(
        t2d.tensor,
        t2d.offset + col_off * R,
        [pdim, [jstep_cols * R, 2], [R, L], [1, R]],
    )


def _build_fp16_paired(n_c, rt, tile_rs):
    """Like _build_fp16 but batches consecutive-a groups in pairs via 4-dim
    APs, halving DVE instruction count.  The second run of each pair reads one
    padded junk column and writes one column past its end; emission order
    guarantees a later group rewrites the overshoot column with real data."""
    KP = 16       # zz padded to 16 cols (col 15 = junk read by pair overshoot)
    D2 = 6        # input padded to 6 cols (col 5 = junk)
    NP = NOUT + 1 # out_k padded by 1 col for im-pair overshoot
    nc = bass.Bass()
    xr_d = nc.dram_tensor("x_re", [n_c, D], F32, kind="ExternalInput")
    xi_d = nc.dram_tensor("x_im", [n_c, D], F32, kind="ExternalInput")
    out_d = nc.dram_tensor("out", [n_c, NOUT], F32, kind="ExternalOutput")

    xr_v = xr_d[:, :].rearrange("(p r) d -> p r d", p=P)
    xi_v = xi_d[:, :].rearrange("(p r) d -> p r d", p=P)
    out_v = out_d[:, :].rearrange("(p r) c -> p r c", p=P)

    with TileContext(nc) as tc:
        with (
            tc.tile_pool(name="io", bufs=2) as iop,
            tc.tile_pool(name="km", bufs=2) as kmp,
            tc.tile_pool(name="zz", bufs=2) as zzp,
            tc.tile_pool(name="scr", bufs=2) as scp,
            tc.tile_pool(name="outk", bufs=2) as okp,
            tc.tile_pool(name="outf", bufs=4) as ofp,
        ):
            r0 = 0
            for R in tile_rs:
                assert R % 2 == 0
                xr_s = iop.tile([P, R * D], F32, tag="xr")
                xi_s = iop.tile([P, R * D], F32, tag="xi")
                nc.sync.dma_start(
                    out=xr_s[:].rearrange("p (r d) -> p r d", d=D),
                    in_=xr_v[:, r0 : r0 + R, :],
                )
                nc.sync.dma_start(
                    out=xi_s[:].rearrange("p (r d) -> p r d", d=D),
                    in_=xi_v[:, r0 : r0 + R, :],
                )
                xr_k = kmp.tile([P, D2 * R], F16, tag="xrk")
                xi_k = kmp.tile([P, D2 * R], F16, tag="xik")
                nc.scalar.copy(
                    out=xr_k[:].rearrange("p (d r) -> p d r", d=D2)[:, :D, :],
                    in_=xr_s[:].rearrange("p (r d) -> p r d", d=D).transpose([0, 2, 1]),
                )
                nc.scalar.copy(
                    out=xi_k[:].rearrange("p (d r) -> p d r", d=D2)[:, :D, :],
                    in_=xi_s[:].rearrange("p (r d) -> p r d", d=D).transpose([0, 2, 1]),
                )
                xr3 = xr_k[:].rearrange("p (d r) -> p d r", d=D2)   # [128,6,R]
                xi3 = xi_k[:].rearrange("p (d r) -> p d r", d=D2)

                ar_t = zzp.tile([P, KP * R], F16, tag="ar")
                ai_t = zzp.tile([P, KP * R], F16, tag="ai")
                ar3 = ar_t[:].rearrange("p (k r) -> p k r", k=KP)
                ai3 = ai_t[:].rearrange("p (k r) -> p k r", k=KP)

                # ---- step 1 (pairs (0,1),(2,3) then single a=4) ----
                for a in (0, 2):
                    L = D - a
                    b_r = xr3[:, a : a + 2, :].unsqueeze(2).broadcast_to([P, 2, L, R])
                    b_i = xi3[:, a : a + 2, :].unsqueeze(2).broadcast_to([P, 2, L, R])
                    w_r = _ap4(xr_k[:], a, 1, L, R)
                    w_i = _ap4(xi_k[:], a, 1, L, R)
                    s1 = scp.tile([P, 2 * D * R], F16, tag="s1")
                    s2 = scp.tile([P, 2 * D * R], F16, tag="s2")
                    s3 = scp.tile([P, 2 * D * R], F16, tag="s3")
                    s4 = scp.tile([P, 2 * D * R], F16, tag="s4")
                    s1v = s1[:, : 2 * L * R].rearrange("p (j k r) -> p j k r", j=2, k=L)
                    s2v = s2[:, : 2 * L * R].rearrange("p (j k r) -> p j k r", j=2, k=L)
                    s3v = s3[:, : 2 * L * R].rearrange("p (j k r) -> p j k r", j=2, k=L)
                    s4v = s4[:, : 2 * L * R].rearrange("p (j k r) -> p j k r", j=2, k=L)
                    nc.vector.tensor_mul(out=s1v, in0=b_r, in1=w_r)
                    nc.vector.tensor_mul(out=s2v, in0=b_i, in1=w_i)
                    nc.vector.tensor_sub(out=_ap4(ar_t[:], O1[a], L, L, R), in0=s1v, in1=s2v)
                    nc.vector.tensor_mul(out=s3v, in0=b_r, in1=w_i)
                    nc.vector.tensor_mul(out=s4v, in0=b_i, in1=w_r)
                    nc.vector.tensor_add(out=_ap4(ai_t[:], O1[a], L, L, R), in0=s3v, in1=s4v)
                # single a=4 (L=1)
                a = 4
                bra = xr3[:, a : a + 1, :]
                bia = xi3[:, a : a + 1, :]
                s1 = scp.tile([P, 2 * D * R], F16, tag="s1")
                s2 = scp.tile([P, 2 * D * R], F16, tag="s2")
                s1v = s1[:, :R].unsqueeze(1)
                s2v = s2[:, :R].unsqueeze(1)
                nc.vector.tensor_mul(out=s1v, in0=bra, in1=xr3[:, a : a + 1, :])
                nc.vector.tensor_mul(out=s2v, in0=bia, in1=xi3[:, a : a + 1, :])
                nc.vector.tensor_sub(out=ar3[:, 14:15, :], in0=s1v, in1=s2v)
                s3 = scp.tile([P, 2 * D * R], F16, tag="s3")
                s4 = scp.tile([P, 2 * D * R], F16, tag="s4")
                s3v = s3[:, :R].unsqueeze(1)
                s4v = s4[:, :R].unsqueeze(1)
                nc.vector.tensor_mul(out=s3v, in0=bra, in1=xi3[:, a : a + 1, :])
                nc.vector.tensor_mul(out=s4v, in0=bia, in1=xr3[:, a : a + 1, :])
                nc.vector.tensor_add(out=ai3[:, 14:15, :], in0=s3v, in1=s4v)

                out_k = okp.tile([P, NP * R], F16, tag="outk")
                ok3 = out_k[:].rearrange("p (c r) -> p c r", c=NP)

                # Gauss 3-mult: with v = ar+ai, u = ar-ai:
                #   k1 = v_a * ar_b;  k3 = ai_a * u_b;  k2 = ar_a * v_b
                #   re(a,b) = k1 - k3;  im(a,b) = k1 - k2   (b >= a+1 for im)
                v_t = zzp.tile([P, KP * R], F16, tag="vt")
                u_t = zzp.tile([P, KP * R], F16, tag="ut")
                nc.vector.tensor_add(out=v_t[:], in0=ar_t[:], in1=ai_t[:])
                nc.vector.tensor_sub(out=u_t[:], in0=ar_t[:], in1=ai_t[:])
                v3 = v_t[:].rearrange("p (k r) -> p k r", k=KP)

                # ---- step 2: re pairs a=0,2,..,12 + single a=14 ----
                im_eng = nc.gpsimd if GPSIMD_IM else nc.vector
                for a in range(0, K - 1, 2):
                    L = K - a
                    L2 = L - 1
                    b_v = v3[:, a : a + 2, :].unsqueeze(2).broadcast_to([P, 2, L, R])
                    b_ai = ai3[:, a : a + 2, :].unsqueeze(2).broadcast_to([P, 2, L, R])
                    b_ar = ar3[:, a : a + 2, :].unsqueeze(2).broadcast_to([P, 2, L2, R])
                    w_ar = _ap4(ar_t[:], a, 1, L, R)
                    w_u = _ap4(u_t[:], a, 1, L, R)
                    w_v = _ap4(v_t[:], a + 1, 1, L2, R)
                    k1 = scp.tile([P, 2 * K * R], F16, tag="m1")
                    k2 = scp.tile([P, 2 * K * R], F16, tag="m2")
                    k3 = scp.tile([P, 2 * K * R], F16, tag="m3")
                    k1v = k1[:, : 2 * L * R].rearrange("p (j k r) -> p j k r", j=2, k=L)
                    k3v = k3[:, : 2 * L * R].rearrange("p (j k r) -> p j k r", j=2, k=L)
                    k2v = k2[:, : 2 * L2 * R].rearrange("p (j k r) -> p j k r", j=2, k=L2)
                    nc.vector.tensor_mul(out=k1v, in0=b_v, in1=w_ar)
                    nc.vector.tensor_mul(out=k3v, in0=b_ai, in1=w_u)
                    nc.vector.tensor_mul(out=k2v, in0=b_ar, in1=w_v)
                    re_eng = nc.gpsimd if a in GP_RE_PAIRS else nc.vector
                    re_eng.tensor_sub(
                        out=_ap4(out_k[:], int(RO[a]), L, L, R), in0=k1v, in1=k3v
                    )
                    im_eng.tensor_sub(
                        out=_ap4(out_k[:], int(IO[a]), L2, L2, R),
                        in0=k1v[:, :, 1:, :],
                        in1=k2v,
                    )
                # single a=14 re (L=1): pr = ar^2 + ai^2
                m1 = scp.tile([P, 2 * K * R], F16, tag="m1")
                m2 = scp.tile([P, 2 * K * R], F16, tag="m2")
                m1v = m1[:, :R].unsqueeze(1)
                m2v = m2[:, :R].unsqueeze(1)
                nc.vector.tensor_mul(out=m1v, in0=ar3[:, 14:15, :], in1=ar3[:, 14:15, :])
                nc.vector.tensor_mul(out=m2v, in0=ai3[:, 14:15, :], in1=ai3[:, 14:15, :])
                nc.vector.tensor_add(out=ok3[:, 119:120, :], in0=m1v, in1=m2v)

                # ---- ScalarE: fp16 k-major -> fp32 row-major, ~12-row chunks ----
                RC = 8
                h0 = 0
                while h0 < R:
                    rc = min(RC, R - h0)
                    of = ofp.tile([P, RC * NOUT], F32, tag="outf")
                    of3 = of[:, : rc * NOUT].rearrange("p (r c) -> p r c", c=NOUT)
                    src = ok3[:, :NOUT, h0 : h0 + rc].transpose([0, 2, 1])
                    nc.scalar.copy(out=of3, in_=src)
                    nc.sync.dma_start(
                        out=out_v[:, r0 + h0 : r0 + h0 + rc, :], in_=of3
                    )
                    h0 += rc
                r0 += R

    _split_excess_waits(nc)
    return nc


def _build_fp16_kmout(
    n_c,
    rt,
    tile_rs,
    gp_k2=None,
    gp_im=None,
    gp_re=None,
    k1_bufs=4,
    k2_bufs=4,
    k2_first=True,
):
    """fp16 k-major compute (paired 4-dim AP groups) with the output DMA'd
    straight from the k-major fp16 tile: no on-chip transpose-out and no
    fp32 upconvert.  The DRAM output is a per-partition slab of
    rt*NOUT fp16 values laid out [tile][col][row-in-tile]; the host
    de-interleaves and upcasts during unshard.  Mult ops that run on
    GPSIMD go through scalar_tensor_tensor (out=(in0*1)op in1).
    k1 scratch gets its own deeper ring so DVE isn't back-pressured by the
    GPSIMD im consumer."""
    if gp_k2 is None:
        gp_k2 = GP_K2_PAIRS
    if gp_im is None:
        gp_im = GP_IM_PAIRS
    if gp_re is None:
        gp_re = GP_RE_PAIRS
    KP = 16       # zz padded to 16 cols (col 15 = junk read by pair overshoot)
    D2 = 6        # input padded to 6 cols (col 5 = junk)
    NP = NOUT + 1 # out_k padded by 1 col for im-pair overshoot
    nc = bass.Bass()
    xr_d = nc.dram_tensor("x_re", [n_c, D], F32, kind="ExternalInput")
    xi_d = nc.dram_tensor("x_im", [n_c, D], F32, kind="ExternalInput")
    out_d = nc.dram_tensor("out", [P, rt * NOUT], F16, kind="ExternalOutput")

    xr_v = xr_d[:, :].rearrange("(p r) d -> p r d", p=P)
    xi_v = xi_d[:, :].rearrange("(p r) d -> p r d", p=P)

    mul_op = mybir.AluOpType.mult
    sub_op = mybir.AluOpType.subtract

    def gp_mul(out, in0, in1):
        nc.gpsimd.scalar_tensor_tensor(
            out=out, in0=in0, scalar=1.0, in1=in1, op0=mul_op, op1=mul_op
        )

    def gp_sub(out, in0, in1):
        nc.gpsimd.scalar_tensor_tensor(
            out=out, in0=in0, scalar=1.0, in1=in1, op0=mul_op, op1=sub_op
        )

    with TileContext(nc) as tc:
        with (
            tc.tile_pool(name="io", bufs=2) as iop,
            tc.tile_pool(name="km", bufs=2) as kmp,
            tc.tile_pool(name="zz", bufs=2) as zzp,
            tc.tile_pool(name="scr", bufs=2) as scp,
            tc.tile_pool(name="k1p", bufs=k1_bufs) as k1p,
            tc.tile_pool(name="k2p", bufs=k2_bufs) as k2p,
            tc.tile_pool(name="outk", bufs=2) as okp,
        ):
            r0 = 0
            for R in tile_rs:
                assert R % 2 == 0
                xr_s = iop.tile([P, R * D], F32, tag="xr")
                xi_s = iop.tile([P, R * D], F32, tag="xi")
                nc.sync.dma_start(
                    out=xr_s[:].rearrange("p (r d) -> p r d", d=D),
                    in_=xr_v[:, r0 : r0 + R, :],
                )
                nc.sync.dma_start(
                    out=xi_s[:].rearrange("p (r d) -> p r d", d=D),
                    in_=xi_v[:, r0 : r0 + R, :],
                )
                xr_k = kmp.tile([P, D2 * R], F16, tag="xrk")
                xi_k = kmp.tile([P, D2 * R], F16, tag="xik")
                nc.scalar.copy(
                    out=xr_k[:].rearrange("p (d r) -> p d r", d=D2)[:, :D, :],
                    in_=xr_s[:].rearrange("p (r d) -> p r d", d=D).transpose([0, 2, 1]),
                )
                nc.scalar.copy(
                    out=xi_k[:].rearrange("p (d r) -> p d r", d=D2)[:, :D, :],
                    in_=xi_s[:].rearrange("p (r d) -> p r d", d=D).transpose([0, 2, 1]),
                )
                xr3 = xr_k[:].rearrange("p (d r) -> p d r", d=D2)   # [128,6,R]
                xi3 = xi_k[:].rearrange("p (d r) -> p d r", d=D2)

                ar_t = zzp.tile([P, KP * R], F16, tag="ar")
                ai_t = zzp.tile([P, KP * R], F16, tag="ai")
                ar3 = ar_t[:].rearrange("p (k r) -> p k r", k=KP)
                ai3 = ai_t[:].rearrange("p (k r) -> p k r", k=KP)

                # ---- step 1 (pairs (0,1),(2,3) then single a=4) ----
                for a in (0, 2):
                    L = D - a
                    b_r = xr3[:, a : a + 2, :].unsqueeze(2).broadcast_to([P, 2, L, R])
                    b_i = xi3[:, a : a + 2, :].unsqueeze(2).broadcast_to([P, 2, L, R])
                    w_r = _ap4(xr_k[:], a, 1, L, R)
                    w_i = _ap4(xi_k[:], a, 1, L, R)
                    s1 = scp.tile([P, 2 * D * R], F16, tag="s1")
                    s2 = scp.tile([P, 2 * D * R], F16, tag="s2")
                    s3 = scp.tile([P, 2 * D * R], F16, tag="s3")
                    s4 = scp.tile([P, 2 * D * R], F16, tag="s4")
                    s1v = s1[:, : 2 * L * R].rearrange("p (j k r) -> p j k r", j=2, k=L)
                    s2v = s2[:, : 2 * L * R].rearrange("p (j k r) -> p j k r", j=2, k=L)
                    s3v = s3[:, : 2 * L * R].rearrange("p (j k r) -> p j k r", j=2, k=L)
                    s4v = s4[:, : 2 * L * R].rearrange("p (j k r) -> p j k r", j=2, k=L)
                    nc.vector.tensor_mul(out=s1v, in0=b_r, in1=w_r)
                    nc.vector.tensor_mul(out=s2v, in0=b_i, in1=w_i)
                    nc.vector.tensor_sub(out=_ap4(ar_t[:], O1[a], L, L, R), in0=s1v, in1=s2v)
                    nc.vector.tensor_mul(out=s3v, in0=b_r, in1=w_i)
                    nc.vector.tensor_mul(out=s4v, in0=b_i, in1=w_r)
                    nc.vector.tensor_add(out=_ap4(ai_t[:], O1[a], L, L, R), in0=s3v, in1=s4v)
                # single a=4 (L=1)
                a = 4
                bra = xr3[:, a : a + 1, :]
                bia = xi3[:, a : a + 1, :]
                s1 = scp.tile([P, 2 * D * R], F16, tag="s1")
                s2 = scp.tile([P, 2 * D * R], F16, tag="s2")
                s1v = s1[:, :R].unsqueeze(1)
                s2v = s2[:, :R].unsqueeze(1)
                nc.vector.tensor_mul(out=s1v, in0=bra, in1=xr3[:, a : a + 1, :])
                nc.vector.tensor_mul(out=s2v, in0=bia, in1=xi3[:, a : a + 1, :])
                nc.vector.tensor_sub(out=ar3[:, 14:15, :], in0=s1v, in1=s2v)
                s3 = scp.tile([P, 2 * D * R], F16, tag="s3")
                s4 = scp.tile([P, 2 * D * R], F16, tag="s4")
                s3v = s3[:, :R].unsqueeze(1)
                s4v = s4[:, :R].unsqueeze(1)
                nc.vector.tensor_mul(out=s3v, in0=bra, in1=xi3[:, a : a + 1, :])
                nc.vector.tensor_mul(out=s4v, in0=bia, in1=xr3[:, a : a + 1, :])
                nc.vector.tensor_add(out=ai3[:, 14:15, :], in0=s3v, in1=s4v)

                out_k = okp.tile([P, NP * R], F16, tag="outk")
                ok3 = out_k[:].rearrange("p (c r) -> p c r", c=NP)

                # Gauss 3-mult: with v = ar+ai, u = ar-ai:
                #   k1 = v_a * ar_b;  k3 = ai_a * u_b;  k2 = ar_a * v_b
                #   re(a,b) = k1 - k3;  im(a,b) = k1 - k2   (b >= a+1 for im)
                v_t = zzp.tile([P, KP * R], F16, tag="vt")
                u_t = zzp.tile([P, KP * R], F16, tag="ut")
                nc.vector.tensor_add(out=v_t[:], in0=ar_t[:], in1=ai_t[:])
                nc.vector.tensor_sub(out=u_t[:], in0=ar_t[:], in1=ai_t[:])
                v3 = v_t[:].rearrange("p (k r) -> p k r", k=KP)

                # ---- step 2: re pairs a=0,2,..,12 + single a=14 ----
                for a in range(0, K - 1, 2):
                    L = K - a
                    L2 = L - 1
                    b_v = v3[:, a : a + 2, :].unsqueeze(2).broadcast_to([P, 2, L, R])
                    b_ai = ai3[:, a : a + 2, :].unsqueeze(2).broadcast_to([P, 2, L, R])
                    b_ar = ar3[:, a : a + 2, :].unsqueeze(2).broadcast_to([P, 2, L2, R])
                    w_ar = _ap4(ar_t[:], a, 1, L, R)
                    w_u = _ap4(u_t[:], a, 1, L, R)
                    w_v = _ap4(v_t[:], a + 1, 1, L2, R)
                    k1 = k1p.tile([P, 2 * K * R], F16, tag="m1")
                    k2 = k2p.tile([P, 2 * K * R], F16, tag="m2")
                    k3 = scp.tile([P, 2 * K * R], F16, tag="m3")
                    k1v = k1[:, : 2 * L * R].rearrange("p (j k r) -> p j k r", j=2, k=L)
                    k3v = k3[:, : 2 * L * R].rearrange("p (j k r) -> p j k r", j=2, k=L)
                    k2v = k2[:, : 2 * L2 * R].rearrange("p (j k r) -> p j k r", j=2, k=L2)

                    def emit_k2():
                        if a in gp_k2:
                            gp_mul(k2v, b_ar, w_v)
                        else:
                            nc.vector.tensor_mul(out=k2v, in0=b_ar, in1=w_v)

                    if k2_first:
                        emit_k2()
                    nc.vector.tensor_mul(out=k1v, in0=b_v, in1=w_ar)
                    nc.vector.tensor_mul(out=k3v, in0=b_ai, in1=w_u)
                    if not k2_first:
                        emit_k2()
                    if a in gp_re:
                        gp_sub(_ap4(out_k[:], int(RO[a]), L, L, R), k1v, k3v)
                    else:
                        nc.vector.tensor_sub(
                            out=_ap4(out_k[:], int(RO[a]), L, L, R), in0=k1v, in1=k3v
                        )
                    if a in gp_im:
                        gp_sub(_ap4(out_k[:], int(IO[a]), L2, L2, R), k1v[:, :, 1:, :], k2v)
                    else:
                        nc.vector.tensor_sub(
                            out=_ap4(out_k[:], int(IO[a]), L2, L2, R),
                            in0=k1v[:, :, 1:, :],
                            in1=k2v,
                        )
                # single a=14 re (L=1): pr = ar^2 + ai^2
                m1 = scp.tile([P, 2 * D * R], F16, tag="s1")
                m2 = scp.tile([P, 2 * D * R], F16, tag="s2")
                m1v = m1[:, :R].unsqueeze(1)
                m2v = m2[:, :R].unsqueeze(1)
                nc.vector.tensor_mul(out=m1v, in0=ar3[:, 14:15, :], in1=ar3[:, 14:15, :])
                nc.vector.tensor_mul(out=m2v, in0=ai3[:, 14:15, :], in1=ai3[:, 14:15, :])
                nc.vector.tensor_add(out=ok3[:, 119:120, :], in0=m1v, in1=m2v)

                # ---- direct k-major fp16 DMA out (contiguous slab) ----
                nc.sync.dma_start(
                    out=out_d[:, r0 * NOUT : (r0 + R) * NOUT],
                    in_=out_k[:, : NOUT * R],
                )
                r0 += R

    _split_excess_waits(nc)
    return nc


def _build_fp16_pe(
    n_c,
    rt,
    tile_rs,
    im_asn=None,     # pair-start a -> 'pe' | 'dve' | 'pool'
    ring_bufs=6,
    psum_bufs=4,
    chunk=512,
    psb=1024,        # PSUM super-chunk (elements; 2 banks) per Act copy
):
    """fp16 k-major compute with the PE (tensor) engine doing the re (and
    selected im) combines as identity-weight matmul accumulations in PSUM,
    Act converting PSUM fp32 -> fp16 out_k, and the output DMA'd k-major.

    Per pair-group (a, a+1), both runs length L = 15-a:
      k1 = v_a * ar[b]      (DVE)   v = ar+ai
      k3'= ai_a * u2[b]     (DVE)   u2 = ai-ar   (negated Gauss operand)
      k2'= -(ar_a * v[b+1]) (Pool STT, scalar=-1)
      re(a,b)  = k1 + k3'   (PE accumulate, Act copy)
      im(a,b)  = k1[1:] + k2'  (PE / DVE / Pool per im_asn)
    The identity stationary matrix ships as an extra ExternalInput."""
    if im_asn is None:
        im_asn = {a: ("dve" if a >= 10 else "pe") for a in range(0, K - 1, 2)}
    KP = 16
    D2 = 6
    NP = NOUT + 1
    nc = bass.Bass()
    xr_d = nc.dram_tensor("x_re", [n_c, D], F32, kind="ExternalInput")
    xi_d = nc.dram_tensor("x_im", [n_c, D], F32, kind="ExternalInput")
    id_d = nc.dram_tensor("ident", [P, P], F16, kind="ExternalInput")
    out_d = nc.dram_tensor("out", [P, rt * NOUT], F16, kind="ExternalOutput")

    xr_v = xr_d[:, :].rearrange("(p r) d -> p r d", p=P)
    xi_v = xi_d[:, :].rearrange("(p r) d -> p r d", p=P)

    mul_op = mybir.AluOpType.mult
    add_op = mybir.AluOpType.add
    IOr = IO - 120  # im col offsets within the im triangle

    with TileContext(nc) as tc:
        with (
            tc.tile_pool(name="io", bufs=2) as iop,
            tc.tile_pool(name="km", bufs=2) as kmp,
            tc.tile_pool(name="ident", bufs=1) as idp,
            tc.tile_pool(name="zz", bufs=2) as zzp,
            tc.tile_pool(name="scr", bufs=2) as scp,
            tc.tile_pool(name="k1p", bufs=ring_bufs) as k1p,
            tc.tile_pool(name="k2p", bufs=ring_bufs) as k2p,
            tc.tile_pool(name="k3p", bufs=ring_bufs) as k3p,
            tc.psum_pool(name="ps", bufs=psum_bufs) as psp,
            tc.tile_pool(name="outk", bufs=2) as okp,
        ):
            ident = idp.tile([P, P], F16, tag="I")
            nc.sync.dma_start(out=ident[:], in_=id_d[:, :])

            def load_tile(R, r0):
                """Input DMA + Act transpose-cast for one tile; emitted one
                tile ahead so Act's in-order queue never parks the next
                tile's transposes behind this tile's PSUM copies."""
                xr_s = iop.tile([P, R * D], F32, tag="xr")
                xi_s = iop.tile([P, R * D], F32, tag="xi")
                nc.sync.dma_start(
                    out=xr_s[:].rearrange("p (r d) -> p r d", d=D),
                    in_=xr_v[:, r0 : r0 + R, :],
                )
                nc.sync.dma_start(
                    out=xi_s[:].rearrange("p (r d) -> p r d", d=D),
                    in_=xi_v[:, r0 : r0 + R, :],
                )
                xr_k = kmp.tile([P, D2 * R], F16, tag="xrk")
                xi_k = kmp.tile([P, D2 * R], F16, tag="xik")
                nc.scalar.copy(
                    out=xr_k[:].rearrange("p (d r) -> p d r", d=D2)[:, :D, :],
                    in_=xr_s[:].rearrange("p (r d) -> p r d", d=D).transpose([0, 2, 1]),
                )
                nc.scalar.copy(
                    out=xi_k[:].rearrange("p (d r) -> p d r", d=D2)[:, :D, :],
                    in_=xi_s[:].rearrange("p (r d) -> p r d", d=D).transpose([0, 2, 1]),
                )
                return xr_k, xi_k

            offs = []
            _o = 0
            for _R in tile_rs:
                offs.append(_o)
                _o += _R
            loaded = load_tile(tile_rs[0], offs[0])
            for ti, R in enumerate(tile_rs):
                assert R % 2 == 0
                r0 = offs[ti]
                xr_k, xi_k = loaded
                if ti + 1 < len(tile_rs):
                    loaded = load_tile(tile_rs[ti + 1], offs[ti + 1])
                xr3 = xr_k[:].rearrange("p (d r) -> p d r", d=D2)
                xi3 = xi_k[:].rearrange("p (d r) -> p d r", d=D2)

                ar_t = zzp.tile([P, KP * R], F16, tag="ar")
                ai_t = zzp.tile([P, KP * R], F16, tag="ai")
                ar3 = ar_t[:].rearrange("p (k r) -> p k r", k=KP)
                ai3 = ai_t[:].rearrange("p (k r) -> p k r", k=KP)

                out_k = okp.tile([P, NP * R], F16, tag="outk")
                ok3 = out_k[:].rearrange("p (c r) -> p c r", c=NP)

                v_t = zzp.tile([P, KP * R], F16, tag="vt")
                u2_t = zzp.tile([P, KP * R], F16, tag="ut")
                an_t = zzp.tile([P, KP * R], F16, tag="an")
                v3 = v_t[:].rearrange("p (k r) -> p k r", k=KP)
                an3 = an_t[:].rearrange("p (k r) -> p k r", k=KP)

                def vua(c0, c1):
                    """v = ar+ai, u2 = ai-ar, an = -ar for zz cols [c0, c1) —
                    emitted right after the step-1 block producing them so
                    step-2 work on high columns can start early."""
                    sl = slice(c0 * R, c1 * R)
                    nc.vector.tensor_add(out=v_t[:, sl], in0=ar_t[:, sl], in1=ai_t[:, sl])
                    nc.vector.tensor_sub(out=u2_t[:, sl], in0=ai_t[:, sl], in1=ar_t[:, sl])
                    nc.vector.tensor_scalar_mul(an_t[:, sl], ar_t[:, sl], -1.0)

                # ---- step 1, high zz columns first (single a=4, then pair
                # (2,3), then pair (0,1)), v/u/an produced incrementally ----
                a = 4
                bra = xr3[:, a : a + 1, :]
                bia = xi3[:, a : a + 1, :]
                s1 = scp.tile([P, 2 * D * R], F16, tag="s1")
                s2 = scp.tile([P, 2 * D * R], F16, tag="s2")
                s1v = s1[:, :R].unsqueeze(1)
                s2v = s2[:, :R].unsqueeze(1)
                nc.vector.tensor_mul(out=s1v, in0=bra, in1=xr3[:, a : a + 1, :])
                nc.vector.tensor_mul(out=s2v, in0=bia, in1=xi3[:, a : a + 1, :])
                nc.vector.tensor_sub(out=ar3[:, 14:15, :], in0=s1v, in1=s2v)
                s3 = scp.tile([P, 2 * D * R], F16, tag="s3")
                s4 = scp.tile([P, 2 * D * R], F16, tag="s4")
                s3v = s3[:, :R].unsqueeze(1)
                s4v = s4[:, :R].unsqueeze(1)
                nc.vector.tensor_mul(out=s3v, in0=bra, in1=xi3[:, a : a + 1, :])
                nc.vector.tensor_mul(out=s4v, in0=bia, in1=xr3[:, a : a + 1, :])
                nc.vector.tensor_add(out=ai3[:, 14:15, :], in0=s3v, in1=s4v)
                vua(14, KP)  # col 15 pad reads junk ar/ai: never consumed

                # single a=14 mults right away (re col 119 inputs)
                k1s = k1p.tile([P, 2 * K * R], F16, tag="m1")
                k3s = k3p.tile([P, 2 * K * R], F16, tag="m3")
                nc.vector.tensor_mul(
                    out=k1s[:, :R].unsqueeze(1), in0=v3[:, 14:15, :], in1=ar3[:, 14:15, :]
                )
                nc.vector.tensor_mul(
                    out=k3s[:, :R].unsqueeze(1),
                    in0=ai3[:, 14:15, :],
                    in1=u2_t[:].rearrange("p (k r) -> p k r", k=KP)[:, 14:15, :],
                )

                for a in (2, 0):
                    L = D - a
                    b_r = xr3[:, a : a + 2, :].unsqueeze(2).broadcast_to([P, 2, L, R])
                    b_i = xi3[:, a : a + 2, :].unsqueeze(2).broadcast_to([P, 2, L, R])
                    w_r = _ap4(xr_k[:], a, 1, L, R)
                    w_i = _ap4(xi_k[:], a, 1, L, R)
                    s1 = scp.tile([P, 2 * D * R], F16, tag="s1")
                    s2 = scp.tile([P, 2 * D * R], F16, tag="s2")
                    s3 = scp.tile([P, 2 * D * R], F16, tag="s3")
                    s4 = scp.tile([P, 2 * D * R], F16, tag="s4")
                    s1v = s1[:, : 2 * L * R].rearrange("p (j k r) -> p j k r", j=2, k=L)
                    s2v = s2[:, : 2 * L * R].rearrange("p (j k r) -> p j k r", j=2, k=L)
                    s3v = s3[:, : 2 * L * R].rearrange("p (j k r) -> p j k r", j=2, k=L)
                    s4v = s4[:, : 2 * L * R].rearrange("p (j k r) -> p j k r", j=2, k=L)
                    nc.vector.tensor_mul(out=s1v, in0=b_r, in1=w_r)
                    nc.vector.tensor_mul(out=s2v, in0=b_i, in1=w_i)
                    nc.vector.tensor_mul(out=s3v, in0=b_r, in1=w_i)
                    nc.vector.tensor_mul(out=s4v, in0=b_i, in1=w_r)
                    # exact-length combines per group: the paired scratch's
                    # j=1 run has a junk tail column that must not reach zz
                    # (later groups are already written in descending order)
                    def _j0(t):
                        return t[:, : L * R].rearrange("p (k r) -> p k r", k=L)

                    def _j1(t):
                        return t[:, L * R : (2 * L - 1) * R].rearrange(
                            "p (k r) -> p k r", k=L - 1
                        )

                    nc.vector.tensor_sub(
                        out=ar3[:, O1[a] : O1[a] + L, :], in0=_j0(s1), in1=_j0(s2)
                    )
                    nc.vector.tensor_sub(
                        out=ar3[:, O1[a + 1] : O1[a + 1] + L - 1, :],
                        in0=_j1(s1), in1=_j1(s2),
                    )
                    nc.vector.tensor_add(
                        out=ai3[:, O1[a] : O1[a] + L, :], in0=_j0(s3), in1=_j0(s4)
                    )
                    nc.vector.tensor_add(
                        out=ai3[:, O1[a + 1] : O1[a + 1] + L - 1, :],
                        in0=_j1(s3), in1=_j1(s4),
                    )
                    vua(O1[a], O1[a + 2] if a + 2 < D else 14)

                # ---- step 2 mults, pair-grouped, descending (high pairs
                # depend only on high zz cols -> unlock earliest) ----
                pair_tiles = {}
                for a in range(K - 3, -1, -2):
                    L = K - a
                    L2 = L - 1
                    b_v = v3[:, a : a + 2, :].unsqueeze(2).broadcast_to([P, 2, L, R])
                    b_ai = ai3[:, a : a + 2, :].unsqueeze(2).broadcast_to([P, 2, L, R])
                    w_ar = _ap4(ar_t[:], a, 1, L, R)
                    w_u = _ap4(u2_t[:], a, 1, L, R)
                    w_v = _ap4(v_t[:], a + 1, 1, L2, R)
                    k1 = k1p.tile([P, 2 * K * R], F16, tag="m1")
                    k2 = k2p.tile([P, 2 * K * R], F16, tag="m2")
                    k3 = k3p.tile([P, 2 * K * R], F16, tag="m3")
                    k1v = k1[:, : 2 * L * R].rearrange("p (j k r) -> p j k r", j=2, k=L)
                    k3v = k3[:, : 2 * L * R].rearrange("p (j k r) -> p j k r", j=2, k=L)
                    # k2' = (-ar)_a * v[b] on Pool (plain TT mult; the only
                    # tensor op the Pool engine supports on hw)
                    b_an = an3[:, a : a + 2, :].unsqueeze(2).broadcast_to([P, 2, L2, R])
                    k2v = k2[:, : 2 * L2 * R].rearrange("p (j k r) -> p j k r", j=2, k=L2)
                    nc.gpsimd.tensor_mul(out=k2v, in0=b_an, in1=w_v)
                    nc.vector.tensor_mul(out=k1v, in0=b_v, in1=w_ar)
                    nc.vector.tensor_mul(out=k3v, in0=b_ai, in1=w_u)
                    pair_tiles[a] = (k1, k2, k3, L, L2)

                # ---- combines ----
                def pe_accum(dst_off, n_el, rhs1, rhs2):
                    """PSUM-accumulate rhs1+rhs2 (each [P, n_el] fp16 slices)
                    into out_k[:, dst_off : dst_off + n_el], in psb-sized
                    super-chunks each finished by one Act convert-copy."""
                    s0 = 0
                    while s0 < n_el:
                        se = min(s0 + psb, n_el)
                        ps = psp.tile([P, psb], F32, tag="ps")
                        c0 = s0
                        while c0 < se:
                            ce = min(c0 + chunk, se)
                            nc.tensor.matmul(
                                out=ps[:, c0 - s0 : ce - s0], lhsT=ident[:],
                                rhs=rhs1[:, c0:ce], start=True, stop=False,
                            )
                            nc.tensor.matmul(
                                out=ps[:, c0 - s0 : ce - s0], lhsT=ident[:],
                                rhs=rhs2[:, c0:ce], start=False, stop=True,
                            )
                            c0 = ce
                        nc.scalar.copy(
                            out=out_k[:, dst_off + s0 : dst_off + se],
                            in_=ps[:, : se - s0],
                        )
                        s0 = se

                # combines, descending, re/im interleaved per pair: aligned
                # with the descending mult production so every consumer's
                # inputs are the producers' most recent outputs
                pe_accum(119 * R, R, k1s[:, :R], k3s[:, :R])
                for a in range(K - 3, -1, -2):
                    k1, k2, k3, L, L2 = pair_tiles[a]
                    # re pair: k1 + k3' over the contiguous (2L-1) col region
                    # (excludes the pair tile's final junk column)
                    n_re = (2 * L - 1) * R
                    pe_accum(RO[a] * R, n_re, k1[:, :n_re], k3[:, :n_re])
                    # im pair: k1[1:] + k2' ; groups a (L2 cols) and a+1 (L2-1)
                    asn = im_asn.get(a, "dve")
                    if asn == "pe":
                        pe_accum(IO[a] * R, L2 * R, k1[:, R : L * R], k2[:, : L2 * R])
                        if L2 - 1 > 0:
                            pe_accum(
                                IO[a + 1] * R, (L2 - 1) * R,
                                k1[:, L * R + R : L * R + (L - 1) * R],
                                k2[:, L2 * R : L2 * R + (L2 - 1) * R],
                            )
                    elif asn == "pool":
                        nc.gpsimd.tensor_add(
                            out=ok3[:, IO[a] : IO[a] + L2, :],
                            in0=k1[:, R : L * R].rearrange("p (k r) -> p k r", k=L2),
                            in1=k2[:, : L2 * R].rearrange("p (k r) -> p k r", k=L2),
                        )
                        if L2 - 1 > 0:
                            nc.gpsimd.tensor_add(
                                out=ok3[:, IO[a + 1] : IO[a + 1] + L2 - 1, :],
                                in0=k1[:, L * R + R : L * R + (L - 1) * R].rearrange(
                                    "p (k r) -> p k r", k=L2 - 1
                                ),
                                in1=k2[:, L2 * R : L2 * R + (L2 - 1) * R].rearrange(
                                    "p (k r) -> p k r", k=L2 - 1
                                ),
                            )
                    else:
                        nc.vector.tensor_add(
                            out=ok3[:, IO[a] : IO[a] + L2, :],
                            in0=k1[:, R : L * R].rearrange("p (k r) -> p k r", k=L2),
                            in1=k2[:, : L2 * R].rearrange("p (k r) -> p k r", k=L2),
                        )
                        if L2 - 1 > 0:
                            nc.vector.tensor_add(
                                out=ok3[:, IO[a + 1] : IO[a + 1] + L2 - 1, :],
                                in0=k1[:, L * R + R : L * R + (L - 1) * R].rearrange(
                                    "p (k r) -> p k r", k=L2 - 1
                                ),
                                in1=k2[:, L2 * R : L2 * R + (L2 - 1) * R].rearrange(
                                    "p (k r) -> p k r", k=L2 - 1
                                ),
                            )

                # ---- direct k-major fp16 DMA out ----
                nc.sync.dma_start(
                    out=out_d[:, r0 * NOUT : (r0 + R) * NOUT],
                    in_=out_k[:, : NOUT * R],
                )

    _split_excess_waits(nc)
    return nc


_CACHE = {}


def _make_tiles(rt, r_max, first=0, last=0):
    """Split rt rows into tiles of r_max with optional small first/last
    tiles (fast pipeline fill/drain)."""
    rem = rt - first - last
    tiles = ([first] if first else []) + [r_max] * (rem // r_max)
    r = rem % r_max
    if r:
        tiles.append(r)
    if last:
        tiles.append(last)
    assert sum(tiles) == rt and all(t % 2 == 0 for t in tiles)
    return tiles


def _get_program(n):
    """Geometry + compiled program for total row count n.
    Returns (nc, n_c, rt, tile_rs)."""
    key = (n, PRECISION)
    if key in _CACHE:
        return _CACHE[key]
    per_core = -(-n // N_CORES)              # ceil
    rt = -(-per_core // P)                   # rows per partition
    if PRECISION == "fp16_pe":
        rt += rt % 2
        n_c = P * rt
        tile_rs = _make_tiles(rt, 72, first=24, last=32)
        nc = _build_fp16_pe(n_c, rt, tile_rs)
    elif PRECISION == "fp16":
        rt += rt % 2                         # even rt (fp16 4B alignment needs even R only)
        n_c = P * rt
        r_max = 100                          # divisible by 4; best per cost-model sweep
        tile_rs = [r_max] * (rt // r_max)
        if rt % r_max:
            tile_rs.append(rt % r_max)
        nc = _build_fp16_kmout(n_c, rt, tile_rs)
    else:
        n_c = P * rt
        r_max = 64
        tile_rs = [r_max] * (rt // r_max)
        if rt % r_max:
            tile_rs.append(rt % r_max)
        nc = _build(n_c, rt, tile_rs)
    _CACHE[key] = (nc, n_c, rt, tile_rs)
    return _CACHE[key]


def kernel(x_re, x_im, _trace=False):
    x_re = np.ascontiguousarray(np.asarray(x_re), dtype=np.float32)
    x_im = np.ascontiguousarray(np.asarray(x_im), dtype=np.float32)
    n = x_re.shape[0]
    nc, n_c, rt, tile_rs = _get_program(n)
    n_pad = n_c * N_CORES
    if n_pad != n:
        pad = np.zeros((n_pad - n, D), dtype=np.float32)
        xr = np.concatenate([x_re, pad], axis=0)
        xi = np.concatenate([x_im, pad], axis=0)
    else:
        xr, xi = x_re, x_im
    xr_sh = xr.reshape(N_CORES, n_c, D)
    xi_sh = xi.reshape(N_CORES, n_c, D)
    in_maps = [
        {"x_re": np.ascontiguousarray(xr_sh[i]), "x_im": np.ascontiguousarray(xi_sh[i])}
        for i in range(N_CORES)
    ]
    if PRECISION == "fp16_pe":
        ident = np.eye(P, dtype=np.float16)
        for m in in_maps:
            m["ident"] = ident
    res = bass_utils.run_bass_kernel_spmd(
        nc, in_maps, core_ids=list(range(N_CORES)), trace=_trace
    )
    if PRECISION in ("fp16", "fp16_pe"):
        # device output is [P, rt*NOUT] fp16, per tile [col][row-in-tile]
        # (k-major); de-interleave to [n_c, NOUT] and upcast per core.
        out = np.empty((n_pad, NOUT), dtype=np.float32)
        for i, r in enumerate(res.results):
            raw = r["out"]  # [P, rt*NOUT] fp16
            core_out = out[i * n_c : (i + 1) * n_c].reshape(P, rt, NOUT)
            r0 = 0
            for R in tile_rs:
                blk = raw[:, r0 * NOUT : (r0 + R) * NOUT].reshape(P, NOUT, R)
                core_out[:, r0 : r0 + R, :] = blk.transpose(0, 2, 1)
                r0 += R
        out = out[:n]
    else:
        out = np.concatenate([r["out"] for r in res.results], axis=0)[:n]
    if _trace:
        return out, res
    return out



# revision 44
# speedup vs baseline: 1.4761x; 1.0198x over previous
"""Trainium2 Bass kernel for nn_Biholomorphic_k2.

Per row (N=1e6 rows, D=5):
  z = x_re + i*x_im                                  [5] complex
  zz = z[i5] * z[j5]          (triu pairs of 5)      [15] complex
  prod = zz[i15] * conj(zz[j15])  (triu pairs of 15) [120] complex
  out = [Re(prod) (120 cols), Im(prod offdiag) (105 cols)]  [225] f32

Sharding: pure data parallel over 8 cores; rows on SBUF partitions in
contiguous blocks (partition p of core c owns rows [c*NC + p*RT, ...)),
features along the free dim so every DMA is fully contiguous.

All compute is VectorE tensor_tensor ops with stride-0 (broadcast) access
patterns on the "a"-side operand; results are written directly into the
packed [128, R*225] output tile, so there is no separate gather step.
"""

import sys

import numpy as np

try:
    import concourse.bass as bass
except ImportError:
    for _p in ("/opt/trn_rl_repo", "/root/.axon_site/_ro/trn_rl_repo"):
        if _p not in sys.path:
            sys.path.insert(0, _p)
    import concourse.bass as bass
import concourse.mybir as mybir
from concourse.tile import TileContext
from concourse import bass_utils

P = 128          # SBUF partitions
D = 5
K = 15           # triu pairs of 5
NOUT = 225       # 120 re + 105 im
N_CORES = 8

# --- static index tables (row-major triu, matching np.triu_indices) ---
# step-1: for a in 0..4 produce zz[o1[a] : o1[a]+5-a] = z[a] * z[a:5]
O1 = [0, 5, 9, 12, 14]
# step-2 re: for a in 0..14, out[ro[a] : ro[a]+15-a] = Re(zz[a] * conj(zz[a:15]))
RO = np.concatenate([[0], np.cumsum([15 - a for a in range(15)])]).astype(int)
# step-2 im: for a in 0..13, out[120+io[a] : ...+14-a] = Im(zz[a] * conj(zz[a+1:15]))
IO = 120 + np.concatenate([[0], np.cumsum([14 - a for a in range(14)])]).astype(int)

F32 = mybir.dt.float32
F16 = mybir.dt.float16

# "fp16": k-major fp16 compute (2x DVE mode), ~7e-4 rel error.
# "fp32": row-major fp32 compute, ~6e-8 rel error, ~2x slower.
PRECISION = "fp16_pe"
# route the step-2 im combines to GPSIMD (overlaps with DVE; shares one of
# DVE's two SBUF ports -- cost model says net win)
GPSIMD_IM = True
# additionally route re combines for these pair-start values to GPSIMD
GP_RE_PAIRS = ()
# pair-start values whose k2 mult runs on GPSIMD (fp16-kmout build)
GP_K2_PAIRS = (0, 2, 4, 6)
# pair-start values whose im combine runs on GPSIMD (fp16-kmout build)
GP_IM_PAIRS = (0, 2, 4, 6, 8, 10, 12)

_MAX_CTRL_WAITS = 1


def _split_excess_waits(nc):
    """Workaround: this walrus build rejects Drain instructions carrying
    more than one sync wait ("Too many sync wait commands").  Move excess
    waits onto NOPs inserted immediately before, on the same engine."""
    engmap = {
        mybir.EngineType.SP: nc.sync,
        mybir.EngineType.DVE: nc.vector,
        mybir.EngineType.Activation: nc.scalar,
        mybir.EngineType.PE: nc.tensor,
        mybir.EngineType.Pool: nc.gpsimd,
    }
    for f in nc.m.functions:
        for blk in f.blocks:
            newlist = []
            for inst in blk.instructions:
                si = inst.sync_info
                if (
                    si is not None
                    and si.on_wait is not None
                    and len(si.on_wait) > _MAX_CTRL_WAITS
                ):
                    waits = list(si.on_wait)
                    head = waits[:-_MAX_CTRL_WAITS]
                    tail = waits[-_MAX_CTRL_WAITS:]
                    for s in range(0, len(head), _MAX_CTRL_WAITS):
                        chunk = head[s : s + _MAX_CTRL_WAITS]
                        bi = engmap[inst.engine].nop()
                        nop_inst = bi.ins if hasattr(bi, "ins") else bi
                        for b2 in f.blocks:
                            if nop_inst in b2.instructions:
                                b2.instructions.remove(nop_inst)
                        nop_inst.sync_info = mybir.SyncInfo(on_wait=chunk, on_update=[])
                        nop_inst.engine = inst.engine
                        newlist.append(nop_inst)
                    inst.sync_info = mybir.SyncInfo(
                        on_wait=tail, on_update=list(si.on_update or [])
                    )
                newlist.append(inst)
            blk.instructions[:] = newlist


def _build(n_c, rt, tile_rs):
    """Build the Bass program for one core's shard: [n_c, 5] x2 -> [n_c, 225].
    n_c = P * rt rows; processed in free-dim chunks of R rows/partition."""
    nc = bass.Bass()
    xr_d = nc.dram_tensor("x_re", [n_c, D], F32, kind="ExternalInput")
    xi_d = nc.dram_tensor("x_im", [n_c, D], F32, kind="ExternalInput")
    out_d = nc.dram_tensor("out", [n_c, NOUT], F32, kind="ExternalOutput")

    xr_v = xr_d[:, :].rearrange("(p r) d -> p r d", p=P)     # [128, rt, 5]
    xi_v = xi_d[:, :].rearrange("(p r) d -> p r d", p=P)
    out_v = out_d[:, :].rearrange("(p r) c -> p r c", p=P)   # [128, rt, 225]

    with TileContext(nc) as tc:
        with (
            tc.tile_pool(name="io", bufs=3) as iop,
            tc.tile_pool(name="zz", bufs=2) as zzp,
            tc.tile_pool(name="scr", bufs=2) as scp,
            tc.tile_pool(name="outp", bufs=2) as outp,
        ):
            r0 = 0
            for R in tile_rs:
                xr_t = iop.tile([P, R * D], F32, tag="xr")
                xi_t = iop.tile([P, R * D], F32, tag="xi")
                nc.sync.dma_start(
                    out=xr_t[:].rearrange("p (r d) -> p r d", d=D),
                    in_=xr_v[:, r0 : r0 + R, :],
                )
                nc.sync.dma_start(
                    out=xi_t[:].rearrange("p (r d) -> p r d", d=D),
                    in_=xi_v[:, r0 : r0 + R, :],
                )
                xr3 = xr_t[:].rearrange("p (r d) -> p r d", d=D)   # [128,R,5]
                xi3 = xi_t[:].rearrange("p (r d) -> p r d", d=D)

                ar_t = zzp.tile([P, R * K], F32, tag="ar")
                ai_t = zzp.tile([P, R * K], F32, tag="ai")
                ar3 = ar_t[:].rearrange("p (r k) -> p r k", k=K)   # [128,R,15]
                ai3 = ai_t[:].rearrange("p (r k) -> p r k", k=K)

                # ---- step 1: zz = z[a] * z[a:5] for a in 0..4 ----
                for a in range(D):
                    L = D - a
                    o = O1[a]
                    s1 = scp.tile([P, R * D], F32, tag="s1")
                    s2 = scp.tile([P, R * D], F32, tag="s2")
                    s1v = s1[:].rearrange("p (r d) -> p r d", d=D)[:, :, :L]
                    s2v = s2[:].rearrange("p (r d) -> p r d", d=D)[:, :, :L]
                    bra = xr3[:, :, a : a + 1].broadcast_to([P, R, L])
                    bia = xi3[:, :, a : a + 1].broadcast_to([P, R, L])
                    # re: xr_a*xr_b - xi_a*xi_b
                    nc.vector.tensor_mul(out=s1v, in0=bra, in1=xr3[:, :, a:D])
                    nc.vector.tensor_mul(out=s2v, in0=bia, in1=xi3[:, :, a:D])
                    nc.vector.tensor_sub(
                        out=ar3[:, :, o : o + L], in0=s1v, in1=s2v
                    )
                    # im: xr_a*xi_b + xi_a*xr_b
                    s3 = scp.tile([P, R * D], F32, tag="s3")
                    s4 = scp.tile([P, R * D], F32, tag="s4")
                    s3v = s3[:].rearrange("p (r d) -> p r d", d=D)[:, :, :L]
                    s4v = s4[:].rearrange("p (r d) -> p r d", d=D)[:, :, :L]
                    nc.vector.tensor_mul(out=s3v, in0=bra, in1=xi3[:, :, a:D])
                    nc.vector.tensor_mul(out=s4v, in0=bia, in1=xr3[:, :, a:D])
                    nc.vector.tensor_add(
                        out=ai3[:, :, o : o + L], in0=s3v, in1=s4v
                    )

                out_t = outp.tile([P, R * NOUT], F32, tag="out")
                out3 = out_t[:].rearrange("p (r c) -> p r c", c=NOUT)

                # ---- step 2: prod = zz[a] * conj(zz[b]), b >= a ----
                for a in range(K):
                    L = K - a
                    bar = ar3[:, :, a : a + 1].broadcast_to([P, R, L])
                    bai = ai3[:, :, a : a + 1].broadcast_to([P, R, L])
                    m1 = scp.tile([P, R * K], F32, tag="m1")
                    m2 = scp.tile([P, R * K], F32, tag="m2")
                    m1v = m1[:].rearrange("p (r k) -> p r k", k=K)[:, :, :L]
                    m2v = m2[:].rearrange("p (r k) -> p r k", k=K)[:, :, :L]
                    # re: ar_a*ar_b + ai_a*ai_b
                    nc.vector.tensor_mul(out=m1v, in0=bar, in1=ar3[:, :, a:K])
                    nc.vector.tensor_mul(out=m2v, in0=bai, in1=ai3[:, :, a:K])
                    ro = int(RO[a])
                    nc.vector.tensor_add(
                        out=out3[:, :, ro : ro + L], in0=m1v, in1=m2v
                    )
                    # im (offdiag only): ai_a*ar_b - ar_a*ai_b
                    if a < K - 1:
                        L2 = L - 1
                        m3 = scp.tile([P, R * K], F32, tag="m3")
                        m4 = scp.tile([P, R * K], F32, tag="m4")
                        m3v = m3[:].rearrange("p (r k) -> p r k", k=K)[:, :, :L2]
                        m4v = m4[:].rearrange("p (r k) -> p r k", k=K)[:, :, :L2]
                        bar2 = ar3[:, :, a : a + 1].broadcast_to([P, R, L2])
                        bai2 = ai3[:, :, a : a + 1].broadcast_to([P, R, L2])
                        nc.vector.tensor_mul(
                            out=m3v, in0=bai2, in1=ar3[:, :, a + 1 : K]
                        )
                        nc.vector.tensor_mul(
                            out=m4v, in0=bar2, in1=ai3[:, :, a + 1 : K]
                        )
                        io = int(IO[a])
                        nc.vector.tensor_sub(
                            out=out3[:, :, io : io + L2], in0=m3v, in1=m4v
                        )

                nc.sync.dma_start(out=out_v[:, r0 : r0 + R, :], in_=out3)
                r0 += R

    _split_excess_waits(nc)
    return nc


def _build_fp16(n_c, rt, tile_rs):
    """k-major fp16 build: within each partition, every tensor is stored
    feature-major ([k, r] with r innermost, step 1) so all DVE tensor_tensor
    operands have a 2-byte dtype, innermost step 1, and 4B-aligned run starts
    (R even) -> 2x_1p DVE mode throughout.  ScalarE does the fp32->fp16
    transpose-in and the fp16->fp32 transpose-out (its own SBUF ports, 1x).
    """
    KP = 16  # zz column padding (alignment headroom)
    nc = bass.Bass()
    xr_d = nc.dram_tensor("x_re", [n_c, D], F32, kind="ExternalInput")
    xi_d = nc.dram_tensor("x_im", [n_c, D], F32, kind="ExternalInput")
    out_d = nc.dram_tensor("out", [n_c, NOUT], F32, kind="ExternalOutput")

    xr_v = xr_d[:, :].rearrange("(p r) d -> p r d", p=P)     # [128, rt, 5]
    xi_v = xi_d[:, :].rearrange("(p r) d -> p r d", p=P)
    out_v = out_d[:, :].rearrange("(p r) c -> p r c", p=P)   # [128, rt, 225]

    with TileContext(nc) as tc:
        with (
            tc.tile_pool(name="io", bufs=3) as iop,
            tc.tile_pool(name="km", bufs=2) as kmp,
            tc.tile_pool(name="zz", bufs=2) as zzp,
            tc.tile_pool(name="scr", bufs=2) as scp,
            tc.tile_pool(name="outk", bufs=2) as okp,
            tc.tile_pool(name="outf", bufs=2) as ofp,
        ):
            r0 = 0
            for R in tile_rs:
                assert R % 2 == 0
                R2 = R // 2
                xr_s = iop.tile([P, R * D], F32, tag="xr")
                xi_s = iop.tile([P, R * D], F32, tag="xi")
                nc.sync.dma_start(
                    out=xr_s[:].rearrange("p (r d) -> p r d", d=D),
                    in_=xr_v[:, r0 : r0 + R, :],
                )
                nc.sync.dma_start(
                    out=xi_s[:].rearrange("p (r d) -> p r d", d=D),
                    in_=xi_v[:, r0 : r0 + R, :],
                )
                # ScalarE: cast fp32->fp16 + transpose row-major -> k-major
                xr_k = kmp.tile([P, D * R], F16, tag="xrk")
                xi_k = kmp.tile([P, D * R], F16, tag="xik")
                # src [r, d] -> view [d, r]
                nc.scalar.copy(
                    out=xr_k[:].rearrange("p (d r) -> p d r", d=D),
                    in_=xr_s[:].rearrange("p (r d) -> p r d", d=D).transpose([0, 2, 1]),
                )
                nc.scalar.copy(
                    out=xi_k[:].rearrange("p (d r) -> p d r", d=D),
                    in_=xi_s[:].rearrange("p (r d) -> p r d", d=D).transpose([0, 2, 1]),
                )
                xr3 = xr_k[:].rearrange("p (d r) -> p d r", d=D)   # [128,5,R]
                xi3 = xi_k[:].rearrange("p (d r) -> p d r", d=D)

                ar_t = zzp.tile([P, KP * R], F16, tag="ar")
                ai_t = zzp.tile([P, KP * R], F16, tag="ai")
                ar3 = ar_t[:].rearrange("p (k r) -> p k r", k=KP)  # [128,16,R]
                ai3 = ai_t[:].rearrange("p (k r) -> p k r", k=KP)

                # ---- step 1: zz[o1[a]:o1[a]+L] = z[a] * z[a:5] ----
                for a in range(D):
                    L = D - a
                    o = O1[a]
                    bra = xr3[:, a : a + 1, :].broadcast_to([P, L, R])
                    bia = xi3[:, a : a + 1, :].broadcast_to([P, L, R])
                    s1 = scp.tile([P, D * R], F16, tag="s1")
                    s2 = scp.tile([P, D * R], F16, tag="s2")
                    s1v = s1[:].rearrange("p (k r) -> p k r", k=D)[:, :L, :]
                    s2v = s2[:].rearrange("p (k r) -> p k r", k=D)[:, :L, :]
                    nc.vector.tensor_mul(out=s1v, in0=bra, in1=xr3[:, a:D, :])
                    nc.vector.tensor_mul(out=s2v, in0=bia, in1=xi3[:, a:D, :])
                    nc.vector.tensor_sub(out=ar3[:, o : o + L, :], in0=s1v, in1=s2v)
                    s3 = scp.tile([P, D * R], F16, tag="s3")
                    s4 = scp.tile([P, D * R], F16, tag="s4")
                    s3v = s3[:].rearrange("p (k r) -> p k r", k=D)[:, :L, :]
                    s4v = s4[:].rearrange("p (k r) -> p k r", k=D)[:, :L, :]
                    nc.vector.tensor_mul(out=s3v, in0=bra, in1=xi3[:, a:D, :])
                    nc.vector.tensor_mul(out=s4v, in0=bia, in1=xr3[:, a:D, :])
                    nc.vector.tensor_add(out=ai3[:, o : o + L, :], in0=s3v, in1=s4v)

                out_k = okp.tile([P, NOUT * R], F16, tag="outk")
                ok3 = out_k[:].rearrange("p (c r) -> p c r", c=NOUT)  # [128,225,R]

                # ---- step 2 ----
                for a in range(K):
                    L = K - a
                    bar = ar3[:, a : a + 1, :].broadcast_to([P, L, R])
                    bai = ai3[:, a : a + 1, :].broadcast_to([P, L, R])
                    m1 = scp.tile([P, KP * R], F16, tag="m1")
                    m2 = scp.tile([P, KP * R], F16, tag="m2")
                    m1v = m1[:].rearrange("p (k r) -> p k r", k=KP)[:, :L, :]
                    m2v = m2[:].rearrange("p (k r) -> p k r", k=KP)[:, :L, :]
                    nc.vector.tensor_mul(out=m1v, in0=bar, in1=ar3[:, a:K, :])
                    nc.vector.tensor_mul(out=m2v, in0=bai, in1=ai3[:, a:K, :])
                    ro = int(RO[a])
                    nc.vector.tensor_add(out=ok3[:, ro : ro + L, :], in0=m1v, in1=m2v)
                    if a < K - 1:
                        L2 = L - 1
                        bar2 = ar3[:, a : a + 1, :].broadcast_to([P, L2, R])
                        bai2 = ai3[:, a : a + 1, :].broadcast_to([P, L2, R])
                        m3 = scp.tile([P, KP * R], F16, tag="m3")
                        m4 = scp.tile([P, KP * R], F16, tag="m4")
                        m3v = m3[:].rearrange("p (k r) -> p k r", k=KP)[:, :L2, :]
                        m4v = m4[:].rearrange("p (k r) -> p k r", k=KP)[:, :L2, :]
                        nc.vector.tensor_mul(out=m3v, in0=bai2, in1=ar3[:, a + 1 : K, :])
                        nc.vector.tensor_mul(out=m4v, in0=bar2, in1=ai3[:, a + 1 : K, :])
                        io = int(IO[a])
                        nc.vector.tensor_sub(
                            out=ok3[:, io : io + L2, :], in0=m3v, in1=m4v
                        )

                # ---- ScalarE: fp16 k-major -> fp32 row-major, in row-chunks ----
                n_chunks = 4 if R % 4 == 0 else 2
                Rc = R // n_chunks
                for h in range(n_chunks):
                    of = ofp.tile([P, Rc * NOUT], F32, tag="outf")
                    of3 = of[:].rearrange("p (r c) -> p r c", c=NOUT)  # [128,Rc,225]
                    src = ok3[:, :, h * Rc : (h + 1) * Rc].transpose([0, 2, 1])
                    nc.scalar.copy(out=of3, in_=src)
                    nc.sync.dma_start(
                        out=out_v[:, r0 + h * Rc : r0 + (h + 1) * Rc, :], in_=of3
                    )
                r0 += R

    _split_excess_waits(nc)
    return nc


def _ap4(t2d, col_off, jstep_cols, L, R):
    """4-dim AP over a k-major [cols x R] SBUF tile view: two runs (j=0,1)
    of L columns x R rows, run j starting at column col_off + j*jstep_cols."""
    pdim = list(t2d.ap[0])
    return bass.AP(
        t2d.tensor,
        t2d.offset + col_off * R,
        [pdim, [jstep_cols * R, 2], [R, L], [1, R]],
    )


def _build_fp16_paired(n_c, rt, tile_rs):
    """Like _build_fp16 but batches consecutive-a groups in pairs via 4-dim
    APs, halving DVE instruction count.  The second run of each pair reads one
    padded junk column and writes one column past its end; emission order
    guarantees a later group rewrites the overshoot column with real data."""
    KP = 16       # zz padded to 16 cols (col 15 = junk read by pair overshoot)
    D2 = 6        # input padded to 6 cols (col 5 = junk)
    NP = NOUT + 1 # out_k padded by 1 col for im-pair overshoot
    nc = bass.Bass()
    xr_d = nc.dram_tensor("x_re", [n_c, D], F32, kind="ExternalInput")
    xi_d = nc.dram_tensor("x_im", [n_c, D], F32, kind="ExternalInput")
    out_d = nc.dram_tensor("out", [n_c, NOUT], F32, kind="ExternalOutput")

    xr_v = xr_d[:, :].rearrange("(p r) d -> p r d", p=P)
    xi_v = xi_d[:, :].rearrange("(p r) d -> p r d", p=P)
    out_v = out_d[:, :].rearrange("(p r) c -> p r c", p=P)

    with TileContext(nc) as tc:
        with (
            tc.tile_pool(name="io", bufs=2) as iop,
            tc.tile_pool(name="km", bufs=2) as kmp,
            tc.tile_pool(name="zz", bufs=2) as zzp,
            tc.tile_pool(name="scr", bufs=2) as scp,
            tc.tile_pool(name="outk", bufs=2) as okp,
            tc.tile_pool(name="outf", bufs=4) as ofp,
        ):
            r0 = 0
            for R in tile_rs:
                assert R % 2 == 0
                xr_s = iop.tile([P, R * D], F32, tag="xr")
                xi_s = iop.tile([P, R * D], F32, tag="xi")
                nc.sync.dma_start(
                    out=xr_s[:].rearrange("p (r d) -> p r d", d=D),
                    in_=xr_v[:, r0 : r0 + R, :],
                )
                nc.sync.dma_start(
                    out=xi_s[:].rearrange("p (r d) -> p r d", d=D),
                    in_=xi_v[:, r0 : r0 + R, :],
                )
                xr_k = kmp.tile([P, D2 * R], F16, tag="xrk")
                xi_k = kmp.tile([P, D2 * R], F16, tag="xik")
                nc.scalar.copy(
                    out=xr_k[:].rearrange("p (d r) -> p d r", d=D2)[:, :D, :],
                    in_=xr_s[:].rearrange("p (r d) -> p r d", d=D).transpose([0, 2, 1]),
                )
                nc.scalar.copy(
                    out=xi_k[:].rearrange("p (d r) -> p d r", d=D2)[:, :D, :],
                    in_=xi_s[:].rearrange("p (r d) -> p r d", d=D).transpose([0, 2, 1]),
                )
                xr3 = xr_k[:].rearrange("p (d r) -> p d r", d=D2)   # [128,6,R]
                xi3 = xi_k[:].rearrange("p (d r) -> p d r", d=D2)

                ar_t = zzp.tile([P, KP * R], F16, tag="ar")
                ai_t = zzp.tile([P, KP * R], F16, tag="ai")
                ar3 = ar_t[:].rearrange("p (k r) -> p k r", k=KP)
                ai3 = ai_t[:].rearrange("p (k r) -> p k r", k=KP)

                # ---- step 1 (pairs (0,1),(2,3) then single a=4) ----
                for a in (0, 2):
                    L = D - a
                    b_r = xr3[:, a : a + 2, :].unsqueeze(2).broadcast_to([P, 2, L, R])
                    b_i = xi3[:, a : a + 2, :].unsqueeze(2).broadcast_to([P, 2, L, R])
                    w_r = _ap4(xr_k[:], a, 1, L, R)
                    w_i = _ap4(xi_k[:], a, 1, L, R)
                    s1 = scp.tile([P, 2 * D * R], F16, tag="s1")
                    s2 = scp.tile([P, 2 * D * R], F16, tag="s2")
                    s3 = scp.tile([P, 2 * D * R], F16, tag="s3")
                    s4 = scp.tile([P, 2 * D * R], F16, tag="s4")
                    s1v = s1[:, : 2 * L * R].rearrange("p (j k r) -> p j k r", j=2, k=L)
                    s2v = s2[:, : 2 * L * R].rearrange("p (j k r) -> p j k r", j=2, k=L)
                    s3v = s3[:, : 2 * L * R].rearrange("p (j k r) -> p j k r", j=2, k=L)
                    s4v = s4[:, : 2 * L * R].rearrange("p (j k r) -> p j k r", j=2, k=L)
                    nc.vector.tensor_mul(out=s1v, in0=b_r, in1=w_r)
                    nc.vector.tensor_mul(out=s2v, in0=b_i, in1=w_i)
                    nc.vector.tensor_sub(out=_ap4(ar_t[:], O1[a], L, L, R), in0=s1v, in1=s2v)
                    nc.vector.tensor_mul(out=s3v, in0=b_r, in1=w_i)
                    nc.vector.tensor_mul(out=s4v, in0=b_i, in1=w_r)
                    nc.vector.tensor_add(out=_ap4(ai_t[:], O1[a], L, L, R), in0=s3v, in1=s4v)
                # single a=4 (L=1)
                a = 4
                bra = xr3[:, a : a + 1, :]
                bia = xi3[:, a : a + 1, :]
                s1 = scp.tile([P, 2 * D * R], F16, tag="s1")
                s2 = scp.tile([P, 2 * D * R], F16, tag="s2")
                s1v = s1[:, :R].unsqueeze(1)
                s2v = s2[:, :R].unsqueeze(1)
                nc.vector.tensor_mul(out=s1v, in0=bra, in1=xr3[:, a : a + 1, :])
                nc.vector.tensor_mul(out=s2v, in0=bia, in1=xi3[:, a : a + 1, :])
                nc.vector.tensor_sub(out=ar3[:, 14:15, :], in0=s1v, in1=s2v)
                s3 = scp.tile([P, 2 * D * R], F16, tag="s3")
                s4 = scp.tile([P, 2 * D * R], F16, tag="s4")
                s3v = s3[:, :R].unsqueeze(1)
                s4v = s4[:, :R].unsqueeze(1)
                nc.vector.tensor_mul(out=s3v, in0=bra, in1=xi3[:, a : a + 1, :])
                nc.vector.tensor_mul(out=s4v, in0=bia, in1=xr3[:, a : a + 1, :])
                nc.vector.tensor_add(out=ai3[:, 14:15, :], in0=s3v, in1=s4v)

                out_k = okp.tile([P, NP * R], F16, tag="outk")
                ok3 = out_k[:].rearrange("p (c r) -> p c r", c=NP)

                # Gauss 3-mult: with v = ar+ai, u = ar-ai:
                #   k1 = v_a * ar_b;  k3 = ai_a * u_b;  k2 = ar_a * v_b
                #   re(a,b) = k1 - k3;  im(a,b) = k1 - k2   (b >= a+1 for im)
                v_t = zzp.tile([P, KP * R], F16, tag="vt")
                u_t = zzp.tile([P, KP * R], F16, tag="ut")
                nc.vector.tensor_add(out=v_t[:], in0=ar_t[:], in1=ai_t[:])
                nc.vector.tensor_sub(out=u_t[:], in0=ar_t[:], in1=ai_t[:])
                v3 = v_t[:].rearrange("p (k r) -> p k r", k=KP)

                # ---- step 2: re pairs a=0,2,..,12 + single a=14 ----
                im_eng = nc.gpsimd if GPSIMD_IM else nc.vector
                for a in range(0, K - 1, 2):
                    L = K - a
                    L2 = L - 1
                    b_v = v3[:, a : a + 2, :].unsqueeze(2).broadcast_to([P, 2, L, R])
                    b_ai = ai3[:, a : a + 2, :].unsqueeze(2).broadcast_to([P, 2, L, R])
                    b_ar = ar3[:, a : a + 2, :].unsqueeze(2).broadcast_to([P, 2, L2, R])
                    w_ar = _ap4(ar_t[:], a, 1, L, R)
                    w_u = _ap4(u_t[:], a, 1, L, R)
                    w_v = _ap4(v_t[:], a + 1, 1, L2, R)
                    k1 = scp.tile([P, 2 * K * R], F16, tag="m1")
                    k2 = scp.tile([P, 2 * K * R], F16, tag="m2")
                    k3 = scp.tile([P, 2 * K * R], F16, tag="m3")
                    k1v = k1[:, : 2 * L * R].rearrange("p (j k r) -> p j k r", j=2, k=L)
                    k3v = k3[:, : 2 * L * R].rearrange("p (j k r) -> p j k r", j=2, k=L)
                    k2v = k2[:, : 2 * L2 * R].rearrange("p (j k r) -> p j k r", j=2, k=L2)
                    nc.vector.tensor_mul(out=k1v, in0=b_v, in1=w_ar)
                    nc.vector.tensor_mul(out=k3v, in0=b_ai, in1=w_u)
                    nc.vector.tensor_mul(out=k2v, in0=b_ar, in1=w_v)
                    re_eng = nc.gpsimd if a in GP_RE_PAIRS else nc.vector
                    re_eng.tensor_sub(
                        out=_ap4(out_k[:], int(RO[a]), L, L, R), in0=k1v, in1=k3v
                    )
                    im_eng.tensor_sub(
                        out=_ap4(out_k[:], int(IO[a]), L2, L2, R),
                        in0=k1v[:, :, 1:, :],
                        in1=k2v,
                    )
                # single a=14 re (L=1): pr = ar^2 + ai^2
                m1 = scp.tile([P, 2 * K * R], F16, tag="m1")
                m2 = scp.tile([P, 2 * K * R], F16, tag="m2")
                m1v = m1[:, :R].unsqueeze(1)
                m2v = m2[:, :R].unsqueeze(1)
                nc.vector.tensor_mul(out=m1v, in0=ar3[:, 14:15, :], in1=ar3[:, 14:15, :])
                nc.vector.tensor_mul(out=m2v, in0=ai3[:, 14:15, :], in1=ai3[:, 14:15, :])
                nc.vector.tensor_add(out=ok3[:, 119:120, :], in0=m1v, in1=m2v)

                # ---- ScalarE: fp16 k-major -> fp32 row-major, ~12-row chunks ----
                RC = 8
                h0 = 0
                while h0 < R:
                    rc = min(RC, R - h0)
                    of = ofp.tile([P, RC * NOUT], F32, tag="outf")
                    of3 = of[:, : rc * NOUT].rearrange("p (r c) -> p r c", c=NOUT)
                    src = ok3[:, :NOUT, h0 : h0 + rc].transpose([0, 2, 1])
                    nc.scalar.copy(out=of3, in_=src)
                    nc.sync.dma_start(
                        out=out_v[:, r0 + h0 : r0 + h0 + rc, :], in_=of3
                    )
                    h0 += rc
                r0 += R

    _split_excess_waits(nc)
    return nc


def _build_fp16_kmout(
    n_c,
    rt,
    tile_rs,
    gp_k2=None,
    gp_im=None,
    gp_re=None,
    k1_bufs=4,
    k2_bufs=4,
    k2_first=True,
):
    """fp16 k-major compute (paired 4-dim AP groups) with the output DMA'd
    straight from the k-major fp16 tile: no on-chip transpose-out and no
    fp32 upconvert.  The DRAM output is a per-partition slab of
    rt*NOUT fp16 values laid out [tile][col][row-in-tile]; the host
    de-interleaves and upcasts during unshard.  Mult ops that run on
    GPSIMD go through scalar_tensor_tensor (out=(in0*1)op in1).
    k1 scratch gets its own deeper ring so DVE isn't back-pressured by the
    GPSIMD im consumer."""
    if gp_k2 is None:
        gp_k2 = GP_K2_PAIRS
    if gp_im is None:
        gp_im = GP_IM_PAIRS
    if gp_re is None:
        gp_re = GP_RE_PAIRS
    KP = 16       # zz padded to 16 cols (col 15 = junk read by pair overshoot)
    D2 = 6        # input padded to 6 cols (col 5 = junk)
    NP = NOUT + 1 # out_k padded by 1 col for im-pair overshoot
    nc = bass.Bass()
    xr_d = nc.dram_tensor("x_re", [n_c, D], F32, kind="ExternalInput")
    xi_d = nc.dram_tensor("x_im", [n_c, D], F32, kind="ExternalInput")
    out_d = nc.dram_tensor("out", [P, rt * NOUT], F16, kind="ExternalOutput")

    xr_v = xr_d[:, :].rearrange("(p r) d -> p r d", p=P)
    xi_v = xi_d[:, :].rearrange("(p r) d -> p r d", p=P)

    mul_op = mybir.AluOpType.mult
    sub_op = mybir.AluOpType.subtract

    def gp_mul(out, in0, in1):
        nc.gpsimd.scalar_tensor_tensor(
            out=out, in0=in0, scalar=1.0, in1=in1, op0=mul_op, op1=mul_op
        )

    def gp_sub(out, in0, in1):
        nc.gpsimd.scalar_tensor_tensor(
            out=out, in0=in0, scalar=1.0, in1=in1, op0=mul_op, op1=sub_op
        )

    with TileContext(nc) as tc:
        with (
            tc.tile_pool(name="io", bufs=2) as iop,
            tc.tile_pool(name="km", bufs=2) as kmp,
            tc.tile_pool(name="zz", bufs=2) as zzp,
            tc.tile_pool(name="scr", bufs=2) as scp,
            tc.tile_pool(name="k1p", bufs=k1_bufs) as k1p,
            tc.tile_pool(name="k2p", bufs=k2_bufs) as k2p,
            tc.tile_pool(name="outk", bufs=2) as okp,
        ):
            r0 = 0
            for R in tile_rs:
                assert R % 2 == 0
                xr_s = iop.tile([P, R * D], F32, tag="xr")
                xi_s = iop.tile([P, R * D], F32, tag="xi")
                nc.sync.dma_start(
                    out=xr_s[:].rearrange("p (r d) -> p r d", d=D),
                    in_=xr_v[:, r0 : r0 + R, :],
                )
                nc.sync.dma_start(
                    out=xi_s[:].rearrange("p (r d) -> p r d", d=D),
                    in_=xi_v[:, r0 : r0 + R, :],
                )
                xr_k = kmp.tile([P, D2 * R], F16, tag="xrk")
                xi_k = kmp.tile([P, D2 * R], F16, tag="xik")
                nc.scalar.copy(
                    out=xr_k[:].rearrange("p (d r) -> p d r", d=D2)[:, :D, :],
                    in_=xr_s[:].rearrange("p (r d) -> p r d", d=D).transpose([0, 2, 1]),
                )
                nc.scalar.copy(
                    out=xi_k[:].rearrange("p (d r) -> p d r", d=D2)[:, :D, :],
                    in_=xi_s[:].rearrange("p (r d) -> p r d", d=D).transpose([0, 2, 1]),
                )
                xr3 = xr_k[:].rearrange("p (d r) -> p d r", d=D2)   # [128,6,R]
                xi3 = xi_k[:].rearrange("p (d r) -> p d r", d=D2)

                ar_t = zzp.tile([P, KP * R], F16, tag="ar")
                ai_t = zzp.tile([P, KP * R], F16, tag="ai")
                ar3 = ar_t[:].rearrange("p (k r) -> p k r", k=KP)
                ai3 = ai_t[:].rearrange("p (k r) -> p k r", k=KP)

                # ---- step 1 (pairs (0,1),(2,3) then single a=4) ----
                for a in (0, 2):
                    L = D - a
                    b_r = xr3[:, a : a + 2, :].unsqueeze(2).broadcast_to([P, 2, L, R])
                    b_i = xi3[:, a : a + 2, :].unsqueeze(2).broadcast_to([P, 2, L, R])
                    w_r = _ap4(xr_k[:], a, 1, L, R)
                    w_i = _ap4(xi_k[:], a, 1, L, R)
                    s1 = scp.tile([P, 2 * D * R], F16, tag="s1")
                    s2 = scp.tile([P, 2 * D * R], F16, tag="s2")
                    s3 = scp.tile([P, 2 * D * R], F16, tag="s3")
                    s4 = scp.tile([P, 2 * D * R], F16, tag="s4")
                    s1v = s1[:, : 2 * L * R].rearrange("p (j k r) -> p j k r", j=2, k=L)
                    s2v = s2[:, : 2 * L * R].rearrange("p (j k r) -> p j k r", j=2, k=L)
                    s3v = s3[:, : 2 * L * R].rearrange("p (j k r) -> p j k r", j=2, k=L)
                    s4v = s4[:, : 2 * L * R].rearrange("p (j k r) -> p j k r", j=2, k=L)
                    nc.vector.tensor_mul(out=s1v, in0=b_r, in1=w_r)
                    nc.vector.tensor_mul(out=s2v, in0=b_i, in1=w_i)
                    nc.vector.tensor_sub(out=_ap4(ar_t[:], O1[a], L, L, R), in0=s1v, in1=s2v)
                    nc.vector.tensor_mul(out=s3v, in0=b_r, in1=w_i)
                    nc.vector.tensor_mul(out=s4v, in0=b_i, in1=w_r)
                    nc.vector.tensor_add(out=_ap4(ai_t[:], O1[a], L, L, R), in0=s3v, in1=s4v)
                # single a=4 (L=1)
                a = 4
                bra = xr3[:, a : a + 1, :]
                bia = xi3[:, a : a + 1, :]
                s1 = scp.tile([P, 2 * D * R], F16, tag="s1")
                s2 = scp.tile([P, 2 * D * R], F16, tag="s2")
                s1v = s1[:, :R].unsqueeze(1)
                s2v = s2[:, :R].unsqueeze(1)
                nc.vector.tensor_mul(out=s1v, in0=bra, in1=xr3[:, a : a + 1, :])
                nc.vector.tensor_mul(out=s2v, in0=bia, in1=xi3[:, a : a + 1, :])
                nc.vector.tensor_sub(out=ar3[:, 14:15, :], in0=s1v, in1=s2v)
                s3 = scp.tile([P, 2 * D * R], F16, tag="s3")
                s4 = scp.tile([P, 2 * D * R], F16, tag="s4")
                s3v = s3[:, :R].unsqueeze(1)
                s4v = s4[:, :R].unsqueeze(1)
                nc.vector.tensor_mul(out=s3v, in0=bra, in1=xi3[:, a : a + 1, :])
                nc.vector.tensor_mul(out=s4v, in0=bia, in1=xr3[:, a : a + 1, :])
                nc.vector.tensor_add(out=ai3[:, 14:15, :], in0=s3v, in1=s4v)

                out_k = okp.tile([P, NP * R], F16, tag="outk")
                ok3 = out_k[:].rearrange("p (c r) -> p c r", c=NP)

                # Gauss 3-mult: with v = ar+ai, u = ar-ai:
                #   k1 = v_a * ar_b;  k3 = ai_a * u_b;  k2 = ar_a * v_b
                #   re(a,b) = k1 - k3;  im(a,b) = k1 - k2   (b >= a+1 for im)
                v_t = zzp.tile([P, KP * R], F16, tag="vt")
                u_t = zzp.tile([P, KP * R], F16, tag="ut")
                nc.vector.tensor_add(out=v_t[:], in0=ar_t[:], in1=ai_t[:])
                nc.vector.tensor_sub(out=u_t[:], in0=ar_t[:], in1=ai_t[:])
                v3 = v_t[:].rearrange("p (k r) -> p k r", k=KP)

                # ---- step 2: re pairs a=0,2,..,12 + single a=14 ----
                for a in range(0, K - 1, 2):
                    L = K - a
                    L2 = L - 1
                    b_v = v3[:, a : a + 2, :].unsqueeze(2).broadcast_to([P, 2, L, R])
                    b_ai = ai3[:, a : a + 2, :].unsqueeze(2).broadcast_to([P, 2, L, R])
                    b_ar = ar3[:, a : a + 2, :].unsqueeze(2).broadcast_to([P, 2, L2, R])
                    w_ar = _ap4(ar_t[:], a, 1, L, R)
                    w_u = _ap4(u_t[:], a, 1, L, R)
                    w_v = _ap4(v_t[:], a + 1, 1, L2, R)
                    k1 = k1p.tile([P, 2 * K * R], F16, tag="m1")
                    k2 = k2p.tile([P, 2 * K * R], F16, tag="m2")
                    k3 = scp.tile([P, 2 * K * R], F16, tag="m3")
                    k1v = k1[:, : 2 * L * R].rearrange("p (j k r) -> p j k r", j=2, k=L)
                    k3v = k3[:, : 2 * L * R].rearrange("p (j k r) -> p j k r", j=2, k=L)
                    k2v = k2[:, : 2 * L2 * R].rearrange("p (j k r) -> p j k r", j=2, k=L2)

                    def emit_k2():
                        if a in gp_k2:
                            gp_mul(k2v, b_ar, w_v)
                        else:
                            nc.vector.tensor_mul(out=k2v, in0=b_ar, in1=w_v)

                    if k2_first:
                        emit_k2()
                    nc.vector.tensor_mul(out=k1v, in0=b_v, in1=w_ar)
                    nc.vector.tensor_mul(out=k3v, in0=b_ai, in1=w_u)
                    if not k2_first:
                        emit_k2()
                    if a in gp_re:
                        gp_sub(_ap4(out_k[:], int(RO[a]), L, L, R), k1v, k3v)
                    else:
                        nc.vector.tensor_sub(
                            out=_ap4(out_k[:], int(RO[a]), L, L, R), in0=k1v, in1=k3v
                        )
                    if a in gp_im:
                        gp_sub(_ap4(out_k[:], int(IO[a]), L2, L2, R), k1v[:, :, 1:, :], k2v)
                    else:
                        nc.vector.tensor_sub(
                            out=_ap4(out_k[:], int(IO[a]), L2, L2, R),
                            in0=k1v[:, :, 1:, :],
                            in1=k2v,
                        )
                # single a=14 re (L=1): pr = ar^2 + ai^2
                m1 = scp.tile([P, 2 * D * R], F16, tag="s1")
                m2 = scp.tile([P, 2 * D * R], F16, tag="s2")
                m1v = m1[:, :R].unsqueeze(1)
                m2v = m2[:, :R].unsqueeze(1)
                nc.vector.tensor_mul(out=m1v, in0=ar3[:, 14:15, :], in1=ar3[:, 14:15, :])
                nc.vector.tensor_mul(out=m2v, in0=ai3[:, 14:15, :], in1=ai3[:, 14:15, :])
                nc.vector.tensor_add(out=ok3[:, 119:120, :], in0=m1v, in1=m2v)

                # ---- direct k-major fp16 DMA out (contiguous slab) ----
                nc.sync.dma_start(
                    out=out_d[:, r0 * NOUT : (r0 + R) * NOUT],
                    in_=out_k[:, : NOUT * R],
                )
                r0 += R

    _split_excess_waits(nc)
    return nc


def _build_fp16_pe(
    n_c,
    rt,
    tile_rs,
    im_asn=None,     # pair-start a -> 'pe' | 'dve' | 'pool'
    ring_bufs=6,
    psum_bufs=4,
    chunk=512,
    psb=1024,        # PSUM super-chunk (elements; 2 banks) per Act copy
):
    """fp16 k-major compute with the PE (tensor) engine doing the re (and
    selected im) combines as identity-weight matmul accumulations in PSUM,
    Act converting PSUM fp32 -> fp16 out_k, and the output DMA'd k-major.

    Per pair-group (a, a+1), both runs length L = 15-a:
      k1 = v_a * ar[b]      (DVE)   v = ar+ai
      k3'= ai_a * u2[b]     (DVE)   u2 = ai-ar   (negated Gauss operand)
      k2'= -(ar_a * v[b+1]) (Pool STT, scalar=-1)
      re(a,b)  = k1 + k3'   (PE accumulate, Act copy)
      im(a,b)  = k1[1:] + k2'  (PE / DVE / Pool per im_asn)
    The identity stationary matrix ships as an extra ExternalInput."""
    if im_asn is None:
        im_asn = {a: "pe" for a in range(0, K - 1, 2)}
    KP = 16
    D2 = 6
    NP = NOUT + 1
    nc = bass.Bass()
    xr_d = nc.dram_tensor("x_re", [n_c, D], F32, kind="ExternalInput")
    xi_d = nc.dram_tensor("x_im", [n_c, D], F32, kind="ExternalInput")
    id_d = nc.dram_tensor("ident", [P, P], F16, kind="ExternalInput")
    out_d = nc.dram_tensor("out", [P, rt * NOUT], F16, kind="ExternalOutput")

    xr_v = xr_d[:, :].rearrange("(p r) d -> p r d", p=P)
    xi_v = xi_d[:, :].rearrange("(p r) d -> p r d", p=P)

    mul_op = mybir.AluOpType.mult
    add_op = mybir.AluOpType.add
    IOr = IO - 120  # im col offsets within the im triangle

    with TileContext(nc) as tc:
        with (
            tc.tile_pool(name="io", bufs=2) as iop,
            tc.tile_pool(name="km", bufs=2) as kmp,
            tc.tile_pool(name="ident", bufs=1) as idp,
            tc.tile_pool(name="zz", bufs=2) as zzp,
            tc.tile_pool(name="scr", bufs=2) as scp,
            tc.tile_pool(name="k1p", bufs=ring_bufs) as k1p,
            tc.tile_pool(name="k2p", bufs=ring_bufs) as k2p,
            tc.tile_pool(name="k3p", bufs=ring_bufs) as k3p,
            tc.psum_pool(name="ps", bufs=psum_bufs) as psp,
            tc.tile_pool(name="outk", bufs=2) as okp,
        ):
            ident = idp.tile([P, P], F16, tag="I")
            nc.sync.dma_start(out=ident[:], in_=id_d[:, :])

            def load_tile(R, r0):
                """Input DMA + Act transpose-cast for one tile; emitted one
                tile ahead so Act's in-order queue never parks the next
                tile's transposes behind this tile's PSUM copies."""
                xr_s = iop.tile([P, R * D], F32, tag="xr")
                xi_s = iop.tile([P, R * D], F32, tag="xi")
                nc.sync.dma_start(
                    out=xr_s[:].rearrange("p (r d) -> p r d", d=D),
                    in_=xr_v[:, r0 : r0 + R, :],
                )
                nc.sync.dma_start(
                    out=xi_s[:].rearrange("p (r d) -> p r d", d=D),
                    in_=xi_v[:, r0 : r0 + R, :],
                )
                xr_k = kmp.tile([P, D2 * R], F16, tag="xrk")
                xi_k = kmp.tile([P, D2 * R], F16, tag="xik")
                nc.scalar.copy(
                    out=xr_k[:].rearrange("p (d r) -> p d r", d=D2)[:, :D, :],
                    in_=xr_s[:].rearrange("p (r d) -> p r d", d=D).transpose([0, 2, 1]),
                )
                nc.scalar.copy(
                    out=xi_k[:].rearrange("p (d r) -> p d r", d=D2)[:, :D, :],
                    in_=xi_s[:].rearrange("p (r d) -> p r d", d=D).transpose([0, 2, 1]),
                )
                return xr_k, xi_k

            offs = []
            _o = 0
            for _R in tile_rs:
                offs.append(_o)
                _o += _R
            loaded = load_tile(tile_rs[0], offs[0])
            for ti, R in enumerate(tile_rs):
                assert R % 2 == 0
                r0 = offs[ti]
                xr_k, xi_k = loaded
                if ti + 1 < len(tile_rs):
                    loaded = load_tile(tile_rs[ti + 1], offs[ti + 1])
                xr3 = xr_k[:].rearrange("p (d r) -> p d r", d=D2)
                xi3 = xi_k[:].rearrange("p (d r) -> p d r", d=D2)

                ar_t = zzp.tile([P, KP * R], F16, tag="ar")
                ai_t = zzp.tile([P, KP * R], F16, tag="ai")
                ar3 = ar_t[:].rearrange("p (k r) -> p k r", k=KP)
                ai3 = ai_t[:].rearrange("p (k r) -> p k r", k=KP)

                out_k = okp.tile([P, NP * R], F16, tag="outk")
                ok3 = out_k[:].rearrange("p (c r) -> p c r", c=NP)

                v_t = zzp.tile([P, KP * R], F16, tag="vt")
                u2_t = zzp.tile([P, KP * R], F16, tag="ut")
                an_t = zzp.tile([P, KP * R], F16, tag="an")
                v3 = v_t[:].rearrange("p (k r) -> p k r", k=KP)
                an3 = an_t[:].rearrange("p (k r) -> p k r", k=KP)

                def vua(c0, c1):
                    """v = ar+ai, u2 = ai-ar, an = -ar for zz cols [c0, c1) —
                    emitted right after the step-1 block producing them so
                    step-2 work on high columns can start early.  The low
                    (last-produced, least-urgent) u2 chunk runs on Pool to
                    shed DVE load."""
                    sl = slice(c0 * R, c1 * R)
                    nc.vector.tensor_add(out=v_t[:, sl], in0=ar_t[:, sl], in1=ai_t[:, sl])
                    nc.vector.tensor_sub(out=u2_t[:, sl], in0=ai_t[:, sl], in1=ar_t[:, sl])
                    nc.vector.tensor_scalar_mul(an_t[:, sl], ar_t[:, sl], -1.0)

                # ---- step 1, high zz columns first (single a=4, then pair
                # (2,3), then pair (0,1)), v/u/an produced incrementally ----
                a = 4
                bra = xr3[:, a : a + 1, :]
                bia = xi3[:, a : a + 1, :]
                s1 = scp.tile([P, 2 * D * R], F16, tag="s1")
                s2 = scp.tile([P, 2 * D * R], F16, tag="s2")
                s1v = s1[:, :R].unsqueeze(1)
                s2v = s2[:, :R].unsqueeze(1)
                nc.vector.tensor_mul(out=s1v, in0=bra, in1=xr3[:, a : a + 1, :])
                nc.vector.tensor_mul(out=s2v, in0=bia, in1=xi3[:, a : a + 1, :])
                nc.vector.tensor_sub(out=ar3[:, 14:15, :], in0=s1v, in1=s2v)
                s3 = scp.tile([P, 2 * D * R], F16, tag="s3")
                s4 = scp.tile([P, 2 * D * R], F16, tag="s4")
                s3v = s3[:, :R].unsqueeze(1)
                s4v = s4[:, :R].unsqueeze(1)
                nc.vector.tensor_mul(out=s3v, in0=bra, in1=xi3[:, a : a + 1, :])
                nc.vector.tensor_mul(out=s4v, in0=bia, in1=xr3[:, a : a + 1, :])
                nc.vector.tensor_add(out=ai3[:, 14:15, :], in0=s3v, in1=s4v)
                vua(14, KP)  # col 15 pad reads junk ar/ai: never consumed

                # single a=14 mults right away (re col 119 inputs)
                k1s = k1p.tile([P, 2 * K * R], F16, tag="m1")
                k3s = k3p.tile([P, 2 * K * R], F16, tag="m3")
                nc.vector.tensor_mul(
                    out=k1s[:, :R].unsqueeze(1), in0=v3[:, 14:15, :], in1=ar3[:, 14:15, :]
                )
                nc.vector.tensor_mul(
                    out=k3s[:, :R].unsqueeze(1),
                    in0=ai3[:, 14:15, :],
                    in1=u2_t[:].rearrange("p (k r) -> p k r", k=KP)[:, 14:15, :],
                )

                for a in (2, 0):
                    L = D - a
                    b_r = xr3[:, a : a + 2, :].unsqueeze(2).broadcast_to([P, 2, L, R])
                    b_i = xi3[:, a : a + 2, :].unsqueeze(2).broadcast_to([P, 2, L, R])
                    w_r = _ap4(xr_k[:], a, 1, L, R)
                    w_i = _ap4(xi_k[:], a, 1, L, R)
                    s1 = scp.tile([P, 2 * D * R], F16, tag="s1")
                    s2 = scp.tile([P, 2 * D * R], F16, tag="s2")
                    s3 = scp.tile([P, 2 * D * R], F16, tag="s3")
                    s4 = scp.tile([P, 2 * D * R], F16, tag="s4")
                    s1v = s1[:, : 2 * L * R].rearrange("p (j k r) -> p j k r", j=2, k=L)
                    s2v = s2[:, : 2 * L * R].rearrange("p (j k r) -> p j k r", j=2, k=L)
                    s3v = s3[:, : 2 * L * R].rearrange("p (j k r) -> p j k r", j=2, k=L)
                    s4v = s4[:, : 2 * L * R].rearrange("p (j k r) -> p j k r", j=2, k=L)
                    nc.vector.tensor_mul(out=s1v, in0=b_r, in1=w_r)
                    nc.vector.tensor_mul(out=s2v, in0=b_i, in1=w_i)
                    nc.vector.tensor_mul(out=s3v, in0=b_r, in1=w_i)
                    nc.vector.tensor_mul(out=s4v, in0=b_i, in1=w_r)
                    # exact-length combines per group: the paired scratch's
                    # j=1 run has a junk tail column that must not reach zz
                    # (later groups are already written in descending order)
                    def _j0(t):
                        return t[:, : L * R].rearrange("p (k r) -> p k r", k=L)

                    def _j1(t):
                        return t[:, L * R : (2 * L - 1) * R].rearrange(
                            "p (k r) -> p k r", k=L - 1
                        )

                    nc.vector.tensor_sub(
                        out=ar3[:, O1[a] : O1[a] + L, :], in0=_j0(s1), in1=_j0(s2)
                    )
                    nc.vector.tensor_sub(
                        out=ar3[:, O1[a + 1] : O1[a + 1] + L - 1, :],
                        in0=_j1(s1), in1=_j1(s2),
                    )
                    nc.vector.tensor_add(
                        out=ai3[:, O1[a] : O1[a] + L, :], in0=_j0(s3), in1=_j0(s4)
                    )
                    nc.vector.tensor_add(
                        out=ai3[:, O1[a + 1] : O1[a + 1] + L - 1, :],
                        in0=_j1(s3), in1=_j1(s4),
                    )
                    vua(O1[a], O1[a + 2] if a + 2 < D else 14)

                # ---- step 2 mults, pair-grouped, descending (high pairs
                # depend only on high zz cols -> unlock earliest) ----
                pair_tiles = {}
                for a in range(K - 3, -1, -2):
                    L = K - a
                    L2 = L - 1
                    b_v = v3[:, a : a + 2, :].unsqueeze(2).broadcast_to([P, 2, L, R])
                    b_ai = ai3[:, a : a + 2, :].unsqueeze(2).broadcast_to([P, 2, L, R])
                    w_ar = _ap4(ar_t[:], a, 1, L, R)
                    w_u = _ap4(u2_t[:], a, 1, L, R)
                    w_v = _ap4(v_t[:], a + 1, 1, L2, R)
                    k1 = k1p.tile([P, 2 * K * R], F16, tag="m1")
                    k2 = k2p.tile([P, 2 * K * R], F16, tag="m2")
                    k3 = k3p.tile([P, 2 * K * R], F16, tag="m3")
                    k1v = k1[:, : 2 * L * R].rearrange("p (j k r) -> p j k r", j=2, k=L)
                    k3v = k3[:, : 2 * L * R].rearrange("p (j k r) -> p j k r", j=2, k=L)
                    # k2' = (-ar)_a * v[b] on Pool (plain TT mult; the only
                    # tensor op the Pool engine supports on hw)
                    b_an = an3[:, a : a + 2, :].unsqueeze(2).broadcast_to([P, 2, L2, R])
                    k2v = k2[:, : 2 * L2 * R].rearrange("p (j k r) -> p j k r", j=2, k=L2)
                    nc.gpsimd.tensor_mul(out=k2v, in0=b_an, in1=w_v)
                    nc.vector.tensor_mul(out=k1v, in0=b_v, in1=w_ar)
                    nc.vector.tensor_mul(out=k3v, in0=b_ai, in1=w_u)
                    pair_tiles[a] = (k1, k2, k3, L, L2)

                # ---- combines ----
                class _Packer:
                    """Pack PE accumulate pieces destined for a contiguous
                    out_k element range [lo, hi) into a fixed psb grid
                    anchored at hi, so Act copies are one per full psb chunk
                    no matter how pieces (pair regions) are sized.  Pieces
                    may arrive in any order; a chunk's convert-copy fires
                    when its coverage completes."""

                    def __init__(self, lo, hi):
                        self.lo, self.hi = lo, hi
                        self.tiles = {}    # chunk idx -> (psum tile, clo, chi)
                        self.filled = {}   # chunk idx -> covered elements

                    def _chunk(self, ci):
                        if ci not in self.tiles:
                            chi = self.hi - ci * psb
                            clo = max(self.lo, chi - psb)
                            ps_t = psp.tile([P, psb], F32, tag="ps")
                            self.tiles[ci] = (ps_t, clo, chi)
                            self.filled[ci] = 0
                        return self.tiles[ci]

                    def add(self, dst_off, n_el, rhs1, rhs2):
                        """Accumulate rhs1+rhs2 into out_k[dst_off:dst_off+n_el]."""
                        g0 = dst_off
                        g1 = dst_off + n_el
                        while g0 < g1:
                            ci = (self.hi - g0 - 1) // psb
                            ps, clo, chi = self._chunk(ci)
                            fe = min(g1, chi)
                            # matmul sub-chunks within [g0, fe)
                            c0 = g0
                            while c0 < fe:
                                ce = min(c0 + chunk, fe)
                                po = c0 - clo
                                ro = c0 - dst_off
                                nc.tensor.matmul(
                                    out=ps[:, po : po + ce - c0], lhsT=ident[:],
                                    rhs=rhs1[:, ro : ro + ce - c0],
                                    start=True, stop=False,
                                )
                                nc.tensor.matmul(
                                    out=ps[:, po : po + ce - c0], lhsT=ident[:],
                                    rhs=rhs2[:, ro : ro + ce - c0],
                                    start=False, stop=True,
                                )
                                c0 = ce
                            self.filled[ci] += fe - g0
                            if self.filled[ci] == chi - clo:
                                nc.scalar.copy(
                                    out=out_k[:, clo:chi], in_=ps[:, : chi - clo]
                                )
                                del self.tiles[ci]
                            g0 = fe

                    def close(self):
                        assert not self.tiles, f"uncovered psum chunks: {self.tiles}"

                # combines, descending, re/im interleaved per pair: aligned
                # with the descending mult production so every consumer's
                # inputs are the producers' most recent outputs.  The re
                # triangle (cols 0..119) and each contiguous run of pe-im
                # pairs share one packer so Act copies are per psb chunk,
                # not per pair.
                IOx = list(IO) + [NOUT]
                re_pk = _Packer(0, 120 * R)
                pe_pairs = [
                    a for a in range(0, K - 1, 2) if im_asn.get(a, "dve") == "pe"
                ]
                im_pks = {}
                ri = 0
                while ri < len(pe_pairs):
                    rj = ri
                    while rj + 1 < len(pe_pairs) and pe_pairs[rj + 1] == pe_pairs[rj] + 2:
                        rj += 1
                    pk = _Packer(IOx[pe_pairs[ri]] * R, IOx[pe_pairs[rj] + 2] * R)
                    for a in pe_pairs[ri : rj + 1]:
                        im_pks[a] = pk
                    ri = rj + 1

                re_pk.add(119 * R, R, k1s[:, :R], k3s[:, :R])
                for a in range(K - 3, -1, -2):
                    k1, k2, k3, L, L2 = pair_tiles[a]
                    # re pair: k1 + k3' over the contiguous (2L-1) col region
                    # (excludes the pair tile's final junk column)
                    n_re = (2 * L - 1) * R
                    re_pk.add(RO[a] * R, n_re, k1[:, :n_re], k3[:, :n_re])
                    # im pair: k1[1:] + k2' ; groups a (L2 cols) and a+1 (L2-1)
                    asn = im_asn.get(a, "dve")
                    if asn == "pe":
                        pk = im_pks[a]
                        pk.add(IO[a] * R, L2 * R, k1[:, R : L * R], k2[:, : L2 * R])
                        if L2 - 1 > 0:
                            pk.add(
                                IO[a + 1] * R, (L2 - 1) * R,
                                k1[:, L * R + R : L * R + (L - 1) * R],
                                k2[:, L2 * R : L2 * R + (L2 - 1) * R],
                            )
                    elif asn == "pool":
                        nc.gpsimd.tensor_add(
                            out=ok3[:, IO[a] : IO[a] + L2, :],
                            in0=k1[:, R : L * R].rearrange("p (k r) -> p k r", k=L2),
                            in1=k2[:, : L2 * R].rearrange("p (k r) -> p k r", k=L2),
                        )
                        if L2 - 1 > 0:
                            nc.gpsimd.tensor_add(
                                out=ok3[:, IO[a + 1] : IO[a + 1] + L2 - 1, :],
                                in0=k1[:, L * R + R : L * R + (L - 1) * R].rearrange(
                                    "p (k r) -> p k r", k=L2 - 1
                                ),
                                in1=k2[:, L2 * R : L2 * R + (L2 - 1) * R].rearrange(
                                    "p (k r) -> p k r", k=L2 - 1
                                ),
                            )
                    else:
                        nc.vector.tensor_add(
                            out=ok3[:, IO[a] : IO[a] + L2, :],
                            in0=k1[:, R : L * R].rearrange("p (k r) -> p k r", k=L2),
                            in1=k2[:, : L2 * R].rearrange("p (k r) -> p k r", k=L2),
                        )
                        if L2 - 1 > 0:
                            nc.vector.tensor_add(
                                out=ok3[:, IO[a + 1] : IO[a + 1] + L2 - 1, :],
                                in0=k1[:, L * R + R : L * R + (L - 1) * R].rearrange(
                                    "p (k r) -> p k r", k=L2 - 1
                                ),
                                in1=k2[:, L2 * R : L2 * R + (L2 - 1) * R].rearrange(
                                    "p (k r) -> p k r", k=L2 - 1
                                ),
                            )

                re_pk.close()
                for pk in set(im_pks.values()):
                    pk.close()

                # ---- direct k-major fp16 DMA out ----
                nc.sync.dma_start(
                    out=out_d[:, r0 * NOUT : (r0 + R) * NOUT],
                    in_=out_k[:, : NOUT * R],
                )

    _split_excess_waits(nc)
    return nc


_CACHE = {}


def _make_tiles(rt, r_max, first=0, last=0):
    """Split rt rows into tiles of r_max with optional small first/last
    tiles (fast pipeline fill/drain)."""
    rem = rt - first - last
    tiles = ([first] if first else []) + [r_max] * (rem // r_max)
    r = rem % r_max
    if r:
        tiles.append(r)
    if last:
        tiles.append(last)
    assert sum(tiles) == rt and all(t % 2 == 0 for t in tiles)
    return tiles


def _get_program(n):
    """Geometry + compiled program for total row count n.
    Returns (nc, n_c, rt, tile_rs)."""
    key = (n, PRECISION)
    if key in _CACHE:
        return _CACHE[key]
    per_core = -(-n // N_CORES)              # ceil
    rt = -(-per_core // P)                   # rows per partition
    if PRECISION == "fp16_pe":
        rt += rt % 2
        n_c = P * rt
        tile_rs = _make_tiles(rt, 72, first=24, last=32)
        nc = _build_fp16_pe(n_c, rt, tile_rs)
    elif PRECISION == "fp16":
        rt += rt % 2                         # even rt (fp16 4B alignment needs even R only)
        n_c = P * rt
        r_max = 100                          # divisible by 4; best per cost-model sweep
        tile_rs = [r_max] * (rt // r_max)
        if rt % r_max:
            tile_rs.append(rt % r_max)
        nc = _build_fp16_kmout(n_c, rt, tile_rs)
    else:
        n_c = P * rt
        r_max = 64
        tile_rs = [r_max] * (rt // r_max)
        if rt % r_max:
            tile_rs.append(rt % r_max)
        nc = _build(n_c, rt, tile_rs)
    _CACHE[key] = (nc, n_c, rt, tile_rs)
    return _CACHE[key]


def kernel(x_re, x_im, _trace=False):
    x_re = np.ascontiguousarray(np.asarray(x_re), dtype=np.float32)
    x_im = np.ascontiguousarray(np.asarray(x_im), dtype=np.float32)
    n = x_re.shape[0]
    nc, n_c, rt, tile_rs = _get_program(n)
    n_pad = n_c * N_CORES
    if n_pad != n:
        pad = np.zeros((n_pad - n, D), dtype=np.float32)
        xr = np.concatenate([x_re, pad], axis=0)
        xi = np.concatenate([x_im, pad], axis=0)
    else:
        xr, xi = x_re, x_im
    xr_sh = xr.reshape(N_CORES, n_c, D)
    xi_sh = xi.reshape(N_CORES, n_c, D)
    in_maps = [
        {"x_re": np.ascontiguousarray(xr_sh[i]), "x_im": np.ascontiguousarray(xi_sh[i])}
        for i in range(N_CORES)
    ]
    if PRECISION == "fp16_pe":
        ident = np.eye(P, dtype=np.float16)
        for m in in_maps:
            m["ident"] = ident
    res = bass_utils.run_bass_kernel_spmd(
        nc, in_maps, core_ids=list(range(N_CORES)), trace=_trace
    )
    if PRECISION in ("fp16", "fp16_pe"):
        # device output is [P, rt*NOUT] fp16, per tile [col][row-in-tile]
        # (k-major); de-interleave to [n_c, NOUT] and upcast per core.
        out = np.empty((n_pad, NOUT), dtype=np.float32)
        for i, r in enumerate(res.results):
            raw = r["out"]  # [P, rt*NOUT] fp16
            core_out = out[i * n_c : (i + 1) * n_c].reshape(P, rt, NOUT)
            r0 = 0
            for R in tile_rs:
                blk = raw[:, r0 * NOUT : (r0 + R) * NOUT].reshape(P, NOUT, R)
                core_out[:, r0 : r0 + R, :] = blk.transpose(0, 2, 1)
                r0 += R
        out = out[:n]
    else:
        out = np.concatenate([r["out"] for r in res.results], axis=0)[:n]
    if _trace:
        return out, res
    return out

